# revision 1
# baseline (speedup 1.0000x reference)
"""Multi-head self-attention + residual + LayerNorm on 8 Trainium2 NeuronCores.

Problem: B=4, S=2048, D=1024, H=16, d_k=64, fp32.

Sharding: token-parallel, zero collectives. Core c owns batch b=c//2 and a
1024-query-token half of it. Each core recomputes K/V projections for its full
batch (25% redundant FLOPs — cheaper than any collective at this size). The
per-core x^T is rotated on the host so every core's own query tokens sit in
columns 0..1023, keeping the device program identical across cores (softmax
and attn@V are permutation-invariant over the key axis, so rotating K/V rows
together is harmless).

v3 layout (all matmul operands bf16, psum/LN arithmetic f32):
 - x^T stays resident in SBUF all kernel (one 4MB load), so projection
   groups are free-floating: a minimal head computes only what the first
   attention blocks need (kT-ft0/ft1, Q-hp0, all of V for head-group 0);
   every other projection (rest of K0/Q0, all of hg1's K/V/Q) is woven
   into the attention windows' engine slack in deadline order, one
   8-matmul psum group at a time (b0_list / b1_list).
 - Phase B0 (attention for hg0): per (qg, j, kt): two K=64 score matmuls
   packed via tile_position row groups, one Exp on ScalarE (1/sqrt(dk)
   folded into scale, no max-shift: scores ~N(0,1)), then attn@V as eight
   [q=128, 65] matmuls (lhsT = e[k,q], rhs = v_aug[k, dk+1], 65th column
   ones -> row sums land in column 64 for free), plus ~2.5 woven
   projection matmuls per iteration.
 - Normalize: reciprocal of psum column 64 gives per-query (per-partition)
   denominators; one tensor_scalar multiply per q-tile — no partition
   broadcast needed. PE transposes (against a bf16 identity) flip the
   normalized [q, dk] tiles into the o-proj lhsT layout [dk, q].
 - Phase B1 (attention for hg1): same loop; o-proj+residual+LayerNorm
   c_blocks for the finished query group are woven in one matmul per kt.
   Each block's last iteration pre-emits the next block's first two
   scores+exps ahead of the attnV (whose exp-wait overflows PE's 4-deep
   wait queue), so ScalarE never idles across block boundaries.
 - LayerNorm: bn_stats/bn_aggr on DVE; rstd = exp(-0.5*ln(var+eps)) on
   ScalarE — Ln and Exp share one activation table set (pinned at build
   time), so the attention Exp table is loaded exactly once and never
   thrashes (Sqrt's table does not contain Exp).
"""

import numpy as np

import concourse.mybir as mybir
import concourse.tile as tile
from concourse import bacc
from concourse import bass_utils

F32 = mybir.dt.float32
F32R = mybir.dt.float32r
BF16 = mybir.dt.bfloat16

B, S, D, H, DK = 4, 2048, 1024, 16, 64
N_CORES = 8
TOK = (B * S) // N_CORES            # 1024 query tokens per core
NKT = S // 128                      # 16 k-tiles per batch
NFT = D // 128                      # 8 feature tiles
NTG = S // 512                      # 4 token groups per batch
EPS = 1e-5

_CACHE = {}


def build(apply_gb: bool, apply_bias: bool):
    nc = bacc.Bacc("TRN2", target_bir_lowering=False, debug=False,
                   num_devices=N_CORES)
    # The kernel's only ScalarE functions are Exp (attention) and Ln (the
    # LayerNorm rstd = exp(-0.5*ln(var+eps)) path). Both live in the
    # natural_log_exp_and_others table, but the greedy table chooser maps
    # each func to the first table containing it (exp_and_others /
    # natural_log), which forces a 1283ns table reload around every Ln.
    # Emptying the competing sets in the cached tables dict (positions —
    # and thus act_func_set ids — preserved) pins every activation to the
    # shared table, so it is loaded exactly once. Runtime table contents
    # come from the compiler's own act_info.json and are unaffected.
    from concourse.hw_specs import get_activation_tables
    tabs = get_activation_tables(nc.m.arch)
    for name, s in tabs.items():
        if name != "natural_log_exp_and_others":
            s.discard(mybir.ActivationFunctionType.Exp)
            s.discard(mybir.ActivationFunctionType.Ln)
    xT_d = nc.dram_tensor("xT", [D, S], BF16, kind="ExternalInput")
    xmy_d = nc.dram_tensor("xmy", [TOK, D], BF16, kind="ExternalInput")
    wqT_d = nc.dram_tensor("wqT", [D, D], BF16, kind="ExternalInput")
    wkT_d = nc.dram_tensor("wkT", [D, D], BF16, kind="ExternalInput")
    wvT_d = nc.dram_tensor("wvT", [D, D], BF16, kind="ExternalInput")
    woT_d = nc.dram_tensor("woT", [D, D], BF16, kind="ExternalInput")
    ident_d = nc.dram_tensor("ident", [128, 128], BF16, kind="ExternalInput")
    bo_d = nc.dram_tensor("bo", [1, D], F32, kind="ExternalInput")
    gb_d = nc.dram_tensor("gb", [2, D], F32, kind="ExternalInput")
    y_d = nc.dram_tensor("y", [TOK, D], F32, kind="ExternalOutput")

    with tile.TileContext(nc) as tc:
        with (
            tc.tile_pool(name="big", bufs=1) as big,
            tc.tile_pool(name="vpool", bufs=2) as vpool,
            tc.tile_pool(name="xr", bufs=3) as xr,
            tc.tile_pool(name="ev", bufs=4) as ev,
            tc.tile_pool(name="on", bufs=2) as onp,
            tc.tile_pool(name="small", bufs=1) as small,
            tc.tile_pool(name="ln", bufs=2) as lnp,
            tc.tile_pool(name="ps_sc", bufs=2, space="PSUM") as ps_sc,
            tc.tile_pool(name="ps_o", bufs=2, space="PSUM") as ps_o,
            tc.tile_pool(name="ps_mm", bufs=2, space="PSUM") as ps_mm,
        ):
            # persistent operand tiles: both head groups side by side.
            # head pair hp = hg*4 + j lives at index hp; head 2hp in rows
            # 0:64, head 2hp+1 in rows 64:128.
            kT = big.tile([128, NFT, S], BF16, tag="kT")             # 4 MB
            qT = big.tile([128, NFT, TOK], BF16, tag="qT")           # 2 MB
            oT = big.tile([128, NFT, TOK], BF16, tag="oT")           # 2 MB
            wk = big.tile([128, NFT, D], BF16, tag="wk")             # 2 MB
            wv = big.tile([128, NFT, D], BF16, tag="wv")             # 2 MB
            wq = big.tile([128, NFT, D], BF16, tag="wq")             # 2 MB
            ident = big.tile([128, 128], BF16, tag="ident")

            def new_vaug():
                # [k-tile rows, kt, 8 heads x (dk | 1)] with ones in col dk
                va = vpool.tile([128, NKT, 8 * (DK + 1)], BF16, tag="vaug")
                nc.vector.memset(
                    va[:].rearrange("p t (h c) -> p t h c", h=8)[:, :, :, DK:DK + 1],
                    1.0,
                )
                return va

            vaug = [None, None]
            vaug[0] = new_vaug()

            def emit_scores_exp(hp, qg, kt):
                sc = ps_sc.tile([128, 1024], F32, tag="sc")
                nc.tensor.matmul(
                    sc[:, 0:512],
                    kT[0:64, hp, kt * 128:(kt + 1) * 128],
                    qT[0:64, hp, qg * 512:(qg + 1) * 512],
                    start=True, stop=True, tile_position=(0, 0),
                )
                nc.tensor.matmul(
                    sc[:, 512:1024],
                    kT[64:128, hp, kt * 128:(kt + 1) * 128],
                    qT[64:128, hp, qg * 512:(qg + 1) * 512],
                    start=True, stop=True, tile_position=(64, 0),
                )
                e_ab = ev.tile([128, 1024], BF16, tag="exp")
                nc.scalar.activation(
                    out=e_ab[:], in_=sc[:],
                    func=mybir.ActivationFunctionType.Exp,
                    scale=0.125,
                )
                return e_ab

            def emit_attnv(hg, j, kt, e_ab, o_psA, o_psB):
                for hh, o_ps in ((0, o_psA), (1, o_psB)):
                    va = vaug[hg][:, kt, :]
                    for qt in range(4):
                        # one accumulation group per psum bank: start marks
                        # the whole 2KB zero-region pending, so only the
                        # bank's first matmul starts and only its last
                        # stops; the other q-tiles' first writes land on
                        # pending-zero bytes (fresh write)
                        nc.tensor.matmul(
                            o_ps[:, qt * 128:qt * 128 + DK + 1],
                            e_ab[:, hh * 512 + qt * 128:
                                 hh * 512 + (qt + 1) * 128],
                            va[:, (2 * j + hh) * (DK + 1):
                               (2 * j + hh + 1) * (DK + 1)],
                            start=(kt == 0 and qt == 0),
                            stop=(kt == NKT - 1 and qt == 3),
                        )

            # x^T stays resident all kernel (loaded once), so projection
            # groups can be woven into any attention window in deadline
            # order instead of being tied to a streaming token-group sweep
            xall = big.tile([128, NFT, S], BF16, tag="xall")         # 4 MB

            # ---------- projection group emitters ----------
            # "k": (hgx, ft, tg)  K^T head-pair column block for one tg
            # "q": (hp, qgx)      Q^T head pair for one query group (tg==qg)
            # "v": (hgx, kt)      V row block -> v_aug[hgx]
            def groups_gen(worklist):
                for item in worklist:
                    kind = item[0]
                    ps = ps_mm.tile([128, 512], F32, tag="mm512")
                    if kind == "k":
                        _, hgx, ft, tg = item
                        for d in range(NFT):
                            nc.tensor.matmul(
                                ps[:],
                                wk[:, d, hgx * 512 + ft * 128:
                                   hgx * 512 + (ft + 1) * 128],
                                xall[:, d, tg * 512:(tg + 1) * 512],
                                start=(d == 0), stop=(d == NFT - 1),
                            )
                            yield
                        nc.vector.tensor_copy(
                            kT[:, hgx * 4 + ft, tg * 512:(tg + 1) * 512], ps[:])
                    elif kind == "q":
                        _, hp, qgx = item
                        for d in range(NFT):
                            nc.tensor.matmul(
                                ps[:], wq[:, d, hp * 128:(hp + 1) * 128],
                                xall[:, d, qgx * 512:(qgx + 1) * 512],
                                start=(d == 0), stop=(d == NFT - 1),
                            )
                            yield
                        nc.vector.tensor_copy(
                            qT[:, hp, qgx * 512:(qgx + 1) * 512], ps[:])
                    else:
                        _, hgx, kt = item
                        for d in range(NFT):
                            nc.tensor.matmul(
                                ps[:], xall[:, d, kt * 128:(kt + 1) * 128],
                                wv[:, d, hgx * 512:(hgx + 1) * 512],
                                start=(d == 0), stop=(d == NFT - 1),
                            )
                            yield
                        nc.vector.tensor_copy(
                            out=vaug[hgx][:, kt, :]
                            .rearrange("p (h c) -> p h c", h=8)[:, :, 0:DK],
                            in_=ps[:].rearrange("p (h c) -> p h c", h=8),
                        )

            def emit_groups(worklist):
                for _ in groups_gen(worklist):
                    pass

            # ---------- Phase A: minimal head before attention starts ----
            # B0's first two j-blocks need kT-ft0/ft1, Q-hp0 and all of V0;
            # every other projection is woven into the attention windows
            # DMA queue order = first-use order: x tg0 + the K columns the
            # head needs, then the remaining x groups ahead of the bulkier
            # weight halves (the head reads every token group early, but
            # only wk-lo/wv-lo/wq-hp0 before its last group)
            for d in range(NFT):
                # all four token groups of this d-tile together with the
                # head's K columns: the head's K-ft0 groups consume x
                # tg-major at ~1.7us per group, so every tg must land early
                nc.sync.dma_start(
                    xall[:, d, 0:512], xT_d.ap()[d * 128:(d + 1) * 128, 0:512])
                nc.sync.dma_start(wk[:, d, 0:256],
                                  wkT_d.ap()[d * 128:(d + 1) * 128, 0:256])
                nc.sync.dma_start(
                    xall[:, d, 512:1024],
                    xT_d.ap()[d * 128:(d + 1) * 128, 512:1024])
            for tg in range(2, NTG):
                for d in range(NFT):
                    nc.sync.dma_start(
                        xall[:, d, tg * 512:(tg + 1) * 512],
                        xT_d.ap()[d * 128:(d + 1) * 128, tg * 512:(tg + 1) * 512],
                    )
            for d in range(NFT):
                nc.sync.dma_start(wv[:, d, 0:512],
                                  wvT_d.ap()[d * 128:(d + 1) * 128, 0:512])
            for d in range(NFT):
                nc.sync.dma_start(wq[:, d, 0:128],
                                  wqT_d.ap()[d * 128:(d + 1) * 128, 0:128])
            for d in range(NFT):
                nc.sync.dma_start(wk[:, d, 256:1024],
                                  wkT_d.ap()[d * 128:(d + 1) * 128, 256:1024])
            for d in range(NFT):
                nc.sync.dma_start(wq[:, d, 128:1024],
                                  wqT_d.ap()[d * 128:(d + 1) * 128, 128:1024])
            nc.sync.dma_start(ident[:], ident_d.ap())
            for d in range(NFT):
                nc.sync.dma_start(wv[:, d, 512:1024],
                                  wvT_d.ap()[d * 128:(d + 1) * 128, 512:1024])

            vaug[1] = new_vaug()
            emit_groups([("k", 0, 0, tg) for tg in range(NTG)]
                        + [("k", 0, 1, tg) for tg in range(NTG)]
                        + [("q", 0, 0)]
                        + [("v", 0, kt) for kt in range(NKT)])

            # deadline-ordered weave lists. B0 consumes them at ~2.5/kt
            # (pull p lands near iteration p/2.5); B1-qg0 at 2/kt. Each
            # entry is one 8-matmul psum group.
            b0_list = ([("q", 1, 0)]                       # j1 queries
                       + [("k", 0, 2, tg) for tg in range(NTG)]   # j2 keys
                       + [("q", 2, 0)]
                       + [("k", 0, 3, tg) for tg in range(NTG)]   # j3 keys
                       + [("q", 3, 0)]
                       + [("q", hp, 1) for hp in range(4)]  # B0-qg1 queries
                       + [("k", 1, 0, tg) for tg in range(NTG)]   # B1-j0 keys
                       + [("q", 4, 0)]                      # B1-j0 queries
                       + [("v", 1, kt) for kt in range(NKT)]
                       + [("q", 5, 0)])
            b1_list = ([("k", 1, 1, tg) for tg in range(NTG)]
                       + [("k", 1, 2, 0), ("k", 1, 2, 1), ("q", 6, 0),
                          ("k", 1, 2, 2), ("k", 1, 2, 3), ("q", 7, 0)]
                       + [("k", 1, 3, tg) for tg in range(NTG)]
                       + [("q", hp, 1) for hp in range(4, 8)])

            woT_box = [None]

            # ---------- c_block: o-proj + residual + LayerNorm ----------
            if apply_bias:
                bo_bc = small.tile([128, D], F32, tag="bobc")
                nc.sync.dma_start(bo_bc[:],
                                  bo_d.ap()[0:1, :].broadcast_to((128, D)))
            if apply_gb:
                g_bc = small.tile([128, D], F32, tag="gbc")
                b_bc = small.tile([128, D], F32, tag="bbc")
                nc.sync.dma_start(g_bc[:],
                                  gb_d.ap()[0:1, :].broadcast_to((128, D)))
                nc.sync.dma_start(b_bc[:],
                                  gb_d.ap()[1:2, :].broadcast_to((128, D)))

            ys_tags = ("wv", "vaug", "xall")
            eps_t = small.tile([128, 1], F32, tag="eps")
            nc.vector.memset(eps_t[:], EPS)

            def c_block_gen(tt, alt=False):
                """o-proj (16 matmul pulls) then residual+LN+store.

                alt=True draws the psum from the scores pool — free once
                attention has ended — so consecutive tail blocks pipeline
                instead of serializing on ps_mm's two banks."""
                woT = woT_box[0]
                x_t = xr.tile([128, D], BF16, tag="xres")
                nc.sync.dma_start(x_t[:], xmy_d.ap()[tt * 128:(tt + 1) * 128, :])
                if alt:
                    big_ps = ps_sc.tile([128, 1024], F32, tag="sc")
                    pss = [big_ps[:, 0:512], big_ps[:, 512:1024]]
                else:
                    ps_e0 = ps_mm.tile([128, 512], F32, tag="mm512")
                    ps_e1 = ps_mm.tile([128, 512], F32, tag="mm512")
                    pss = [ps_e0[:], ps_e1[:]]
                for eh in range(2):
                    for ft in range(NFT):
                        nc.tensor.matmul(
                            pss[eh], oT[:, ft, tt * 128:(tt + 1) * 128],
                            woT[:, ft, eh * 512:(eh + 1) * 512],
                            start=(ft == 0), stop=(ft == NFT - 1),
                        )
                        yield
                ys_tag = ys_tags[tt % len(ys_tags)]
                if ys_tag == "vaug":
                    y_sb = vpool.tile([128, D], F32, tag=ys_tag)
                else:
                    y_sb = big.tile([128, D], F32, tag=ys_tag)
                for eh in range(2):
                    nc.vector.tensor_add(
                        y_sb[:, eh * 512:(eh + 1) * 512],
                        pss[eh], x_t[:, eh * 512:(eh + 1) * 512],
                    )
                if apply_bias:
                    nc.vector.tensor_add(y_sb[:], y_sb[:], bo_bc[:])
                stats = lnp.tile([128, 2, nc.vector.BN_STATS_DIM], F32, tag="st")
                nc.vector.bn_stats(stats[:, 0, :], y_sb[:, 0:512])
                nc.vector.bn_stats(stats[:, 1, :], y_sb[:, 512:1024])
                mv = lnp.tile([128, nc.vector.BN_AGGR_DIM], F32, tag="mv")
                nc.vector.bn_aggr(mv[:], stats[:])
                lnv = lnp.tile([128, 1], F32, tag="lnv")
                rstd = lnp.tile([128, 1], F32, tag="rstd")
                # rstd = exp(-0.5*ln(var+eps)): Ln and Exp share one ScalarE
                # activation table set, so the attention Exp table never
                # reloads mid-kernel (Sqrt's table does not contain Exp)
                nc.scalar.activation(
                    out=lnv[:], in_=mv[:, 1:2],
                    func=mybir.ActivationFunctionType.Ln,
                    bias=eps_t[:], scale=1.0,
                )
                nc.scalar.activation(
                    out=rstd[:], in_=lnv[:],
                    func=mybir.ActivationFunctionType.Exp,
                    scale=-0.5,
                )
                nc.vector.tensor_scalar(
                    out=y_sb[:], in0=y_sb[:],
                    scalar1=mv[:, 0:1], scalar2=rstd[:],
                    op0=mybir.AluOpType.subtract, op1=mybir.AluOpType.mult,
                )
                if apply_gb:
                    nc.vector.tensor_mul(y_sb[:], y_sb[:], g_bc[:])
                    nc.vector.tensor_add(y_sb[:], y_sb[:], b_bc[:])
                nc.sync.dma_start(y_d.ap()[tt * 128:(tt + 1) * 128, :], y_sb[:])

            def pull(gen, n):
                if gen is None:
                    return None
                for _ in range(n):
                    try:
                        next(gen)
                    except StopIteration:
                        return None
                return gen

            def drain(gen):
                if gen is not None:
                    for _ in gen:
                        pass

            # ---------- Phase B: attention (hg0 then hg1) ----------
            blocks = [(hg, qg, j)
                      for hg in range(2) for qg in range(2) for j in range(4)]
            weave = groups_gen(b0_list)
            weave_n = (3, 2)        # ~2.5 pulls/kt through B0
            lead = {}
            for bi, (hg, qg, j) in enumerate(blocks):
                hp = hg * 4 + j
                if bi == 8:
                    # B1-qg0: late projection groups fill its ScalarE-bound
                    # slack (K1-ft1..3 land one j-block ahead of their use)
                    weave = groups_gen(b1_list)
                    weave_n = (2, 2)
                elif bi >= 12:
                    # c_block for the query group finished one step ago:
                    # tts 0-3 here, the tail covers 4-7
                    weave = c_block_gen(qg * 4 + j - 4)
                    weave_n = (1, 1)
                o_psA = ps_o.tile([128, 512], F32, tag="o")
                o_psB = ps_o.tile([128, 512], F32, tag="o")
                lead_emitted = bi + 1 >= len(blocks)
                for kt in range(NKT):
                    e_ab = lead.pop((hp, qg, kt), None)
                    if e_ab is None:
                        e_ab = emit_scores_exp(hp, qg, kt)
                    if kt == NKT - 1 and not lead_emitted:
                        # pre-emit the next block's first two scores+exps
                        # BEFORE this attnV: attnV(kt15) waits on exp(kt15)
                        # and its 8 matmuls overflow PE's 4-deep wait queue,
                        # so anything after it stalls; emitting the lead
                        # scores first keeps ScalarE fed across the boundary
                        nhg, nqg, nj = blocks[bi + 1]
                        for ktl in range(2):
                            lead[(nhg * 4 + nj, nqg, ktl)] = \
                                emit_scores_exp(nhg * 4 + nj, nqg, ktl)
                        lead_emitted = True
                    emit_attnv(hg, j, kt, e_ab, o_psA, o_psB)
                    if kt == NKT - 1 and lead_emitted and bi + 1 < len(blocks):
                        # two more leads right after the last attnV: four
                        # queued exps (~4.2us) keep ScalarE busy through the
                        # epilogue chain (normalize -> transpose -> evict ->
                        # next block's o_ps rotation, ~3.4us) that delays
                        # the next block's own scores
                        nhg, nqg, nj = blocks[bi + 1]
                        for ktl in (2, 3):
                            if (nhg * 4 + nj, nqg, ktl) not in lead:
                                lead[(nhg * 4 + nj, nqg, ktl)] = \
                                    emit_scores_exp(nhg * 4 + nj, nqg, ktl)
                    weave = pull(weave, weave_n[kt % 2])
                if not lead_emitted:
                    nhg, nqg, nj = blocks[bi + 1]
                    for ktl in range(2):
                        lead[(nhg * 4 + nj, nqg, ktl)] = \
                            emit_scores_exp(nhg * 4 + nj, nqg, ktl)
                if bi == 7 or bi >= 11:
                    # end of B0 / end of B1-qg0: finish weave leftovers;
                    # c_blocks: run the woven block's finalize
                    drain(weave)
                    weave = None
                if bi == 11:
                    # wk's readers (b1_list K groups) are all emitted: its
                    # slot now takes the o-proj weights, first-used half
                    # first; the c_blocks start ~15us later
                    woT_t = big.tile([128, NFT, D], BF16, tag="wk")
                    for d in range(NFT):
                        nc.sync.dma_start(
                            woT_t[:, d, 0:512],
                            woT_d.ap()[d * 128:(d + 1) * 128, 0:512])
                    for d in range(NFT):
                        nc.sync.dma_start(
                            woT_t[:, d, 512:1024],
                            woT_d.ap()[d * 128:(d + 1) * 128, 512:1024])
                    woT_box[0] = woT_t
                # normalize by the softmax sums (psum col 64 of each
                # q-tile), then PE-transpose into o-proj layout
                for hh, o_ps in ((0, o_psA), (1, o_psB)):
                    rec = onp.tile([128, 4], F32, tag="rec")
                    nc.vector.reciprocal(
                        rec[:],
                        o_ps[:].rearrange("p (q c) -> p q c", c=128)[:, :, DK],
                    )
                    o_nrm = onp.tile([128, 4, DK], BF16, tag="onrm")
                    for qt in range(4):
                        nc.vector.tensor_scalar(
                            out=o_nrm[:, qt, :],
                            in0=o_ps[:, qt * 128:qt * 128 + DK],
                            scalar1=rec[:, qt:qt + 1], scalar2=None,
                            op0=mybir.AluOpType.mult,
                        )
                    tr = ps_o.tile([128, 512], BF16, tag="o")
                    for qt in range(4):
                        nc.tensor.transpose(
                            tr[0:DK, qt * 128:(qt + 1) * 128],
                            o_nrm[:, qt, :], ident[:],
                        )
                    nc.vector.tensor_copy(
                        oT[hh * 64:(hh + 1) * 64, hp,
                           qg * 512:(qg + 1) * 512],
                        tr[0:DK, :],
                    )

            # tail: c_blocks for the last query group
            for tt in range(4, 8):
                drain(c_block_gen(tt, alt=bool(tt % 2)))

    nc.compile()
    return nc


def kernel(x, w_q, w_k, w_v, w_o, b_o, ln_g, ln_b):
    import ml_dtypes

    x = np.asarray(x, dtype=np.float32)
    w_q = np.asarray(w_q, dtype=np.float32)
    w_k = np.asarray(w_k, dtype=np.float32)
    w_v = np.asarray(w_v, dtype=np.float32)
    w_o = np.asarray(w_o, dtype=np.float32)
    b_o = np.asarray(b_o, dtype=np.float32)
    ln_g = np.asarray(ln_g, dtype=np.float32)
    ln_b = np.asarray(ln_b, dtype=np.float32)

    apply_gb = not (np.all(ln_g == 1.0) and np.all(ln_b == 0.0))
    apply_bias = bool(np.any(b_o != 0.0))
    key = (apply_gb, apply_bias)
    if key not in _CACHE:
        _CACHE[key] = build(apply_gb, apply_bias)
    nc = _CACHE[key]

    bf16 = ml_dtypes.bfloat16
    wqT = np.ascontiguousarray(w_q.T).astype(bf16)
    wkT = np.ascontiguousarray(w_k.T).astype(bf16)
    wvT = np.ascontiguousarray(w_v.T).astype(bf16)
    woT = np.ascontiguousarray(w_o.T).astype(bf16)
    ident = np.eye(128, dtype=np.float32).astype(bf16)
    gb = np.stack([ln_g, ln_b]).astype(np.float32)
    bo = np.ascontiguousarray(b_o.reshape(1, D))

    in_maps = []
    for c in range(N_CORES):
        b = c // 2
        half = c % 2
        xb = x[b]
        xT = np.ascontiguousarray(xb.T)
        if half == 1:
            xT = np.ascontiguousarray(np.roll(xT, -TOK, axis=1))
        xmy = np.ascontiguousarray(xb[half * TOK:(half + 1) * TOK]).astype(bf16)
        in_maps.append({
            "xT": xT.astype(bf16), "xmy": xmy,
            "wqT": wqT, "wkT": wkT, "wvT": wvT, "woT": woT,
            "ident": ident, "bo": bo, "gb": gb,
        })

    res = bass_utils.run_bass_kernel_spmd(nc, in_maps, core_ids=list(range(N_CORES)))
    y = np.stack([res.results[c]["y"] for c in range(N_CORES)])
    return y.reshape(B, S, D)



# revision 56
# speedup vs baseline: 1.5090x; 1.5090x over previous
"""Multi-head self-attention + residual + LayerNorm on 8 Trainium2 NeuronCores.

Problem: B=4, S=2048, D=1024, H=16, d_k=64, fp32.

Sharding: token-parallel, zero collectives. Core c owns batch b=c//2 and a
1024-query-token half of it (host rotates tokens so own queries are rows
0..1023; softmax/attn@V are permutation-invariant over keys). Each core
recomputes K/V for its full batch.

v9 (256us vs 386us bf16 baseline): fp8 DoubleRow matmuls + engine-balanced
softmax.
 - All projection/score/attnV operands are fp8e4m3 (weights host-scaled x16,
   descale folded into the exp scale and the 16.0 ones-column). DoubleRow
   contracts 256 deep at 0.5 cycles/row: projections cost 1/4, scores 1/2 of
   bf16. Scores put d_k=64 on [32 partitions x 2]; four heads share the
   partition dim via 32-row slots (explicit tile_position (32s, 0)).
 - exp carries bias -4.5 (cancels in softmax, keeps e inside e4m3 range;
   raw scores reach +-9). The work splits across engines per kt-pair:
   ScalarE activation-Exp -> e4m3 (feeds DoubleRow attn@V), DVE Schraudolph
   bit-trick (one tensor_scalar f32->int16, bitcast bf16, feeds mixed
   bf16xfp8 attn@V). The split ratio adapts per phase to DVE's eviction
   load. Ones column gives denominators in psum col 64 for free.
 - Engine schedule: units (head, qg, kt-pair) run software-pipelined one
   unit deep on scores AND one unit deep on attn@V, so PE never blocks on
   the current exp and the two exp engines overlap. Score psums rotate over
   2x[128,1024] plus, once the projection weave drains, the two [128,512]
   ps_mm banks (split exps) - effectively 3-deep.
 - Projections are 4-matmul half-groups on the 2-buffer ps_mm pool
   (ping-pong: group N+1 computes while N evicts). Evictions balance:
   quad0/V-hg0 prep on ScalarE/DVE, later K/Q on DVE, V-hg1 on ScalarE.
   Each group's eviction is emitted before its last yield so woven
   consumers can never be emitted ahead of the data they read.
 - o_nrm: ScalarE Copy(scale=1/den) for head-even, DVE tensor_scalar for
   head-odd, into one [128,(hh,64)] bf16 tile; SBUF->SBUF DMA XBAR
   transpose writes oT directly (no PE transposes, no separate eviction).
 - o-proj/residual/LN stay bf16/f32: o-proj in fp8 fails the 2e-2 error
   budget. LN: bn_stats/aggr + residual adds on DVE, rstd=exp(-.5 ln(var+eps))
   on ScalarE, affine on GPSIMD (woven) or ScalarE Identity (tail, with
   split half stores). Exp/Ln/Copy/Identity pinned to one activation table.

Measured rel err vs f32 reference: 1.17e-2 (gate 2e-2); error budget is
dominated by e4m3 quantization of Q/K/V/e, validated in sim_numerics.py.
"""

import numpy as np

import concourse.mybir as mybir
import concourse.tile as tile
from concourse import bacc
from concourse import bass_utils

F32 = mybir.dt.float32
BF16 = mybir.dt.bfloat16
E4 = mybir.dt.float8e4
I16 = mybir.dt.int16
DR = mybir.MatmulPerfMode.DoubleRow

B, S, D, H, DK = 4, 2048, 1024, 16, 64
N_CORES = 8
TOK = (B * S) // N_CORES            # 1024 query tokens per core
NKT = S // 128                      # 16 k-tiles per batch
NTG = S // 512                      # 4 token groups per batch
EPS = 1e-5
WS = 16.0                           # host weight upscale before fp8 quant
SEFF = 0.125 / (WS * WS)            # exp scale on raw psum scores
EBIAS = -4.5                        # exp bias (cancels in softmax)
LOG2E = 1.4426950408889634
BT_A = float(SEFF * LOG2E * 128.0)  # bit-trick multiplier
BT_B = float((127.0 - 0.0579) * 128.0 + EBIAS * LOG2E * 128.0)

# kt-pairs whose exp runs on DVE (bit-trick); rest on ScalarE
DVE_KTPS = (1, 4, 6)

_CACHE = {}


def build(apply_gb: bool, apply_bias: bool):
    nc = bacc.Bacc("TRN2", target_bir_lowering=False, debug=False,
                   num_devices=N_CORES)
    # Pin every ScalarE function we use (Exp, Ln, Copy) to the one table that
    # holds them all, so the activation table is loaded exactly once.
    from concourse.hw_specs import get_activation_tables
    A = mybir.ActivationFunctionType
    tabs = get_activation_tables(nc.m.arch)
    for name, s in tabs.items():
        if name != "natural_log_exp_and_others":
            s.discard(A.Exp)
            s.discard(A.Ln)
            s.discard(A.Copy)
            s.discard(A.Identity)

    xdr_d = nc.dram_tensor("xdr", [128, 4 * 2 * S], E4, kind="ExternalInput")
    wq_d = nc.dram_tensor("wq", [128, 4 * 2 * 1024], E4, kind="ExternalInput")
    wk_d = nc.dram_tensor("wk", [128, 4 * 2 * 1024], E4, kind="ExternalInput")
    wv_d = nc.dram_tensor("wv", [128, 4 * 2 * 1024], E4, kind="ExternalInput")
    wo_d = nc.dram_tensor("wo", [128, 8 * 1024], BF16, kind="ExternalInput")
    ident_d = nc.dram_tensor("ident", [128, 128], BF16, kind="ExternalInput")
    xmy_d = nc.dram_tensor("xmy", [TOK, D], BF16, kind="ExternalInput")
    bo_d = nc.dram_tensor("bo", [1, D], F32, kind="ExternalInput")
    gb_d = nc.dram_tensor("gb", [2, D], F32, kind="ExternalInput")
    y_d = nc.dram_tensor("y", [TOK, D], F32, kind="ExternalOutput")

    with tile.TileContext(nc) as tc:
        with (
            tc.tile_pool(name="big", bufs=1) as big,
            tc.tile_pool(name="e2p", bufs=6) as e2p,
            tc.tile_pool(name="ebp", bufs=6) as ebp,
            tc.tile_pool(name="onp", bufs=8) as onp,
            tc.tile_pool(name="xr", bufs=4) as xr,
            tc.tile_pool(name="ysb", bufs=4) as ysb,
            tc.tile_pool(name="ln", bufs=6) as lnp,
            tc.tile_pool(name="small", bufs=1) as small,
            tc.tile_pool(name="ps_sc", bufs=2, space="PSUM") as ps_sc,
            tc.tile_pool(name="ps_o", bufs=2, space="PSUM") as ps_o,
            tc.tile_pool(name="ps_mm", bufs=2, space="PSUM") as ps_mm,
        ):
            xdr = big.tile([128, 4, 2, S], E4, tag="xdr")            # 16K/p
            wq = big.tile([128, 4, 2, 1024], E4, tag="wq")           # 8K/p
            wk = big.tile([128, 4, 2, 1024], E4, tag="wk")
            wv = big.tile([128, 4, 2, 1024], E4, tag="wv")
            wo = big.tile([128, 8, 1024], BF16, tag="wo")            # 16K/p
            # K: [quad, tg, i, t]; Q: [quad, qg, i, t]
            kS = big.tile([128, 4, NTG, 2, 512], E4, tag="kS")       # 16K/p
            qS = big.tile([128, 4, 2, 2, 512], E4, tag="qS")         # 8K/p
            # va: [t-part, kt, head, dk+1]; col dk holds 16.0
            va = big.tile([128, NKT, H, DK + 1], E4, tag="va")       # 16.25K/p
            oT = big.tile([128, 8, TOK], BF16, tag="oT")             # 16K/p

            nc.vector.memset(va[:, :, :, DK:DK + 1], WS)
            bias_t = small.tile([128, 1], F32, tag="bias")
            nc.vector.memset(bias_t[:], EBIAS)
            eps_t = small.tile([128, 1], F32, tag="eps")
            nc.vector.memset(eps_t[:], EPS)

            # ---------------- DMA loads (first-use order) ----------------
            # xdr feeds every projection; quad0 K/Q cols + hg0 V cols next.
            xdr_v = xdr_d.ap().rearrange("p (c i t) -> p c i t", c=4, i=2)
            wkv = wk_d.ap().rearrange("p (c i t) -> p c i t", c=4, i=2)
            wqv = wq_d.ap().rearrange("p (c i t) -> p c i t", c=4, i=2)
            wvv = wv_d.ap().rearrange("p (c i t) -> p c i t", c=4, i=2)
            nc.sync.dma_start(xdr[:, :, :, 0:512], xdr_v[:, :, :, 0:512])
            nc.sync.dma_start(wk[:, :, :, 0:256], wkv[:, :, :, 0:256])
            nc.sync.dma_start(wq[:, :, :, 0:256], wqv[:, :, :, 0:256])
            nc.sync.dma_start(wv[:, :, :, 0:512], wvv[:, :, :, 0:512])
            nc.sync.dma_start(xdr[:, :, :, 512:1024], xdr_v[:, :, :, 512:1024])
            nc.sync.dma_start(xdr[:, :, :, 1024:2048], xdr_v[:, :, :, 1024:2048])
            nc.sync.dma_start(wk[:, :, :, 256:1024], wkv[:, :, :, 256:1024])
            nc.sync.dma_start(wq[:, :, :, 256:1024], wqv[:, :, :, 256:1024])
            nc.sync.dma_start(wv[:, :, :, 512:1024], wvv[:, :, :, 512:1024])
            nc.sync.dma_start(
                wo[:], wo_d.ap().rearrange("p (h t) -> p h t", h=8))
            ident = big.tile([128, 128], BF16, tag="ident")
            nc.sync.dma_start(ident[:], ident_d.ap())
            if apply_bias:
                bo_bc = small.tile([128, D], F32, tag="bobc")
                nc.sync.dma_start(bo_bc[:],
                                  bo_d.ap()[0:1, :].broadcast_to((128, D)))
            if apply_gb:
                g_bc = small.tile([128, D], F32, tag="gbc")
                b_bc = small.tile([128, D], F32, tag="bbc")
                nc.sync.dma_start(g_bc[:],
                                  gb_d.ap()[0:1, :].broadcast_to((128, D)))
                nc.sync.dma_start(b_bc[:],
                                  gb_d.ap()[1:2, :].broadcast_to((128, D)))

            # ---------------- projection group emitters ----------------
            # every group is a 4-matmul chain into one [128,512] psum (one
            # bank); the ps_mm pool's two buffers ping-pong so group N+1's
            # matmuls overlap group N's eviction.
            def k_half(quad, tg, ih):
                ps = ps_mm.tile([128, 512], F32, tag="mm")
                for ch in range(4):
                    nc.tensor.matmul(
                        ps[:],
                        wk[:, ch, :, quad * 256 + ih * 128:
                           quad * 256 + (ih + 1) * 128],
                        xdr[:, ch, :, tg * 512:(tg + 1) * 512],
                        start=(ch == 0), stop=(ch == 3), perf_mode=DR)
                    if ch < 3:
                        yield
                if quad == 0:
                    nc.scalar.activation(
                        out=kS[:, quad, tg, ih, :], in_=ps[:], func=A.Copy)
                else:
                    nc.vector.tensor_copy(kS[:, quad, tg, ih, :], ps[:])
                yield

            def q_half(quad, qg, ih):
                ps = ps_mm.tile([128, 512], F32, tag="mm")
                for ch in range(4):
                    nc.tensor.matmul(
                        ps[:],
                        wq[:, ch, :, quad * 256 + ih * 128:
                           quad * 256 + (ih + 1) * 128],
                        xdr[:, ch, :, qg * 512:(qg + 1) * 512],
                        start=(ch == 0), stop=(ch == 3), perf_mode=DR)
                    if ch < 3:
                        yield
                if quad == 0 and qg == 0:
                    nc.scalar.activation(
                        out=qS[:, quad, qg, ih, :], in_=ps[:], func=A.Copy)
                else:
                    nc.vector.tensor_copy(qS[:, quad, qg, ih, :], ps[:])

            def v_group(kt, hg):
                ps = ps_mm.tile([128, 512], F32, tag="mm")
                for ch in range(4):
                    nc.tensor.matmul(
                        ps[:],
                        xdr[:, ch, :, kt * 128:(kt + 1) * 128],
                        wv[:, ch, :, hg * 512:(hg + 1) * 512],
                        start=(ch == 0), stop=(ch == 3), perf_mode=DR)
                    yield
                if hg == 0:
                    nc.vector.tensor_copy(
                        va[:, kt, 0:8, 0:DK],
                        ps[:].rearrange("p (h c) -> p h c", h=8))
                else:
                    nc.scalar.activation(
                        out=va[:, kt, 8:16, 0:DK],
                        in_=ps[:].rearrange("p (h c) -> p h c", h=8),
                        func=A.Copy)

            def groups_gen(worklist):
                for item in worklist:
                    if item[0] == "k":
                        yield from k_half(item[1], item[2], item[3])
                    elif item[0] == "q":
                        yield from q_half(item[1], item[2], item[3])
                    else:
                        yield from v_group(item[1], item[2])

            # ---------------- c_block: o-proj + residual + LN ----------------
            def c_block(tt, alt=False, ln_on_scalar=False, tail=False):
                x_t = xr.tile([128, D], BF16, tag="xres")
                nc.sync.dma_start(x_t[:], xmy_d.ap()[tt * 128:(tt + 1) * 128, :])
                if alt == 2:
                    ps_e0 = ps_o.tile([128, 512], F32, tag="o")
                    ps_e1 = ps_o.tile([128, 512], F32, tag="o")
                    pss = [ps_e0[:], ps_e1[:]]
                elif alt:
                    big_ps = ps_sc.tile([128, 1024], F32, tag="sc")
                    pss = [big_ps[:, 0:512], big_ps[:, 512:1024]]
                else:
                    ps_e0 = ps_mm.tile([128, 512], F32, tag="mm")
                    ps_e1 = ps_mm.tile([128, 512], F32, tag="mm")
                    pss = [ps_e0[:], ps_e1[:]]
                for eh in range(2):
                    for hp in range(8):
                        nc.tensor.matmul(
                            pss[eh], oT[:, hp, tt * 128:(tt + 1) * 128],
                            wo[:, hp, eh * 512:(eh + 1) * 512],
                            start=(hp == 0), stop=(hp == 7 and not tail))
                        yield
                y_sb = ysb.tile([128, D], F32, tag="ysb")
                stats = lnp.tile([128, 2, nc.vector.BN_STATS_DIM], F32, tag="st")
                if tail:
                    # residual add on PE (identity matmul); stats from psum
                    for eh in range(2):
                        nc.tensor.matmul(
                            pss[eh], ident[:],
                            x_t[:, eh * 512:(eh + 1) * 512],
                            start=False, stop=True)
                    for eh in range(2):
                        nc.vector.bn_stats(stats[:, eh, :], pss[eh])
                else:
                    for eh in range(2):
                        nc.vector.tensor_add(
                            y_sb[:, eh * 512:(eh + 1) * 512],
                            pss[eh], x_t[:, eh * 512:(eh + 1) * 512])
                    if apply_bias:
                        nc.vector.tensor_add(y_sb[:], y_sb[:], bo_bc[:])
                    nc.vector.bn_stats(stats[:, 0, :], y_sb[:, 0:512])
                    nc.vector.bn_stats(stats[:, 1, :], y_sb[:, 512:1024])
                mv = lnp.tile([128, nc.vector.BN_AGGR_DIM], F32, tag="mv")
                nc.vector.bn_aggr(mv[:], stats[:])
                lnv = lnp.tile([128, 1], F32, tag="lnv")
                rstd = lnp.tile([128, 1], F32, tag="rstd")
                nc.scalar.activation(
                    out=lnv[:], in_=mv[:, 1:2],
                    func=A.Ln, bias=eps_t[:], scale=1.0)
                nc.scalar.activation(
                    out=rstd[:], in_=lnv[:], func=A.Exp, scale=-0.5)
                if ln_on_scalar:
                    nmu = lnp.tile([128, 1], F32, tag="nmu")
                    nc.vector.tensor_scalar(
                        out=nmu[:], in0=mv[:, 0:1], scalar1=rstd[:],
                        scalar2=-1.0, op0=mybir.AluOpType.mult,
                        op1=mybir.AluOpType.mult)
                    for eh in range(2):
                        nc.scalar.activation(
                            out=y_sb[:, eh * 512:(eh + 1) * 512],
                            in_=y_sb[:, eh * 512:(eh + 1) * 512],
                            func=A.Identity, scale=rstd[:], bias=nmu[:])
                        if not apply_gb:
                            nc.sync.dma_start(
                                y_d.ap()[tt * 128:(tt + 1) * 128,
                                         eh * 512:(eh + 1) * 512],
                                y_sb[:, eh * 512:(eh + 1) * 512])
                else:
                    nc.gpsimd.tensor_scalar(
                        out=y_sb[:], in0=y_sb[:],
                        scalar1=mv[:, 0:1], scalar2=rstd[:],
                        op0=mybir.AluOpType.subtract,
                        op1=mybir.AluOpType.mult)
                if apply_gb:
                    nc.gpsimd.tensor_mul(y_sb[:], y_sb[:], g_bc[:])
                    nc.gpsimd.tensor_add(y_sb[:], y_sb[:], b_bc[:])
                if not ln_on_scalar or apply_gb:
                    nc.sync.dma_start(
                        y_d.ap()[tt * 128:(tt + 1) * 128, :], y_sb[:])

            def pull(gen, n):
                if gen is None:
                    return None
                for _ in range(n):
                    try:
                        next(gen)
                    except StopIteration:
                        return None
                return gen

            def drain(gen):
                if gen is not None:
                    for _ in gen:
                        pass

            # ---------------- prelude projections ----------------
            emit_now = ([("k", 0, 0, ih) for ih in range(2)]
                        + [("k", 0, 1, ih) for ih in range(2)]
                        + [("q", 0, 0, ih) for ih in range(2)]
                        + [("v", kt, 0) for kt in range(4)])
            drain(groups_gen(emit_now))

            # deadline-ordered weave (block h0 pulls 10/unit): V-hg0 paced 2
            # groups/unit just ahead of attn@V, K tg2/tg3 slotted to land
            # before their first scores; then quads 1-3, V-hg1 (before block
            # h8), Q-qg1.
            weave_a = ([("v", 4, 0), ("v", 5, 0)]
                       + [("k", 0, 2, ih) for ih in range(2)]
                       + [("v", 6, 0), ("v", 7, 0), ("v", 8, 0), ("v", 9, 0)]
                       + [("k", 0, 3, ih) for ih in range(2)]
                       + [("v", kt, 0) for kt in range(10, NKT)]
                       + [it for quad in (1, 2, 3) for it in
                          [("k", quad, tg, ih)
                           for tg in range(NTG) for ih in range(2)]
                          + [("q", quad, 0, ih) for ih in range(2)]]
                       + [("v", kt, 1) for kt in range(NKT)]
                       + [("q", quad, 1, ih)
                          for quad in range(4) for ih in range(2)])

            # ---------------- attention blocks ----------------
            # Software-pipelined one unit deep on scores AND on attn@V: PE
            # never waits for the current unit's exp, and a third score slot
            # (the two mm banks) rotates in once the weave is drained so the
            # two exp engines overlap fully.
            o_live = {}
            weave_box = [groups_gen(weave_a)]
            cgen_box = [None]
            cqueue = []

            def wpull(n):
                if weave_box[0] is not None:
                    weave_box[0] = pull(weave_box[0], n)
                    if weave_box[0] is None and cqueue:
                        cgen_box[0] = c_block(cqueue.pop(0))
                elif cgen_box[0] is not None:
                    cgen_box[0] = pull(cgen_box[0], n)
                    if cgen_box[0] is None and cqueue:
                        cgen_box[0] = c_block(cqueue.pop(0))

            units = [(qg, h, ktp)
                     for qg in range(2) for h in range(H)
                     for ktp in range(NKT // 2)]

            def dve_unit(u):
                qg, h, ktp = u
                if qg == 0 and 8 <= h < 12:
                    return ktp in (1, 3, 5, 7)
                return ktp in DVE_KTPS

            def alloc_sc(u):
                free_mm = (weave_box[0] is None and cgen_box[0] is None
                           and not cqueue)
                if free_mm and dve_unit(u):
                    ta = ps_mm.tile([128, 512], F32, tag="mm")
                    tb = ps_mm.tile([128, 512], F32, tag="mm")
                    return (ta, tb)
                sc2 = ps_sc.tile([128, 1024], F32, tag="sc")
                return (sc2,)

            def emit_scores(u, sct):
                qg, h, ktp = u
                quad, sl = h // 4, h % 4
                p0 = 32 * sl
                for j in range(2):
                    kt = 2 * ktp + j
                    if len(sct) == 1:
                        out = sct[0][:, j * 512:(j + 1) * 512]
                    else:
                        out = sct[j][:]
                    nc.tensor.matmul(
                        out,
                        kS[p0:p0 + 32, quad, kt // 4, :,
                           (kt % 4) * 128:(kt % 4 + 1) * 128],
                        qS[p0:p0 + 32, quad, qg, :, :],
                        start=True, stop=True, perf_mode=DR,
                        tile_position=(p0, 0))

            def emit_exp(u, sct):
                qg, h, ktp = u
                if dve_unit(u):
                    et = ebp.tile([128, 2, 512], I16, tag="eb")
                    if len(sct) == 1:
                        nc.vector.tensor_scalar(
                            out=et[:].rearrange("p i t -> p (i t)"),
                            in0=sct[0][:], scalar1=BT_A, scalar2=BT_B,
                            op0=mybir.AluOpType.mult,
                            op1=mybir.AluOpType.add)
                    else:
                        for j in range(2):
                            nc.vector.tensor_scalar(
                                out=et[:, j, :],
                                in0=sct[j][:], scalar1=BT_A, scalar2=BT_B,
                                op0=mybir.AluOpType.mult,
                                op1=mybir.AluOpType.add)
                else:
                    et = e2p.tile([128, 2, 512], E4, tag="e2")
                    if len(sct) == 1:
                        nc.scalar.activation(
                            out=et[:].rearrange("p i t -> p (i t)"),
                            in_=sct[0][:], func=A.Exp,
                            scale=SEFF, bias=bias_t[:])
                    else:
                        for j in range(2):
                            nc.scalar.activation(
                                out=et[:, j, :], in_=sct[j][:], func=A.Exp,
                                scale=SEFF, bias=bias_t[:])
                return et

            def emit_attnv(u, et):
                qg, h, ktp = u
                if ktp == 0:
                    o_ps = ps_o.tile([128, 512], F32, tag="o")
                    o_live[h] = o_ps
                else:
                    o_ps = o_live[h]
                first = ktp == 0
                last_ktp = ktp == NKT // 2 - 1
                if dve_unit(u):
                    for j in range(2):
                        for qt in range(4):
                            nc.tensor.matmul(
                                o_ps[:, qt * 65:qt * 65 + 65],
                                et[:, j, qt * 128:(qt + 1) * 128]
                                .bitcast(BF16),
                                va[:, 2 * ktp + j, h, :],
                                start=(first and j == 0 and qt == 0),
                                stop=(last_ktp and j == 1 and qt == 3))
                else:
                    for qt in range(4):
                        nc.tensor.matmul(
                            o_ps[:, qt * 65:qt * 65 + 65],
                            et[:, :, qt * 128:(qt + 1) * 128],
                            va[:, 2 * ktp:2 * ktp + 2, h, :],
                            start=(first and qt == 0),
                            stop=(last_ktp and qt == 3),
                            perf_mode=DR)
                if last_ktp:
                    finish_block(qg, h)

            def finish_block(qg, h):
                if h % 2 == 1:
                    hp = h // 2
                    opE, opO = o_live.pop(h - 1), o_live.pop(h)
                    rec = lnp.tile([128, 8], F32, tag="rec")
                    for hh, op in ((0, opE), (1, opO)):
                        nc.vector.reciprocal(
                            rec[:, hh * 4:(hh + 1) * 4],
                            op[:, 0:260].rearrange(
                                "p (q c) -> p q c", c=65)[:, :, DK])
                    for qt in range(4):
                        onrm = onp.tile([128, 128], BF16, tag="onrm")
                        nc.scalar.activation(
                            out=onrm[:, 0:64],
                            in_=opE[:, qt * 65:qt * 65 + 64],
                            func=A.Copy, scale=rec[:, qt:qt + 1])
                        nc.vector.tensor_scalar(
                            out=onrm[:, 64:128],
                            in0=opO[:, qt * 65:qt * 65 + 64],
                            scalar1=rec[:, 4 + qt:5 + qt], scalar2=None,
                            op0=mybir.AluOpType.mult)
                        nc.sync.dma_start(
                            oT[:, hp, qg * 512 + qt * 128:
                               qg * 512 + (qt + 1) * 128],
                            onrm[:], transpose=True)
                if h == 0 and qg == 1:
                    cqueue.extend([0, 1, 2, 3])
                    if weave_box[0] is None and cgen_box[0] is None:
                        cgen_box[0] = c_block(cqueue.pop(0))

            sc_next = alloc_sc(units[0])
            emit_scores(units[0], sc_next)
            att_pend = None

            for idx, u in enumerate(units):
                sc2 = sc_next
                if idx + 1 < len(units):
                    sc_next = alloc_sc(units[idx + 1])
                    emit_scores(units[idx + 1], sc_next)
                et = emit_exp(u, sc2)
                half_pull = 4 if idx < 8 else 2
                wpull(half_pull)
                if att_pend is not None:
                    emit_attnv(*att_pend)
                att_pend = (u, et)
                wpull(half_pull)
            emit_attnv(*att_pend)

            # tail: remaining c_blocks (qg0 leftovers + all of qg1)
            drain(weave_box[0])
            drain(cgen_box[0])
            for tt in cqueue:
                drain(c_block(tt))
            for a, b in ((4, 5), (6, 7)):
                ga = c_block(a, ln_on_scalar=True)
                gb_ = c_block(b, alt=True, ln_on_scalar=True)
                while ga is not None or gb_ is not None:
                    ga = pull(ga, 2)
                    gb_ = pull(gb_, 2)

    nc.compile()
    return nc


def _prep_shared(w_q, w_k, w_v, w_o):
    """Host-side weight layouts (shared across cores)."""
    import ml_dtypes
    bf16 = ml_dtypes.bfloat16
    e4 = ml_dtypes.float8_e4m3

    def qk_perm(wT):
        # wT: [c=1024, d-cols=1024] scaled. Column order for quad/ih/slot:
        # col(quad, ih, ptil) = head(4*quad + ptil//32), d = ih*32 + ptil%32
        w = wT.reshape(1024, 16, 64)                      # [c, head, d]
        out = np.empty((1024, 4, 2, 128), np.float32)
        for quad in range(4):
            for ih in range(2):
                for sl in range(4):
                    hsel = 4 * quad + sl
                    out[:, quad, ih, sl * 32:(sl + 1) * 32] = \
                        w[:, hsel, ih * 32:(ih + 1) * 32]
        # rows c -> [ch, i, p]: c = ch*256 + i*128 + p
        out = out.reshape(4, 2, 128, 4, 2, 128)           # ch i p quad ih col
        out = out.transpose(2, 3, 0, 4, 1, 5)             # p quad ch ih i col
        # dram layout [128, ch, i, 1024-cols(quad,ih,128)]
        out = out.transpose(0, 2, 4, 1, 3, 5)             # p ch i quad ih col
        return np.ascontiguousarray(
            out.reshape(128, 4, 2, 1024)).astype(e4).reshape(128, -1)

    def v_perm(wT):
        # plain col order; rows c -> [ch, i, p]
        out = wT.reshape(4, 2, 128, 1024).transpose(2, 0, 1, 3)
        return np.ascontiguousarray(
            out.reshape(128, 4, 2, 1024)).astype(e4).reshape(128, -1)

    wqT = np.ascontiguousarray(w_q.T) * WS
    wkT = np.ascontiguousarray(w_k.T) * WS
    wvT = np.ascontiguousarray(w_v.T) * WS
    # wo tile [p, hp, e] = w_o.T[hp*128 + p, e]
    woT = np.ascontiguousarray(
        w_o.T.reshape(8, 128, 1024).transpose(1, 0, 2)).astype(bf16)
    return {
        "wq": qk_perm(wqT), "wk": qk_perm(wkT), "wv": v_perm(wvT),
        "wo": woT.reshape(128, -1),
    }


def kernel(x, w_q, w_k, w_v, w_o, b_o, ln_g, ln_b):
    import ml_dtypes
    bf16 = ml_dtypes.bfloat16
    e4 = ml_dtypes.float8_e4m3

    x = np.asarray(x, dtype=np.float32)
    w_q = np.asarray(w_q, dtype=np.float32)
    w_k = np.asarray(w_k, dtype=np.float32)
    w_v = np.asarray(w_v, dtype=np.float32)
    w_o = np.asarray(w_o, dtype=np.float32)
    b_o = np.asarray(b_o, dtype=np.float32)
    ln_g = np.asarray(ln_g, dtype=np.float32)
    ln_b = np.asarray(ln_b, dtype=np.float32)

    apply_gb = not (np.all(ln_g == 1.0) and np.all(ln_b == 0.0))
    apply_bias = bool(np.any(b_o != 0.0))
    key = (apply_gb, apply_bias)
    if key not in _CACHE:
        _CACHE[key] = build(apply_gb, apply_bias)
    nc = _CACHE[key]

    shared = _prep_shared(w_q, w_k, w_v, w_o)
    gb = np.stack([ln_g, ln_b]).astype(np.float32)
    ident_np = np.eye(128, dtype=np.float32).astype(bf16)
    bo = np.ascontiguousarray(b_o.reshape(1, D))

    in_maps = []
    for c in range(N_CORES):
        b = c // 2
        half = c % 2
        xb = x[b]
        if half == 1:
            xb = np.roll(xb, -TOK, axis=0)
        # xdr[p, ch, i, t] = xb[t, ch*256 + i*128 + p]
        xdr = xb.T.reshape(4, 2, 128, S).transpose(2, 0, 1, 3)
        xdr = np.ascontiguousarray(xdr).astype(e4).reshape(128, -1)
        xmy = np.ascontiguousarray(xb[0:TOK]).astype(bf16)
        in_maps.append({
            "xdr": xdr, "xmy": xmy, "bo": bo, "gb": gb,
            "ident": ident_np, **shared,
        })

    res = bass_utils.run_bass_kernel_spmd(nc, in_maps,
                                          core_ids=list(range(N_CORES)))
    y = np.stack([res.results[c]["y"] for c in range(N_CORES)])
    return y.reshape(B, S, D)


# revision 64
# speedup vs baseline: 1.5134x; 1.0030x over previous
"""Multi-head self-attention + residual + LayerNorm on 8 Trainium2 NeuronCores.

Problem: B=4, S=2048, D=1024, H=16, d_k=64, fp32.

Sharding: token-parallel, zero collectives. Core c owns batch b=c//2 and a
1024-query-token half of it (host rotates tokens so own queries are rows
0..1023; softmax/attn@V are permutation-invariant over keys). Each core
recomputes K/V for its full batch.

v9 (256us vs 386us bf16 baseline): fp8 DoubleRow matmuls + engine-balanced
softmax.
 - All projection/score/attnV operands are fp8e4m3 (weights host-scaled x16,
   descale folded into the exp scale and the 16.0 ones-column). DoubleRow
   contracts 256 deep at 0.5 cycles/row: projections cost 1/4, scores 1/2 of
   bf16. Scores put d_k=64 on [32 partitions x 2]; four heads share the
   partition dim via 32-row slots (explicit tile_position (32s, 0)).
 - exp carries bias -4.5 (cancels in softmax, keeps e inside e4m3 range;
   raw scores reach +-9). The work splits across engines per kt-pair:
   ScalarE activation-Exp -> e4m3 (feeds DoubleRow attn@V), DVE Schraudolph
   bit-trick (one tensor_scalar f32->int16, bitcast bf16, feeds mixed
   bf16xfp8 attn@V). The split ratio adapts per phase to DVE's eviction
   load. Ones column gives denominators in psum col 64 for free.
 - Engine schedule: units (head, qg, kt-pair) run software-pipelined one
   unit deep on scores AND one unit deep on attn@V, so PE never blocks on
   the current exp and the two exp engines overlap. Score psums rotate over
   2x[128,1024] plus, once the projection weave drains, the two [128,512]
   ps_mm banks (split exps) - effectively 3-deep.
 - Projections are 4-matmul half-groups on the 2-buffer ps_mm pool
   (ping-pong: group N+1 computes while N evicts). Evictions balance:
   quad0/V-hg0 prep on ScalarE/DVE, later K/Q on DVE, V-hg1 on ScalarE.
   Each group's eviction is emitted before its last yield so woven
   consumers can never be emitted ahead of the data they read.
 - o_nrm: ScalarE Copy(scale=1/den) for head-even, DVE tensor_scalar for
   head-odd, into one [128,(hh,64)] bf16 tile; SBUF->SBUF DMA XBAR
   transpose writes oT directly (no PE transposes, no separate eviction).
 - o-proj/residual/LN stay bf16/f32: o-proj in fp8 fails the 2e-2 error
   budget. LN: bn_stats/aggr + residual adds on DVE, rstd=exp(-.5 ln(var+eps))
   on ScalarE, affine on GPSIMD (woven) or ScalarE Identity (tail, with
   split half stores). Exp/Ln/Copy/Identity pinned to one activation table.

Measured rel err vs f32 reference: 1.17e-2 (gate 2e-2); error budget is
dominated by e4m3 quantization of Q/K/V/e, validated in sim_numerics.py.
"""

import numpy as np

import concourse.mybir as mybir
import concourse.tile as tile
from concourse import bacc
from concourse import bass_utils

F32 = mybir.dt.float32
BF16 = mybir.dt.bfloat16
E4 = mybir.dt.float8e4
I16 = mybir.dt.int16
DR = mybir.MatmulPerfMode.DoubleRow

B, S, D, H, DK = 4, 2048, 1024, 16, 64
N_CORES = 8
TOK = (B * S) // N_CORES            # 1024 query tokens per core
NKT = S // 128                      # 16 k-tiles per batch
NTG = S // 512                      # 4 token groups per batch
EPS = 1e-5
WS = 16.0                           # host weight upscale before fp8 quant
SEFF = 0.125 / (WS * WS)            # exp scale on raw psum scores
EBIAS = -4.5                        # exp bias (cancels in softmax)
LOG2E = 1.4426950408889634
BT_A = float(SEFF * LOG2E * 128.0)  # bit-trick multiplier
BT_B = float((127.0 - 0.0579) * 128.0 + EBIAS * LOG2E * 128.0)

# kt-pairs whose exp runs on DVE (bit-trick); rest on ScalarE
DVE_KTPS = (1, 4, 6)

_CACHE = {}


def build(apply_gb: bool, apply_bias: bool):
    nc = bacc.Bacc("TRN2", target_bir_lowering=False, debug=False,
                   num_devices=N_CORES)
    # Pin every ScalarE function we use (Exp, Ln, Copy) to the one table that
    # holds them all, so the activation table is loaded exactly once.
    from concourse.hw_specs import get_activation_tables
    A = mybir.ActivationFunctionType
    tabs = get_activation_tables(nc.m.arch)
    for name, s in tabs.items():
        if name != "natural_log_exp_and_others":
            s.discard(A.Exp)
            s.discard(A.Ln)
            s.discard(A.Copy)
            s.discard(A.Identity)

    xdr_d = nc.dram_tensor("xdr", [128, 4 * 2 * S], E4, kind="ExternalInput")
    wq_d = nc.dram_tensor("wq", [128, 4 * 2 * 1024], E4, kind="ExternalInput")
    wk_d = nc.dram_tensor("wk", [128, 4 * 2 * 1024], E4, kind="ExternalInput")
    wv_d = nc.dram_tensor("wv", [128, 4 * 2 * 1024], E4, kind="ExternalInput")
    wo_d = nc.dram_tensor("wo", [128, 8 * 1024], BF16, kind="ExternalInput")
    ident_d = nc.dram_tensor("ident", [128, 128], BF16, kind="ExternalInput")
    xmy_d = nc.dram_tensor("xmy", [TOK, D], BF16, kind="ExternalInput")
    bo_d = nc.dram_tensor("bo", [1, D], F32, kind="ExternalInput")
    gb_d = nc.dram_tensor("gb", [2, D], F32, kind="ExternalInput")
    y_d = nc.dram_tensor("y", [TOK, D], F32, kind="ExternalOutput")

    with tile.TileContext(nc) as tc:
        with (
            tc.tile_pool(name="big", bufs=1) as big,
            tc.tile_pool(name="e2p", bufs=6) as e2p,
            tc.tile_pool(name="ebp", bufs=6) as ebp,
            tc.tile_pool(name="onp", bufs=8) as onp,
            tc.tile_pool(name="xr", bufs=4) as xr,
            tc.tile_pool(name="ysb", bufs=4) as ysb,
            tc.tile_pool(name="ln", bufs=6) as lnp,
            tc.tile_pool(name="small", bufs=1) as small,
            tc.tile_pool(name="ps_sc", bufs=2, space="PSUM") as ps_sc,
            tc.tile_pool(name="ps_o", bufs=2, space="PSUM") as ps_o,
            tc.tile_pool(name="ps_mm", bufs=2, space="PSUM") as ps_mm,
        ):
            xdr = big.tile([128, 4, 2, S], E4, tag="xdr")            # 16K/p
            wq = big.tile([128, 4, 2, 1024], E4, tag="wq")           # 8K/p
            wk = big.tile([128, 4, 2, 1024], E4, tag="wk")
            wv = big.tile([128, 4, 2, 1024], E4, tag="wv")
            wo = big.tile([128, 8, 1024], BF16, tag="wo")            # 16K/p
            # K: [quad, tg, i, t]; Q: [quad, qg, i, t]
            kS = big.tile([128, 4, NTG, 2, 512], E4, tag="kS")       # 16K/p
            qS = big.tile([128, 4, 2, 2, 512], E4, tag="qS")         # 8K/p
            # va: [t-part, kt, head, dk+1]; col dk holds 16.0
            va = big.tile([128, NKT, H, DK + 1], E4, tag="va")       # 16.25K/p
            oT = big.tile([128, 8, TOK], BF16, tag="oT")             # 16K/p

            nc.vector.memset(va[:, :, :, DK:DK + 1], WS)
            bias_t = small.tile([128, 1], F32, tag="bias")
            nc.vector.memset(bias_t[:], EBIAS)
            eps_t = small.tile([128, 1], F32, tag="eps")
            nc.vector.memset(eps_t[:], EPS)

            # ---------------- DMA loads (first-use order) ----------------
            # xdr feeds every projection; quad0 K/Q cols + hg0 V cols next.
            xdr_v = xdr_d.ap().rearrange("p (c i t) -> p c i t", c=4, i=2)
            wkv = wk_d.ap().rearrange("p (c i t) -> p c i t", c=4, i=2)
            wqv = wq_d.ap().rearrange("p (c i t) -> p c i t", c=4, i=2)
            wvv = wv_d.ap().rearrange("p (c i t) -> p c i t", c=4, i=2)
            nc.sync.dma_start(xdr[:, :, :, 0:512], xdr_v[:, :, :, 0:512])
            nc.sync.dma_start(wk[:, :, :, 0:256], wkv[:, :, :, 0:256])
            nc.sync.dma_start(wq[:, :, :, 0:256], wqv[:, :, :, 0:256])
            nc.sync.dma_start(wv[:, :, :, 0:512], wvv[:, :, :, 0:512])
            nc.sync.dma_start(xdr[:, :, :, 512:1024], xdr_v[:, :, :, 512:1024])
            nc.sync.dma_start(xdr[:, :, :, 1024:2048], xdr_v[:, :, :, 1024:2048])
            nc.sync.dma_start(wk[:, :, :, 256:1024], wkv[:, :, :, 256:1024])
            nc.sync.dma_start(wq[:, :, :, 256:1024], wqv[:, :, :, 256:1024])
            nc.sync.dma_start(wv[:, :, :, 512:1024], wvv[:, :, :, 512:1024])
            nc.sync.dma_start(
                wo[:], wo_d.ap().rearrange("p (h t) -> p h t", h=8))
            ident = big.tile([128, 128], BF16, tag="ident")
            nc.sync.dma_start(ident[:], ident_d.ap())
            if apply_bias:
                bo_bc = small.tile([128, D], F32, tag="bobc")
                nc.sync.dma_start(bo_bc[:],
                                  bo_d.ap()[0:1, :].broadcast_to((128, D)))
            if apply_gb:
                g_bc = small.tile([128, D], F32, tag="gbc")
                b_bc = small.tile([128, D], F32, tag="bbc")
                nc.sync.dma_start(g_bc[:],
                                  gb_d.ap()[0:1, :].broadcast_to((128, D)))
                nc.sync.dma_start(b_bc[:],
                                  gb_d.ap()[1:2, :].broadcast_to((128, D)))

            # ---------------- projection group emitters ----------------
            # every group is a 4-matmul chain into one [128,512] psum (one
            # bank); the ps_mm pool's two buffers ping-pong so group N+1's
            # matmuls overlap group N's eviction.
            def k_half(quad, tg, ih):
                ps = ps_mm.tile([128, 512], F32, tag="mm")
                for ch in range(4):
                    nc.tensor.matmul(
                        ps[:],
                        wk[:, ch, :, quad * 256 + ih * 128:
                           quad * 256 + (ih + 1) * 128],
                        xdr[:, ch, :, tg * 512:(tg + 1) * 512],
                        start=(ch == 0), stop=(ch == 3), perf_mode=DR)
                    if ch < 3:
                        yield
                if quad == 0:
                    nc.scalar.activation(
                        out=kS[:, quad, tg, ih, :], in_=ps[:], func=A.Copy)
                else:
                    nc.vector.tensor_copy(kS[:, quad, tg, ih, :], ps[:])
                yield

            def q_half(quad, qg, ih):
                ps = ps_mm.tile([128, 512], F32, tag="mm")
                for ch in range(4):
                    nc.tensor.matmul(
                        ps[:],
                        wq[:, ch, :, quad * 256 + ih * 128:
                           quad * 256 + (ih + 1) * 128],
                        xdr[:, ch, :, qg * 512:(qg + 1) * 512],
                        start=(ch == 0), stop=(ch == 3), perf_mode=DR)
                    if ch < 3:
                        yield
                if quad == 0 and qg == 0:
                    nc.scalar.activation(
                        out=qS[:, quad, qg, ih, :], in_=ps[:], func=A.Copy)
                else:
                    nc.vector.tensor_copy(qS[:, quad, qg, ih, :], ps[:])

            def v_group(kt, hg):
                ps = ps_mm.tile([128, 512], F32, tag="mm")
                for ch in range(4):
                    nc.tensor.matmul(
                        ps[:],
                        xdr[:, ch, :, kt * 128:(kt + 1) * 128],
                        wv[:, ch, :, hg * 512:(hg + 1) * 512],
                        start=(ch == 0), stop=(ch == 3), perf_mode=DR)
                    yield
                if hg == 0:
                    nc.vector.tensor_copy(
                        va[:, kt, 0:8, 0:DK],
                        ps[:].rearrange("p (h c) -> p h c", h=8))
                else:
                    nc.scalar.activation(
                        out=va[:, kt, 8:16, 0:DK],
                        in_=ps[:].rearrange("p (h c) -> p h c", h=8),
                        func=A.Copy)

            def groups_gen(worklist):
                for item in worklist:
                    if item[0] == "k":
                        yield from k_half(item[1], item[2], item[3])
                    elif item[0] == "q":
                        yield from q_half(item[1], item[2], item[3])
                    else:
                        yield from v_group(item[1], item[2])

            # ---------------- c_block: o-proj + residual + LN ----------------
            def c_block(tt, alt=False, ln_on_scalar=False, tail=False):
                x_t = xr.tile([128, D], BF16, tag="xres")
                nc.sync.dma_start(x_t[:], xmy_d.ap()[tt * 128:(tt + 1) * 128, :])
                if alt == 2:
                    ps_e0 = ps_o.tile([128, 512], F32, tag="o")
                    ps_e1 = ps_o.tile([128, 512], F32, tag="o")
                    pss = [ps_e0[:], ps_e1[:]]
                elif alt:
                    big_ps = ps_sc.tile([128, 1024], F32, tag="sc")
                    pss = [big_ps[:, 0:512], big_ps[:, 512:1024]]
                else:
                    ps_e0 = ps_mm.tile([128, 512], F32, tag="mm")
                    ps_e1 = ps_mm.tile([128, 512], F32, tag="mm")
                    pss = [ps_e0[:], ps_e1[:]]
                for eh in range(2):
                    for hp in range(8):
                        nc.tensor.matmul(
                            pss[eh], oT[:, hp, tt * 128:(tt + 1) * 128],
                            wo[:, hp, eh * 512:(eh + 1) * 512],
                            start=(hp == 0), stop=(hp == 7 and not tail))
                        yield
                y_sb = ysb.tile([128, D], F32, tag="ysb")
                stats = lnp.tile([128, 2, nc.vector.BN_STATS_DIM], F32, tag="st")
                if tail:
                    # residual add on PE (identity matmul); stats from psum
                    for eh in range(2):
                        nc.tensor.matmul(
                            pss[eh], ident[:],
                            x_t[:, eh * 512:(eh + 1) * 512],
                            start=False, stop=True)
                    for eh in range(2):
                        nc.vector.bn_stats(stats[:, eh, :], pss[eh])
                else:
                    for eh in range(2):
                        nc.vector.tensor_add(
                            y_sb[:, eh * 512:(eh + 1) * 512],
                            pss[eh], x_t[:, eh * 512:(eh + 1) * 512])
                    if apply_bias:
                        nc.vector.tensor_add(y_sb[:], y_sb[:], bo_bc[:])
                    nc.vector.bn_stats(stats[:, 0, :], y_sb[:, 0:512])
                    nc.vector.bn_stats(stats[:, 1, :], y_sb[:, 512:1024])
                mv = lnp.tile([128, nc.vector.BN_AGGR_DIM], F32, tag="mv")
                nc.vector.bn_aggr(mv[:], stats[:])
                lnv = lnp.tile([128, 1], F32, tag="lnv")
                rstd = lnp.tile([128, 1], F32, tag="rstd")
                nc.scalar.activation(
                    out=lnv[:], in_=mv[:, 1:2],
                    func=A.Ln, bias=eps_t[:], scale=1.0)
                nc.scalar.activation(
                    out=rstd[:], in_=lnv[:], func=A.Exp, scale=-0.5)
                if ln_on_scalar:
                    nmu = lnp.tile([128, 1], F32, tag="nmu")
                    nc.vector.tensor_scalar(
                        out=nmu[:], in0=mv[:, 0:1], scalar1=rstd[:],
                        scalar2=-1.0, op0=mybir.AluOpType.mult,
                        op1=mybir.AluOpType.mult)
                    for eh in range(2):
                        nc.scalar.activation(
                            out=y_sb[:, eh * 512:(eh + 1) * 512],
                            in_=y_sb[:, eh * 512:(eh + 1) * 512],
                            func=A.Identity, scale=rstd[:], bias=nmu[:])
                        if not apply_gb:
                            nc.sync.dma_start(
                                y_d.ap()[tt * 128:(tt + 1) * 128,
                                         eh * 512:(eh + 1) * 512],
                                y_sb[:, eh * 512:(eh + 1) * 512])
                else:
                    nc.gpsimd.tensor_scalar(
                        out=y_sb[:], in0=y_sb[:],
                        scalar1=mv[:, 0:1], scalar2=rstd[:],
                        op0=mybir.AluOpType.subtract,
                        op1=mybir.AluOpType.mult)
                if apply_gb:
                    nc.gpsimd.tensor_mul(y_sb[:], y_sb[:], g_bc[:])
                    nc.gpsimd.tensor_add(y_sb[:], y_sb[:], b_bc[:])
                if not ln_on_scalar or apply_gb:
                    nc.sync.dma_start(
                        y_d.ap()[tt * 128:(tt + 1) * 128, :], y_sb[:])

            def pull(gen, n):
                if gen is None:
                    return None
                for _ in range(n):
                    try:
                        next(gen)
                    except StopIteration:
                        return None
                return gen

            def drain(gen):
                if gen is not None:
                    for _ in gen:
                        pass

            # ---------------- prelude projections ----------------
            emit_now = ([("k", 0, 0, ih) for ih in range(2)]
                        + [("k", 0, 1, ih) for ih in range(2)]
                        + [("q", 0, 0, ih) for ih in range(2)]
                        + [("v", kt, 0) for kt in range(4)])
            drain(groups_gen(emit_now))

            # deadline-ordered weave (block h0 pulls 10/unit): V-hg0 paced 2
            # groups/unit just ahead of attn@V, K tg2/tg3 slotted to land
            # before their first scores; then quads 1-3, V-hg1 (before block
            # h8), Q-qg1.
            weave_a = ([("v", 4, 0), ("v", 5, 0)]
                       + [("k", 0, 2, ih) for ih in range(2)]
                       + [("v", 6, 0), ("v", 7, 0), ("v", 8, 0), ("v", 9, 0)]
                       + [("k", 0, 3, ih) for ih in range(2)]
                       + [("v", kt, 0) for kt in range(10, NKT)]
                       + [it for quad in (1, 2, 3) for it in
                          [("k", quad, tg, ih)
                           for tg in range(NTG) for ih in range(2)]
                          + [("q", quad, 0, ih) for ih in range(2)]]
                       + [("v", kt, 1) for kt in range(NKT)]
                       + [("q", quad, 1, ih)
                          for quad in range(4) for ih in range(2)])

            # ---------------- attention blocks ----------------
            # Software-pipelined one unit deep on scores AND on attn@V: PE
            # never waits for the current unit's exp, and a third score slot
            # (the two mm banks) rotates in once the weave is drained so the
            # two exp engines overlap fully.
            o_live = {}
            weave_box = [groups_gen(weave_a)]
            cgen_box = [None]
            cqueue = []

            def wpull(n):
                if weave_box[0] is not None:
                    weave_box[0] = pull(weave_box[0], n)
                    if weave_box[0] is None and cqueue:
                        cgen_box[0] = c_block(cqueue.pop(0))
                elif cgen_box[0] is not None:
                    cgen_box[0] = pull(cgen_box[0], n)
                    if cgen_box[0] is None and cqueue:
                        cgen_box[0] = c_block(cqueue.pop(0))

            units = [(qg, h, ktp)
                     for qg in range(2) for h in range(H)
                     for ktp in range(NKT // 2)]

            def dve_unit(u):
                qg, h, ktp = u
                if qg == 0 and 8 <= h < 12:
                    return ktp in (1, 3, 5, 7)
                return ktp in DVE_KTPS

            def alloc_sc(u):
                free_mm = (weave_box[0] is None and cgen_box[0] is None
                           and not cqueue)
                if free_mm and dve_unit(u):
                    ta = ps_mm.tile([128, 512], F32, tag="mm")
                    tb = ps_mm.tile([128, 512], F32, tag="mm")
                    return (ta, tb)
                sc2 = ps_sc.tile([128, 1024], F32, tag="sc")
                return (sc2,)

            def emit_scores(u, sct):
                qg, h, ktp = u
                quad, sl = h // 4, h % 4
                p0 = 32 * sl
                for j in range(2):
                    kt = 2 * ktp + j
                    if len(sct) == 1:
                        out = sct[0][:, j * 512:(j + 1) * 512]
                    else:
                        out = sct[j][:]
                    nc.tensor.matmul(
                        out,
                        kS[p0:p0 + 32, quad, kt // 4, :,
                           (kt % 4) * 128:(kt % 4 + 1) * 128],
                        qS[p0:p0 + 32, quad, qg, :, :],
                        start=True, stop=True, perf_mode=DR,
                        tile_position=(p0, 0))

            def emit_exp(u, sct):
                qg, h, ktp = u
                if dve_unit(u):
                    et = ebp.tile([128, 2, 512], I16, tag="eb")
                    if len(sct) == 1:
                        nc.vector.tensor_scalar(
                            out=et[:].rearrange("p i t -> p (i t)"),
                            in0=sct[0][:], scalar1=BT_A, scalar2=BT_B,
                            op0=mybir.AluOpType.mult,
                            op1=mybir.AluOpType.add)
                    else:
                        for j in range(2):
                            nc.vector.tensor_scalar(
                                out=et[:, j, :],
                                in0=sct[j][:], scalar1=BT_A, scalar2=BT_B,
                                op0=mybir.AluOpType.mult,
                                op1=mybir.AluOpType.add)
                else:
                    et = e2p.tile([128, 2, 512], E4, tag="e2")
                    if len(sct) == 1:
                        nc.scalar.activation(
                            out=et[:].rearrange("p i t -> p (i t)"),
                            in_=sct[0][:], func=A.Exp,
                            scale=SEFF, bias=bias_t[:])
                    else:
                        for j in range(2):
                            nc.scalar.activation(
                                out=et[:, j, :], in_=sct[j][:], func=A.Exp,
                                scale=SEFF, bias=bias_t[:])
                return et

            def emit_attnv(u, et):
                qg, h, ktp = u
                if ktp == 0:
                    o_ps = ps_o.tile([128, 512], F32, tag="o")
                    o_live[h] = o_ps
                else:
                    o_ps = o_live[h]
                first = ktp == 0
                last_ktp = ktp == NKT // 2 - 1
                if dve_unit(u):
                    for j in range(2):
                        for qt in range(4):
                            nc.tensor.matmul(
                                o_ps[:, qt * 65:qt * 65 + 65],
                                et[:, j, qt * 128:(qt + 1) * 128]
                                .bitcast(BF16),
                                va[:, 2 * ktp + j, h, :],
                                start=(first and j == 0 and qt == 0),
                                stop=(last_ktp and j == 1 and qt == 3))
                else:
                    for qt in range(4):
                        nc.tensor.matmul(
                            o_ps[:, qt * 65:qt * 65 + 65],
                            et[:, :, qt * 128:(qt + 1) * 128],
                            va[:, 2 * ktp:2 * ktp + 2, h, :],
                            start=(first and qt == 0),
                            stop=(last_ktp and qt == 3),
                            perf_mode=DR)
                if last_ktp:
                    finish_block(qg, h)

            def finish_block(qg, h):
                if h % 2 == 1:
                    hp = h // 2
                    opE, opO = o_live.pop(h - 1), o_live.pop(h)
                    rec = lnp.tile([128, 8], F32, tag="rec")
                    for hh, op in ((0, opE), (1, opO)):
                        nc.vector.reciprocal(
                            rec[:, hh * 4:(hh + 1) * 4],
                            op[:, 0:260].rearrange(
                                "p (q c) -> p q c", c=65)[:, :, DK])
                    for qt in range(4):
                        onrm = onp.tile([128, 128], BF16, tag="onrm")
                        nc.scalar.activation(
                            out=onrm[:, 0:64],
                            in_=opE[:, qt * 65:qt * 65 + 64],
                            func=A.Copy, scale=rec[:, qt:qt + 1])
                        nc.vector.tensor_scalar(
                            out=onrm[:, 64:128],
                            in0=opO[:, qt * 65:qt * 65 + 64],
                            scalar1=rec[:, 4 + qt:5 + qt], scalar2=None,
                            op0=mybir.AluOpType.mult)
                        nc.sync.dma_start(
                            oT[:, hp, qg * 512 + qt * 128:
                               qg * 512 + (qt + 1) * 128],
                            onrm[:], transpose=True)
                if h == 0 and qg == 1:
                    cqueue.extend([0, 1, 2, 3])
                    if weave_box[0] is None and cgen_box[0] is None:
                        cgen_box[0] = c_block(cqueue.pop(0))

            sc_next = alloc_sc(units[0])
            emit_scores(units[0], sc_next)
            att_pend = None

            for idx, u in enumerate(units):
                sc2 = sc_next
                if idx + 1 < len(units):
                    sc_next = alloc_sc(units[idx + 1])
                    emit_scores(units[idx + 1], sc_next)
                et = emit_exp(u, sc2)
                half_pull = 5 if idx < 8 else 2
                wpull(half_pull)
                if att_pend is not None:
                    emit_attnv(*att_pend)
                att_pend = (u, et)
                wpull(half_pull)
            emit_attnv(*att_pend)

            # tail: remaining c_blocks (qg0 leftovers + all of qg1)
            drain(weave_box[0])
            drain(cgen_box[0])
            for tt in cqueue:
                drain(c_block(tt))
            for a, b in ((4, 5), (6, 7)):
                ga = c_block(a, ln_on_scalar=True)
                gb_ = c_block(b, alt=True, ln_on_scalar=True)
                while ga is not None or gb_ is not None:
                    ga = pull(ga, 2)
                    gb_ = pull(gb_, 2)

    nc.compile()
    return nc


def _prep_shared(w_q, w_k, w_v, w_o):
    """Host-side weight layouts (shared across cores)."""
    import ml_dtypes
    bf16 = ml_dtypes.bfloat16
    e4 = ml_dtypes.float8_e4m3

    def qk_perm(wT):
        # wT: [c=1024, d-cols=1024] scaled. Column order for quad/ih/slot:
        # col(quad, ih, ptil) = head(4*quad + ptil//32), d = ih*32 + ptil%32
        w = wT.reshape(1024, 16, 64)                      # [c, head, d]
        out = np.empty((1024, 4, 2, 128), np.float32)
        for quad in range(4):
            for ih in range(2):
                for sl in range(4):
                    hsel = 4 * quad + sl
                    out[:, quad, ih, sl * 32:(sl + 1) * 32] = \
                        w[:, hsel, ih * 32:(ih + 1) * 32]
        # rows c -> [ch, i, p]: c = ch*256 + i*128 + p
        out = out.reshape(4, 2, 128, 4, 2, 128)           # ch i p quad ih col
        out = out.transpose(2, 3, 0, 4, 1, 5)             # p quad ch ih i col
        # dram layout [128, ch, i, 1024-cols(quad,ih,128)]
        out = out.transpose(0, 2, 4, 1, 3, 5)             # p ch i quad ih col
        return np.ascontiguousarray(
            out.reshape(128, 4, 2, 1024)).astype(e4).reshape(128, -1)

    def v_perm(wT):
        # plain col order; rows c -> [ch, i, p]
        out = wT.reshape(4, 2, 128, 1024).transpose(2, 0, 1, 3)
        return np.ascontiguousarray(
            out.reshape(128, 4, 2, 1024)).astype(e4).reshape(128, -1)

    wqT = np.ascontiguousarray(w_q.T) * WS
    wkT = np.ascontiguousarray(w_k.T) * WS
    wvT = np.ascontiguousarray(w_v.T) * WS
    # wo tile [p, hp, e] = w_o.T[hp*128 + p, e]
    woT = np.ascontiguousarray(
        w_o.T.reshape(8, 128, 1024).transpose(1, 0, 2)).astype(bf16)
    return {
        "wq": qk_perm(wqT), "wk": qk_perm(wkT), "wv": v_perm(wvT),
        "wo": woT.reshape(128, -1),
    }


def kernel(x, w_q, w_k, w_v, w_o, b_o, ln_g, ln_b):
    import ml_dtypes
    bf16 = ml_dtypes.bfloat16
    e4 = ml_dtypes.float8_e4m3

    x = np.asarray(x, dtype=np.float32)
    w_q = np.asarray(w_q, dtype=np.float32)
    w_k = np.asarray(w_k, dtype=np.float32)
    w_v = np.asarray(w_v, dtype=np.float32)
    w_o = np.asarray(w_o, dtype=np.float32)
    b_o = np.asarray(b_o, dtype=np.float32)
    ln_g = np.asarray(ln_g, dtype=np.float32)
    ln_b = np.asarray(ln_b, dtype=np.float32)

    apply_gb = not (np.all(ln_g == 1.0) and np.all(ln_b == 0.0))
    apply_bias = bool(np.any(b_o != 0.0))
    key = (apply_gb, apply_bias)
    if key not in _CACHE:
        _CACHE[key] = build(apply_gb, apply_bias)
    nc = _CACHE[key]

    shared = _prep_shared(w_q, w_k, w_v, w_o)
    gb = np.stack([ln_g, ln_b]).astype(np.float32)
    ident_np = np.eye(128, dtype=np.float32).astype(bf16)
    bo = np.ascontiguousarray(b_o.reshape(1, D))

    in_maps = []
    for c in range(N_CORES):
        b = c // 2
        half = c % 2
        xb = x[b]
        if half == 1:
            xb = np.roll(xb, -TOK, axis=0)
        # xdr[p, ch, i, t] = xb[t, ch*256 + i*128 + p]
        xdr = xb.T.reshape(4, 2, 128, S).transpose(2, 0, 1, 3)
        xdr = np.ascontiguousarray(xdr).astype(e4).reshape(128, -1)
        xmy = np.ascontiguousarray(xb[0:TOK]).astype(bf16)
        in_maps.append({
            "xdr": xdr, "xmy": xmy, "bo": bo, "gb": gb,
            "ident": ident_np, **shared,
        })

    res = bass_utils.run_bass_kernel_spmd(nc, in_maps,
                                          core_ids=list(range(N_CORES)))
    y = np.stack([res.results[c]["y"] for c in range(N_CORES)])
    return y.reshape(B, S, D)


# revision 67
# speedup vs baseline: 1.5148x; 1.0009x over previous
"""Multi-head self-attention + residual + LayerNorm on 8 Trainium2 NeuronCores.

Problem: B=4, S=2048, D=1024, H=16, d_k=64, fp32.

Sharding: token-parallel, zero collectives. Core c owns batch b=c//2 and a
1024-query-token half of it (host rotates tokens so own queries are rows
0..1023; softmax/attn@V are permutation-invariant over keys). Each core
recomputes K/V for its full batch.

v9 (256us vs 386us bf16 baseline): fp8 DoubleRow matmuls + engine-balanced
softmax.
 - All projection/score/attnV operands are fp8e4m3 (weights host-scaled x16,
   descale folded into the exp scale and the 16.0 ones-column). DoubleRow
   contracts 256 deep at 0.5 cycles/row: projections cost 1/4, scores 1/2 of
   bf16. Scores put d_k=64 on [32 partitions x 2]; four heads share the
   partition dim via 32-row slots (explicit tile_position (32s, 0)).
 - exp carries bias -4.5 (cancels in softmax, keeps e inside e4m3 range;
   raw scores reach +-9). The work splits across engines per kt-pair:
   ScalarE activation-Exp -> e4m3 (feeds DoubleRow attn@V), DVE Schraudolph
   bit-trick (one tensor_scalar f32->int16, bitcast bf16, feeds mixed
   bf16xfp8 attn@V). The split ratio adapts per phase to DVE's eviction
   load. Ones column gives denominators in psum col 64 for free.
 - Engine schedule: units (head, qg, kt-pair) run software-pipelined one
   unit deep on scores AND one unit deep on attn@V, so PE never blocks on
   the current exp and the two exp engines overlap. Score psums rotate over
   2x[128,1024] plus, once the projection weave drains, the two [128,512]
   ps_mm banks (split exps) - effectively 3-deep.
 - Projections are 4-matmul half-groups on the 2-buffer ps_mm pool
   (ping-pong: group N+1 computes while N evicts). Evictions balance:
   quad0/V-hg0 prep on ScalarE/DVE, later K/Q on DVE, V-hg1 on ScalarE.
   Each group's eviction is emitted before its last yield so woven
   consumers can never be emitted ahead of the data they read.
 - o_nrm: ScalarE Copy(scale=1/den) for head-even, DVE tensor_scalar for
   head-odd, into one [128,(hh,64)] bf16 tile; SBUF->SBUF DMA XBAR
   transpose writes oT directly (no PE transposes, no separate eviction).
 - o-proj/residual/LN stay bf16/f32: o-proj in fp8 fails the 2e-2 error
   budget. LN: bn_stats/aggr + residual adds on DVE, rstd=exp(-.5 ln(var+eps))
   on ScalarE, affine on GPSIMD (woven) or ScalarE Identity (tail, with
   split half stores). Exp/Ln/Copy/Identity pinned to one activation table.

Measured rel err vs f32 reference: 1.17e-2 (gate 2e-2); error budget is
dominated by e4m3 quantization of Q/K/V/e, validated in sim_numerics.py.
"""

import numpy as np

import concourse.mybir as mybir
import concourse.tile as tile
from concourse import bacc
from concourse import bass_utils

F32 = mybir.dt.float32
BF16 = mybir.dt.bfloat16
E4 = mybir.dt.float8e4
I16 = mybir.dt.int16
DR = mybir.MatmulPerfMode.DoubleRow

B, S, D, H, DK = 4, 2048, 1024, 16, 64
N_CORES = 8
TOK = (B * S) // N_CORES            # 1024 query tokens per core
NKT = S // 128                      # 16 k-tiles per batch
NTG = S // 512                      # 4 token groups per batch
EPS = 1e-5
WS = 16.0                           # host weight upscale before fp8 quant
SEFF = 0.125 / (WS * WS)            # exp scale on raw psum scores
EBIAS = -4.5                        # exp bias (cancels in softmax)
LOG2E = 1.4426950408889634
BT_A = float(SEFF * LOG2E * 128.0)  # bit-trick multiplier
BT_B = float((127.0 - 0.0579) * 128.0 + EBIAS * LOG2E * 128.0)

# kt-pairs whose exp runs on DVE (bit-trick); rest on ScalarE
DVE_KTPS = (2, 4, 6)

_CACHE = {}


def build(apply_gb: bool, apply_bias: bool):
    nc = bacc.Bacc("TRN2", target_bir_lowering=False, debug=False,
                   num_devices=N_CORES)
    # Pin every ScalarE function we use (Exp, Ln, Copy) to the one table that
    # holds them all, so the activation table is loaded exactly once.
    from concourse.hw_specs import get_activation_tables
    A = mybir.ActivationFunctionType
    tabs = get_activation_tables(nc.m.arch)
    for name, s in tabs.items():
        if name != "natural_log_exp_and_others":
            s.discard(A.Exp)
            s.discard(A.Ln)
            s.discard(A.Copy)
            s.discard(A.Identity)

    xdr_d = nc.dram_tensor("xdr", [128, 4 * 2 * S], E4, kind="ExternalInput")
    wq_d = nc.dram_tensor("wq", [128, 4 * 2 * 1024], E4, kind="ExternalInput")
    wk_d = nc.dram_tensor("wk", [128, 4 * 2 * 1024], E4, kind="ExternalInput")
    wv_d = nc.dram_tensor("wv", [128, 4 * 2 * 1024], E4, kind="ExternalInput")
    wo_d = nc.dram_tensor("wo", [128, 8 * 1024], BF16, kind="ExternalInput")
    ident_d = nc.dram_tensor("ident", [128, 128], BF16, kind="ExternalInput")
    xmy_d = nc.dram_tensor("xmy", [TOK, D], BF16, kind="ExternalInput")
    bo_d = nc.dram_tensor("bo", [1, D], F32, kind="ExternalInput")
    gb_d = nc.dram_tensor("gb", [2, D], F32, kind="ExternalInput")
    y_d = nc.dram_tensor("y", [TOK, D], F32, kind="ExternalOutput")

    with tile.TileContext(nc) as tc:
        with (
            tc.tile_pool(name="big", bufs=1) as big,
            tc.tile_pool(name="e2p", bufs=6) as e2p,
            tc.tile_pool(name="ebp", bufs=6) as ebp,
            tc.tile_pool(name="onp", bufs=8) as onp,
            tc.tile_pool(name="xr", bufs=4) as xr,
            tc.tile_pool(name="ysb", bufs=4) as ysb,
            tc.tile_pool(name="ln", bufs=6) as lnp,
            tc.tile_pool(name="small", bufs=1) as small,
            tc.tile_pool(name="ps_sc", bufs=2, space="PSUM") as ps_sc,
            tc.tile_pool(name="ps_o", bufs=2, space="PSUM") as ps_o,
            tc.tile_pool(name="ps_mm", bufs=2, space="PSUM") as ps_mm,
        ):
            xdr = big.tile([128, 4, 2, S], E4, tag="xdr")            # 16K/p
            wq = big.tile([128, 4, 2, 1024], E4, tag="wq")           # 8K/p
            wk = big.tile([128, 4, 2, 1024], E4, tag="wk")
            wv = big.tile([128, 4, 2, 1024], E4, tag="wv")
            wo = big.tile([128, 8, 1024], BF16, tag="wo")            # 16K/p
            # K: [quad, tg, i, t]; Q: [quad, qg, i, t]
            kS = big.tile([128, 4, NTG, 2, 512], E4, tag="kS")       # 16K/p
            qS = big.tile([128, 4, 2, 2, 512], E4, tag="qS")         # 8K/p
            # va: [t-part, kt, head, dk+1]; col dk holds 16.0
            va = big.tile([128, NKT, H, DK + 1], E4, tag="va")       # 16.25K/p
            oT = big.tile([128, 8, TOK], BF16, tag="oT")             # 16K/p

            nc.vector.memset(va[:, :, :, DK:DK + 1], WS)
            bias_t = small.tile([128, 1], F32, tag="bias")
            nc.vector.memset(bias_t[:], EBIAS)
            eps_t = small.tile([128, 1], F32, tag="eps")
            nc.vector.memset(eps_t[:], EPS)

            # ---------------- DMA loads (first-use order) ----------------
            # xdr feeds every projection; quad0 K/Q cols + hg0 V cols next.
            xdr_v = xdr_d.ap().rearrange("p (c i t) -> p c i t", c=4, i=2)
            wkv = wk_d.ap().rearrange("p (c i t) -> p c i t", c=4, i=2)
            wqv = wq_d.ap().rearrange("p (c i t) -> p c i t", c=4, i=2)
            wvv = wv_d.ap().rearrange("p (c i t) -> p c i t", c=4, i=2)
            nc.sync.dma_start(xdr[:, :, :, 0:512], xdr_v[:, :, :, 0:512])
            nc.sync.dma_start(wk[:, :, :, 0:256], wkv[:, :, :, 0:256])
            nc.sync.dma_start(wq[:, :, :, 0:256], wqv[:, :, :, 0:256])
            nc.sync.dma_start(wv[:, :, :, 0:512], wvv[:, :, :, 0:512])
            nc.sync.dma_start(xdr[:, :, :, 512:1024], xdr_v[:, :, :, 512:1024])
            nc.sync.dma_start(xdr[:, :, :, 1024:2048], xdr_v[:, :, :, 1024:2048])
            nc.sync.dma_start(wk[:, :, :, 256:1024], wkv[:, :, :, 256:1024])
            nc.sync.dma_start(wq[:, :, :, 256:1024], wqv[:, :, :, 256:1024])
            nc.sync.dma_start(wv[:, :, :, 512:1024], wvv[:, :, :, 512:1024])
            nc.sync.dma_start(
                wo[:], wo_d.ap().rearrange("p (h t) -> p h t", h=8))
            ident = big.tile([128, 128], BF16, tag="ident")
            nc.sync.dma_start(ident[:], ident_d.ap())
            if apply_bias:
                bo_bc = small.tile([128, D], F32, tag="bobc")
                nc.sync.dma_start(bo_bc[:],
                                  bo_d.ap()[0:1, :].broadcast_to((128, D)))
            if apply_gb:
                g_bc = small.tile([128, D], F32, tag="gbc")
                b_bc = small.tile([128, D], F32, tag="bbc")
                nc.sync.dma_start(g_bc[:],
                                  gb_d.ap()[0:1, :].broadcast_to((128, D)))
                nc.sync.dma_start(b_bc[:],
                                  gb_d.ap()[1:2, :].broadcast_to((128, D)))

            # ---------------- projection group emitters ----------------
            # every group is a 4-matmul chain into one [128,512] psum (one
            # bank); the ps_mm pool's two buffers ping-pong so group N+1's
            # matmuls overlap group N's eviction.
            def k_half(quad, tg, ih):
                ps = ps_mm.tile([128, 512], F32, tag="mm")
                for ch in range(4):
                    nc.tensor.matmul(
                        ps[:],
                        wk[:, ch, :, quad * 256 + ih * 128:
                           quad * 256 + (ih + 1) * 128],
                        xdr[:, ch, :, tg * 512:(tg + 1) * 512],
                        start=(ch == 0), stop=(ch == 3), perf_mode=DR)
                    if ch < 3:
                        yield
                if quad == 0:
                    nc.scalar.activation(
                        out=kS[:, quad, tg, ih, :], in_=ps[:], func=A.Copy)
                else:
                    nc.vector.tensor_copy(kS[:, quad, tg, ih, :], ps[:])
                yield

            def q_half(quad, qg, ih):
                ps = ps_mm.tile([128, 512], F32, tag="mm")
                for ch in range(4):
                    nc.tensor.matmul(
                        ps[:],
                        wq[:, ch, :, quad * 256 + ih * 128:
                           quad * 256 + (ih + 1) * 128],
                        xdr[:, ch, :, qg * 512:(qg + 1) * 512],
                        start=(ch == 0), stop=(ch == 3), perf_mode=DR)
                    if ch < 3:
                        yield
                if quad == 0 and qg == 0:
                    nc.scalar.activation(
                        out=qS[:, quad, qg, ih, :], in_=ps[:], func=A.Copy)
                else:
                    nc.vector.tensor_copy(qS[:, quad, qg, ih, :], ps[:])

            def v_group(kt, hg):
                ps = ps_mm.tile([128, 512], F32, tag="mm")
                for ch in range(4):
                    nc.tensor.matmul(
                        ps[:],
                        xdr[:, ch, :, kt * 128:(kt + 1) * 128],
                        wv[:, ch, :, hg * 512:(hg + 1) * 512],
                        start=(ch == 0), stop=(ch == 3), perf_mode=DR)
                    yield
                if hg == 0:
                    nc.vector.tensor_copy(
                        va[:, kt, 0:8, 0:DK],
                        ps[:].rearrange("p (h c) -> p h c", h=8))
                else:
                    nc.scalar.activation(
                        out=va[:, kt, 8:16, 0:DK],
                        in_=ps[:].rearrange("p (h c) -> p h c", h=8),
                        func=A.Copy)

            def groups_gen(worklist):
                for item in worklist:
                    if item[0] == "k":
                        yield from k_half(item[1], item[2], item[3])
                    elif item[0] == "q":
                        yield from q_half(item[1], item[2], item[3])
                    else:
                        yield from v_group(item[1], item[2])

            # ---------------- c_block: o-proj + residual + LN ----------------
            def c_block(tt, alt=False, ln_on_scalar=False, tail=False):
                x_t = xr.tile([128, D], BF16, tag="xres")
                nc.sync.dma_start(x_t[:], xmy_d.ap()[tt * 128:(tt + 1) * 128, :])
                if alt == 2:
                    ps_e0 = ps_o.tile([128, 512], F32, tag="o")
                    ps_e1 = ps_o.tile([128, 512], F32, tag="o")
                    pss = [ps_e0[:], ps_e1[:]]
                elif alt:
                    big_ps = ps_sc.tile([128, 1024], F32, tag="sc")
                    pss = [big_ps[:, 0:512], big_ps[:, 512:1024]]
                else:
                    ps_e0 = ps_mm.tile([128, 512], F32, tag="mm")
                    ps_e1 = ps_mm.tile([128, 512], F32, tag="mm")
                    pss = [ps_e0[:], ps_e1[:]]
                for eh in range(2):
                    for hp in range(8):
                        nc.tensor.matmul(
                            pss[eh], oT[:, hp, tt * 128:(tt + 1) * 128],
                            wo[:, hp, eh * 512:(eh + 1) * 512],
                            start=(hp == 0), stop=(hp == 7 and not tail))
                        yield
                y_sb = ysb.tile([128, D], F32, tag="ysb")
                stats = lnp.tile([128, 2, nc.vector.BN_STATS_DIM], F32, tag="st")
                if tail:
                    # residual add on PE (identity matmul); stats from psum
                    for eh in range(2):
                        nc.tensor.matmul(
                            pss[eh], ident[:],
                            x_t[:, eh * 512:(eh + 1) * 512],
                            start=False, stop=True)
                    for eh in range(2):
                        nc.vector.bn_stats(stats[:, eh, :], pss[eh])
                else:
                    for eh in range(2):
                        nc.vector.tensor_add(
                            y_sb[:, eh * 512:(eh + 1) * 512],
                            pss[eh], x_t[:, eh * 512:(eh + 1) * 512])
                    if apply_bias:
                        nc.vector.tensor_add(y_sb[:], y_sb[:], bo_bc[:])
                    nc.vector.bn_stats(stats[:, 0, :], y_sb[:, 0:512])
                    nc.vector.bn_stats(stats[:, 1, :], y_sb[:, 512:1024])
                mv = lnp.tile([128, nc.vector.BN_AGGR_DIM], F32, tag="mv")
                nc.vector.bn_aggr(mv[:], stats[:])
                lnv = lnp.tile([128, 1], F32, tag="lnv")
                rstd = lnp.tile([128, 1], F32, tag="rstd")
                nc.scalar.activation(
                    out=lnv[:], in_=mv[:, 1:2],
                    func=A.Ln, bias=eps_t[:], scale=1.0)
                nc.scalar.activation(
                    out=rstd[:], in_=lnv[:], func=A.Exp, scale=-0.5)
                if ln_on_scalar:
                    nmu = lnp.tile([128, 1], F32, tag="nmu")
                    nc.vector.tensor_scalar(
                        out=nmu[:], in0=mv[:, 0:1], scalar1=rstd[:],
                        scalar2=-1.0, op0=mybir.AluOpType.mult,
                        op1=mybir.AluOpType.mult)
                    for eh in range(2):
                        nc.scalar.activation(
                            out=y_sb[:, eh * 512:(eh + 1) * 512],
                            in_=y_sb[:, eh * 512:(eh + 1) * 512],
                            func=A.Identity, scale=rstd[:], bias=nmu[:])
                        if not apply_gb:
                            nc.sync.dma_start(
                                y_d.ap()[tt * 128:(tt + 1) * 128,
                                         eh * 512:(eh + 1) * 512],
                                y_sb[:, eh * 512:(eh + 1) * 512])
                else:
                    nc.gpsimd.tensor_scalar(
                        out=y_sb[:], in0=y_sb[:],
                        scalar1=mv[:, 0:1], scalar2=rstd[:],
                        op0=mybir.AluOpType.subtract,
                        op1=mybir.AluOpType.mult)
                if apply_gb:
                    nc.gpsimd.tensor_mul(y_sb[:], y_sb[:], g_bc[:])
                    nc.gpsimd.tensor_add(y_sb[:], y_sb[:], b_bc[:])
                if not ln_on_scalar or apply_gb:
                    nc.sync.dma_start(
                        y_d.ap()[tt * 128:(tt + 1) * 128, :], y_sb[:])

            def pull(gen, n):
                if gen is None:
                    return None
                for _ in range(n):
                    try:
                        next(gen)
                    except StopIteration:
                        return None
                return gen

            def drain(gen):
                if gen is not None:
                    for _ in gen:
                        pass

            # ---------------- prelude projections ----------------
            emit_now = ([("k", 0, 0, ih) for ih in range(2)]
                        + [("k", 0, 1, ih) for ih in range(2)]
                        + [("q", 0, 0, ih) for ih in range(2)]
                        + [("v", kt, 0) for kt in range(4)])
            drain(groups_gen(emit_now))

            # deadline-ordered weave (block h0 pulls 10/unit): V-hg0 paced 2
            # groups/unit just ahead of attn@V, K tg2/tg3 slotted to land
            # before their first scores; then quads 1-3, V-hg1 (before block
            # h8), Q-qg1.
            weave_a = ([("v", 4, 0), ("v", 5, 0)]
                       + [("k", 0, 2, ih) for ih in range(2)]
                       + [("v", 6, 0), ("v", 7, 0), ("v", 8, 0), ("v", 9, 0)]
                       + [("k", 0, 3, ih) for ih in range(2)]
                       + [("v", kt, 0) for kt in range(10, NKT)]
                       + [it for quad in (1, 2, 3) for it in
                          [("k", quad, tg, ih)
                           for tg in range(NTG) for ih in range(2)]
                          + [("q", quad, 0, ih) for ih in range(2)]]
                       + [("v", kt, 1) for kt in range(NKT)]
                       + [("q", quad, 1, ih)
                          for quad in range(4) for ih in range(2)])

            # ---------------- attention blocks ----------------
            # Software-pipelined one unit deep on scores AND on attn@V: PE
            # never waits for the current unit's exp, and a third score slot
            # (the two mm banks) rotates in once the weave is drained so the
            # two exp engines overlap fully.
            o_live = {}
            weave_box = [groups_gen(weave_a)]
            cgen_box = [None]
            cqueue = []

            def wpull(n):
                if weave_box[0] is not None:
                    weave_box[0] = pull(weave_box[0], n)
                    if weave_box[0] is None and cqueue:
                        cgen_box[0] = c_block(cqueue.pop(0))
                elif cgen_box[0] is not None:
                    cgen_box[0] = pull(cgen_box[0], n)
                    if cgen_box[0] is None and cqueue:
                        cgen_box[0] = c_block(cqueue.pop(0))

            units = [(qg, h, ktp)
                     for qg in range(2) for h in range(H)
                     for ktp in range(NKT // 2)]

            def dve_unit(u):
                qg, h, ktp = u
                if qg == 0 and 8 <= h < 12:
                    return ktp in (1, 3, 5, 7)
                return ktp in DVE_KTPS

            def alloc_sc(u):
                free_mm = (weave_box[0] is None and cgen_box[0] is None
                           and not cqueue)
                if free_mm and dve_unit(u):
                    ta = ps_mm.tile([128, 512], F32, tag="mm")
                    tb = ps_mm.tile([128, 512], F32, tag="mm")
                    return (ta, tb)
                sc2 = ps_sc.tile([128, 1024], F32, tag="sc")
                return (sc2,)

            def emit_scores(u, sct):
                qg, h, ktp = u
                quad, sl = h // 4, h % 4
                p0 = 32 * sl
                for j in range(2):
                    kt = 2 * ktp + j
                    if len(sct) == 1:
                        out = sct[0][:, j * 512:(j + 1) * 512]
                    else:
                        out = sct[j][:]
                    nc.tensor.matmul(
                        out,
                        kS[p0:p0 + 32, quad, kt // 4, :,
                           (kt % 4) * 128:(kt % 4 + 1) * 128],
                        qS[p0:p0 + 32, quad, qg, :, :],
                        start=True, stop=True, perf_mode=DR,
                        tile_position=(p0, 0))

            def emit_exp(u, sct):
                qg, h, ktp = u
                if dve_unit(u):
                    et = ebp.tile([128, 2, 512], I16, tag="eb")
                    if len(sct) == 1:
                        nc.vector.tensor_scalar(
                            out=et[:].rearrange("p i t -> p (i t)"),
                            in0=sct[0][:], scalar1=BT_A, scalar2=BT_B,
                            op0=mybir.AluOpType.mult,
                            op1=mybir.AluOpType.add)
                    else:
                        for j in range(2):
                            nc.vector.tensor_scalar(
                                out=et[:, j, :],
                                in0=sct[j][:], scalar1=BT_A, scalar2=BT_B,
                                op0=mybir.AluOpType.mult,
                                op1=mybir.AluOpType.add)
                else:
                    et = e2p.tile([128, 2, 512], E4, tag="e2")
                    if len(sct) == 1:
                        nc.scalar.activation(
                            out=et[:].rearrange("p i t -> p (i t)"),
                            in_=sct[0][:], func=A.Exp,
                            scale=SEFF, bias=bias_t[:])
                    else:
                        for j in range(2):
                            nc.scalar.activation(
                                out=et[:, j, :], in_=sct[j][:], func=A.Exp,
                                scale=SEFF, bias=bias_t[:])
                return et

            def emit_attnv(u, et):
                qg, h, ktp = u
                if ktp == 0:
                    o_ps = ps_o.tile([128, 512], F32, tag="o")
                    o_live[h] = o_ps
                else:
                    o_ps = o_live[h]
                first = ktp == 0
                last_ktp = ktp == NKT // 2 - 1
                if dve_unit(u):
                    for j in range(2):
                        for qt in range(4):
                            nc.tensor.matmul(
                                o_ps[:, qt * 65:qt * 65 + 65],
                                et[:, j, qt * 128:(qt + 1) * 128]
                                .bitcast(BF16),
                                va[:, 2 * ktp + j, h, :],
                                start=(first and j == 0 and qt == 0),
                                stop=(last_ktp and j == 1 and qt == 3))
                else:
                    for qt in range(4):
                        nc.tensor.matmul(
                            o_ps[:, qt * 65:qt * 65 + 65],
                            et[:, :, qt * 128:(qt + 1) * 128],
                            va[:, 2 * ktp:2 * ktp + 2, h, :],
                            start=(first and qt == 0),
                            stop=(last_ktp and qt == 3),
                            perf_mode=DR)
                if last_ktp:
                    finish_block(qg, h)

            def finish_block(qg, h):
                if h % 2 == 1:
                    hp = h // 2
                    opE, opO = o_live.pop(h - 1), o_live.pop(h)
                    rec = lnp.tile([128, 8], F32, tag="rec")
                    for hh, op in ((0, opE), (1, opO)):
                        nc.vector.reciprocal(
                            rec[:, hh * 4:(hh + 1) * 4],
                            op[:, 0:260].rearrange(
                                "p (q c) -> p q c", c=65)[:, :, DK])
                    for qt in range(4):
                        onrm = onp.tile([128, 128], BF16, tag="onrm")
                        nc.scalar.activation(
                            out=onrm[:, 0:64],
                            in_=opE[:, qt * 65:qt * 65 + 64],
                            func=A.Copy, scale=rec[:, qt:qt + 1])
                        nc.vector.tensor_scalar(
                            out=onrm[:, 64:128],
                            in0=opO[:, qt * 65:qt * 65 + 64],
                            scalar1=rec[:, 4 + qt:5 + qt], scalar2=None,
                            op0=mybir.AluOpType.mult)
                        nc.sync.dma_start(
                            oT[:, hp, qg * 512 + qt * 128:
                               qg * 512 + (qt + 1) * 128],
                            onrm[:], transpose=True)
                if h == 0 and qg == 1:
                    cqueue.extend([0, 1, 2, 3])
                    if weave_box[0] is None and cgen_box[0] is None:
                        cgen_box[0] = c_block(cqueue.pop(0))

            sc_next = alloc_sc(units[0])
            emit_scores(units[0], sc_next)
            att_pend = None

            for idx, u in enumerate(units):
                sc2 = sc_next
                if idx + 1 < len(units):
                    sc_next = alloc_sc(units[idx + 1])
                    emit_scores(units[idx + 1], sc_next)
                et = emit_exp(u, sc2)
                half_pull = 5 if idx < 8 else 2
                wpull(half_pull)
                if att_pend is not None:
                    emit_attnv(*att_pend)
                att_pend = (u, et)
                wpull(half_pull)
            emit_attnv(*att_pend)

            # tail: remaining c_blocks (qg0 leftovers + all of qg1)
            drain(weave_box[0])
            drain(cgen_box[0])
            for tt in cqueue:
                drain(c_block(tt))
            for a, b in ((4, 5), (6, 7)):
                ga = c_block(a, ln_on_scalar=True)
                gb_ = c_block(b, alt=True, ln_on_scalar=True)
                while ga is not None or gb_ is not None:
                    ga = pull(ga, 2)
                    gb_ = pull(gb_, 2)

    nc.compile()
    return nc


def _prep_shared(w_q, w_k, w_v, w_o):
    """Host-side weight layouts (shared across cores)."""
    import ml_dtypes
    bf16 = ml_dtypes.bfloat16
    e4 = ml_dtypes.float8_e4m3

    def qk_perm(wT):
        # wT: [c=1024, d-cols=1024] scaled. Column order for quad/ih/slot:
        # col(quad, ih, ptil) = head(4*quad + ptil//32), d = ih*32 + ptil%32
        w = wT.reshape(1024, 16, 64)                      # [c, head, d]
        out = np.empty((1024, 4, 2, 128), np.float32)
        for quad in range(4):
            for ih in range(2):
                for sl in range(4):
                    hsel = 4 * quad + sl
                    out[:, quad, ih, sl * 32:(sl + 1) * 32] = \
                        w[:, hsel, ih * 32:(ih + 1) * 32]
        # rows c -> [ch, i, p]: c = ch*256 + i*128 + p
        out = out.reshape(4, 2, 128, 4, 2, 128)           # ch i p quad ih col
        out = out.transpose(2, 3, 0, 4, 1, 5)             # p quad ch ih i col
        # dram layout [128, ch, i, 1024-cols(quad,ih,128)]
        out = out.transpose(0, 2, 4, 1, 3, 5)             # p ch i quad ih col
        return np.ascontiguousarray(
            out.reshape(128, 4, 2, 1024)).astype(e4).reshape(128, -1)

    def v_perm(wT):
        # plain col order; rows c -> [ch, i, p]
        out = wT.reshape(4, 2, 128, 1024).transpose(2, 0, 1, 3)
        return np.ascontiguousarray(
            out.reshape(128, 4, 2, 1024)).astype(e4).reshape(128, -1)

    wqT = np.ascontiguousarray(w_q.T) * WS
    wkT = np.ascontiguousarray(w_k.T) * WS
    wvT = np.ascontiguousarray(w_v.T) * WS
    # wo tile [p, hp, e] = w_o.T[hp*128 + p, e]
    woT = np.ascontiguousarray(
        w_o.T.reshape(8, 128, 1024).transpose(1, 0, 2)).astype(bf16)
    return {
        "wq": qk_perm(wqT), "wk": qk_perm(wkT), "wv": v_perm(wvT),
        "wo": woT.reshape(128, -1),
    }


def kernel(x, w_q, w_k, w_v, w_o, b_o, ln_g, ln_b):
    import ml_dtypes
    bf16 = ml_dtypes.bfloat16
    e4 = ml_dtypes.float8_e4m3

    x = np.asarray(x, dtype=np.float32)
    w_q = np.asarray(w_q, dtype=np.float32)
    w_k = np.asarray(w_k, dtype=np.float32)
    w_v = np.asarray(w_v, dtype=np.float32)
    w_o = np.asarray(w_o, dtype=np.float32)
    b_o = np.asarray(b_o, dtype=np.float32)
    ln_g = np.asarray(ln_g, dtype=np.float32)
    ln_b = np.asarray(ln_b, dtype=np.float32)

    apply_gb = not (np.all(ln_g == 1.0) and np.all(ln_b == 0.0))
    apply_bias = bool(np.any(b_o != 0.0))
    key = (apply_gb, apply_bias)
    if key not in _CACHE:
        _CACHE[key] = build(apply_gb, apply_bias)
    nc = _CACHE[key]

    shared = _prep_shared(w_q, w_k, w_v, w_o)
    gb = np.stack([ln_g, ln_b]).astype(np.float32)
    ident_np = np.eye(128, dtype=np.float32).astype(bf16)
    bo = np.ascontiguousarray(b_o.reshape(1, D))

    in_maps = []
    for c in range(N_CORES):
        b = c // 2
        half = c % 2
        xb = x[b]
        if half == 1:
            xb = np.roll(xb, -TOK, axis=0)
        # xdr[p, ch, i, t] = xb[t, ch*256 + i*128 + p]
        xdr = xb.T.reshape(4, 2, 128, S).transpose(2, 0, 1, 3)
        xdr = np.ascontiguousarray(xdr).astype(e4).reshape(128, -1)
        xmy = np.ascontiguousarray(xb[0:TOK]).astype(bf16)
        in_maps.append({
            "xdr": xdr, "xmy": xmy, "bo": bo, "gb": gb,
            "ident": ident_np, **shared,
        })

    res = bass_utils.run_bass_kernel_spmd(nc, in_maps,
                                          core_ids=list(range(N_CORES)))
    y = np.stack([res.results[c]["y"] for c in range(N_CORES)])
    return y.reshape(B, S, D)


# revision 73
# speedup vs baseline: 1.5190x; 1.0028x over previous
"""Multi-head self-attention + residual + LayerNorm on 8 Trainium2 NeuronCores.

Problem: B=4, S=2048, D=1024, H=16, d_k=64, fp32.

Sharding: token-parallel, zero collectives. Core c owns batch b=c//2 and a
1024-query-token half of it (host rotates tokens so own queries are rows
0..1023; softmax/attn@V are permutation-invariant over keys). Each core
recomputes K/V for its full batch.

v9 (256us vs 386us bf16 baseline): fp8 DoubleRow matmuls + engine-balanced
softmax.
 - All projection/score/attnV operands are fp8e4m3 (weights host-scaled x16,
   descale folded into the exp scale and the 16.0 ones-column). DoubleRow
   contracts 256 deep at 0.5 cycles/row: projections cost 1/4, scores 1/2 of
   bf16. Scores put d_k=64 on [32 partitions x 2]; four heads share the
   partition dim via 32-row slots (explicit tile_position (32s, 0)).
 - exp carries bias -4.5 (cancels in softmax, keeps e inside e4m3 range;
   raw scores reach +-9). The work splits across engines per kt-pair:
   ScalarE activation-Exp -> e4m3 (feeds DoubleRow attn@V), DVE Schraudolph
   bit-trick (one tensor_scalar f32->int16, bitcast bf16, feeds mixed
   bf16xfp8 attn@V). The split ratio adapts per phase to DVE's eviction
   load. Ones column gives denominators in psum col 64 for free.
 - Engine schedule: units (head, qg, kt-pair) run software-pipelined one
   unit deep on scores AND one unit deep on attn@V, so PE never blocks on
   the current exp and the two exp engines overlap. Score psums rotate over
   2x[128,1024] plus, once the projection weave drains, the two [128,512]
   ps_mm banks (split exps) - effectively 3-deep.
 - Projections are 4-matmul half-groups on the 2-buffer ps_mm pool
   (ping-pong: group N+1 computes while N evicts). Evictions balance:
   quad0/V-hg0 prep on ScalarE/DVE, later K/Q on DVE, V-hg1 on ScalarE.
   Each group's eviction is emitted before its last yield so woven
   consumers can never be emitted ahead of the data they read.
 - o_nrm: ScalarE Copy(scale=1/den) for head-even, DVE tensor_scalar for
   head-odd, into one [128,(hh,64)] bf16 tile; SBUF->SBUF DMA XBAR
   transpose writes oT directly (no PE transposes, no separate eviction).
 - o-proj/residual/LN stay bf16/f32: o-proj in fp8 fails the 2e-2 error
   budget. LN: bn_stats/aggr + residual adds on DVE, rstd=exp(-.5 ln(var+eps))
   on ScalarE, affine on GPSIMD (woven) or ScalarE Identity (tail, with
   split half stores). Exp/Ln/Copy/Identity pinned to one activation table.

Measured rel err vs f32 reference: 1.17e-2 (gate 2e-2); error budget is
dominated by e4m3 quantization of Q/K/V/e, validated in sim_numerics.py.
"""

import numpy as np

import concourse.mybir as mybir
import concourse.tile as tile
from concourse import bacc
from concourse import bass_utils

F32 = mybir.dt.float32
BF16 = mybir.dt.bfloat16
E4 = mybir.dt.float8e4
I16 = mybir.dt.int16
DR = mybir.MatmulPerfMode.DoubleRow

B, S, D, H, DK = 4, 2048, 1024, 16, 64
N_CORES = 8
TOK = (B * S) // N_CORES            # 1024 query tokens per core
NKT = S // 128                      # 16 k-tiles per batch
NTG = S // 512                      # 4 token groups per batch
EPS = 1e-5
WS = 16.0                           # host weight upscale before fp8 quant
SEFF = 0.125 / (WS * WS)            # exp scale on raw psum scores
EBIAS = -4.5                        # exp bias (cancels in softmax)
LOG2E = 1.4426950408889634
BT_A = float(SEFF * LOG2E * 128.0)  # bit-trick multiplier
BT_B = float((127.0 - 0.0579) * 128.0 + EBIAS * LOG2E * 128.0)

# kt-pairs whose exp runs on DVE (bit-trick); rest on ScalarE
DVE_KTPS = (2, 4, 6)

_CACHE = {}


def build(apply_gb: bool, apply_bias: bool):
    nc = bacc.Bacc("TRN2", target_bir_lowering=False, debug=False,
                   num_devices=N_CORES)
    # Pin every ScalarE function we use (Exp, Ln, Copy) to the one table that
    # holds them all, so the activation table is loaded exactly once.
    from concourse.hw_specs import get_activation_tables
    A = mybir.ActivationFunctionType
    tabs = get_activation_tables(nc.m.arch)
    for name, s in tabs.items():
        if name != "natural_log_exp_and_others":
            s.discard(A.Exp)
            s.discard(A.Ln)
            s.discard(A.Copy)
            s.discard(A.Identity)

    xdr_d = nc.dram_tensor("xdr", [128, 4 * 2 * S], E4, kind="ExternalInput")
    wq_d = nc.dram_tensor("wq", [128, 4 * 2 * 1024], E4, kind="ExternalInput")
    wk_d = nc.dram_tensor("wk", [128, 4 * 2 * 1024], E4, kind="ExternalInput")
    wv_d = nc.dram_tensor("wv", [128, 4 * 2 * 1024], E4, kind="ExternalInput")
    wo_d = nc.dram_tensor("wo", [128, 8 * 1024], BF16, kind="ExternalInput")
    ident_d = nc.dram_tensor("ident", [128, 128], BF16, kind="ExternalInput")
    xmy_d = nc.dram_tensor("xmy", [TOK, D], BF16, kind="ExternalInput")
    bo_d = nc.dram_tensor("bo", [1, D], F32, kind="ExternalInput")
    gb_d = nc.dram_tensor("gb", [2, D], F32, kind="ExternalInput")
    y_d = nc.dram_tensor("y", [TOK, D], F32, kind="ExternalOutput")

    with tile.TileContext(nc) as tc:
        with (
            tc.tile_pool(name="big", bufs=1) as big,
            tc.tile_pool(name="e2p", bufs=6) as e2p,
            tc.tile_pool(name="ebp", bufs=6) as ebp,
            tc.tile_pool(name="onp", bufs=8) as onp,
            tc.tile_pool(name="xr", bufs=4) as xr,
            tc.tile_pool(name="ysb", bufs=4) as ysb,
            tc.tile_pool(name="ln", bufs=6) as lnp,
            tc.tile_pool(name="small", bufs=1) as small,
            tc.tile_pool(name="ps_sc", bufs=2, space="PSUM") as ps_sc,
            tc.tile_pool(name="ps_o", bufs=2, space="PSUM") as ps_o,
            tc.tile_pool(name="ps_mm", bufs=2, space="PSUM") as ps_mm,
        ):
            xdr = big.tile([128, 4, 2, S], E4, tag="xdr")            # 16K/p
            wq = big.tile([128, 4, 2, 1024], E4, tag="wq")           # 8K/p
            wk = big.tile([128, 4, 2, 1024], E4, tag="wk")
            wv = big.tile([128, 4, 2, 1024], E4, tag="wv")
            wo = big.tile([128, 8, 1024], BF16, tag="wo")            # 16K/p
            # K: [quad, tg, i, t]; Q: [quad, qg, i, t]
            kS = big.tile([128, 4, NTG, 2, 512], E4, tag="kS")       # 16K/p
            qS = big.tile([128, 4, 2, 2, 512], E4, tag="qS")         # 8K/p
            # va: [t-part, kt, head, dk+1]; col dk holds 16.0
            va = big.tile([128, NKT, H, DK + 1], E4, tag="va")       # 16.25K/p
            oT = big.tile([128, 8, TOK], BF16, tag="oT")             # 16K/p

            nc.vector.memset(va[:, :, :, DK:DK + 1], WS)
            bias_t = small.tile([128, 1], F32, tag="bias")
            nc.vector.memset(bias_t[:], EBIAS)
            eps_t = small.tile([128, 1], F32, tag="eps")
            nc.vector.memset(eps_t[:], EPS)

            # ---------------- DMA loads (first-use order) ----------------
            # xdr feeds every projection; quad0 K/Q cols + hg0 V cols next.
            xdr_v = xdr_d.ap().rearrange("p (c i t) -> p c i t", c=4, i=2)
            wkv = wk_d.ap().rearrange("p (c i t) -> p c i t", c=4, i=2)
            wqv = wq_d.ap().rearrange("p (c i t) -> p c i t", c=4, i=2)
            wvv = wv_d.ap().rearrange("p (c i t) -> p c i t", c=4, i=2)
            nc.sync.dma_start(xdr[:, :, :, 0:512], xdr_v[:, :, :, 0:512])
            nc.sync.dma_start(wk[:, :, :, 0:256], wkv[:, :, :, 0:256])
            nc.sync.dma_start(wq[:, :, :, 0:256], wqv[:, :, :, 0:256])
            nc.sync.dma_start(wv[:, :, :, 0:512], wvv[:, :, :, 0:512])
            nc.sync.dma_start(xdr[:, :, :, 512:1024], xdr_v[:, :, :, 512:1024])
            nc.sync.dma_start(xdr[:, :, :, 1024:2048], xdr_v[:, :, :, 1024:2048])
            nc.sync.dma_start(wk[:, :, :, 256:1024], wkv[:, :, :, 256:1024])
            nc.sync.dma_start(wq[:, :, :, 256:1024], wqv[:, :, :, 256:1024])
            nc.sync.dma_start(wv[:, :, :, 512:1024], wvv[:, :, :, 512:1024])
            nc.sync.dma_start(
                wo[:], wo_d.ap().rearrange("p (h t) -> p h t", h=8))
            ident = big.tile([128, 128], BF16, tag="ident")
            nc.sync.dma_start(ident[:], ident_d.ap())
            if apply_bias:
                bo_bc = small.tile([128, D], F32, tag="bobc")
                nc.sync.dma_start(bo_bc[:],
                                  bo_d.ap()[0:1, :].broadcast_to((128, D)))
            if apply_gb:
                g_bc = small.tile([128, D], F32, tag="gbc")
                b_bc = small.tile([128, D], F32, tag="bbc")
                nc.sync.dma_start(g_bc[:],
                                  gb_d.ap()[0:1, :].broadcast_to((128, D)))
                nc.sync.dma_start(b_bc[:],
                                  gb_d.ap()[1:2, :].broadcast_to((128, D)))

            # ---------------- projection group emitters ----------------
            # every group is a 4-matmul chain into one [128,512] psum (one
            # bank); the ps_mm pool's two buffers ping-pong so group N+1's
            # matmuls overlap group N's eviction.
            def k_half(quad, tg, ih):
                ps = ps_mm.tile([128, 512], F32, tag="mm")
                for ch in range(4):
                    nc.tensor.matmul(
                        ps[:],
                        wk[:, ch, :, quad * 256 + ih * 128:
                           quad * 256 + (ih + 1) * 128],
                        xdr[:, ch, :, tg * 512:(tg + 1) * 512],
                        start=(ch == 0), stop=(ch == 3), perf_mode=DR)
                    if ch < 3:
                        yield
                if quad == 0:
                    nc.scalar.activation(
                        out=kS[:, quad, tg, ih, :], in_=ps[:], func=A.Copy)
                else:
                    nc.vector.tensor_copy(kS[:, quad, tg, ih, :], ps[:])
                yield

            def q_half(quad, qg, ih):
                ps = ps_mm.tile([128, 512], F32, tag="mm")
                for ch in range(4):
                    nc.tensor.matmul(
                        ps[:],
                        wq[:, ch, :, quad * 256 + ih * 128:
                           quad * 256 + (ih + 1) * 128],
                        xdr[:, ch, :, qg * 512:(qg + 1) * 512],
                        start=(ch == 0), stop=(ch == 3), perf_mode=DR)
                    if ch < 3:
                        yield
                if quad == 0 and qg == 0:
                    nc.scalar.activation(
                        out=qS[:, quad, qg, ih, :], in_=ps[:], func=A.Copy)
                else:
                    nc.vector.tensor_copy(qS[:, quad, qg, ih, :], ps[:])

            def v_group(kt, hg):
                ps = ps_mm.tile([128, 512], F32, tag="mm")
                for ch in range(4):
                    nc.tensor.matmul(
                        ps[:],
                        xdr[:, ch, :, kt * 128:(kt + 1) * 128],
                        wv[:, ch, :, hg * 512:(hg + 1) * 512],
                        start=(ch == 0), stop=(ch == 3), perf_mode=DR)
                    yield
                if hg == 0:
                    nc.vector.tensor_copy(
                        va[:, kt, 0:8, 0:DK],
                        ps[:].rearrange("p (h c) -> p h c", h=8))
                else:
                    nc.scalar.activation(
                        out=va[:, kt, 8:16, 0:DK],
                        in_=ps[:].rearrange("p (h c) -> p h c", h=8),
                        func=A.Copy)

            def groups_gen(worklist):
                for item in worklist:
                    if item[0] == "k":
                        yield from k_half(item[1], item[2], item[3])
                    elif item[0] == "q":
                        yield from q_half(item[1], item[2], item[3])
                    else:
                        yield from v_group(item[1], item[2])

            # ---------------- c_block: o-proj + residual + LN ----------------
            def c_block(tt, alt=False, ln_on_scalar=False, tail=False):
                x_t = xr.tile([128, D], BF16, tag="xres")
                nc.sync.dma_start(x_t[:], xmy_d.ap()[tt * 128:(tt + 1) * 128, :])
                if alt == 2:
                    ps_e0 = ps_o.tile([128, 512], F32, tag="o")
                    ps_e1 = ps_o.tile([128, 512], F32, tag="o")
                    pss = [ps_e0[:], ps_e1[:]]
                elif alt:
                    big_ps = ps_sc.tile([128, 1024], F32, tag="sc")
                    pss = [big_ps[:, 0:512], big_ps[:, 512:1024]]
                else:
                    ps_e0 = ps_mm.tile([128, 512], F32, tag="mm")
                    ps_e1 = ps_mm.tile([128, 512], F32, tag="mm")
                    pss = [ps_e0[:], ps_e1[:]]
                for eh in range(2):
                    for hp in range(8):
                        nc.tensor.matmul(
                            pss[eh], oT[:, hp, tt * 128:(tt + 1) * 128],
                            wo[:, hp, eh * 512:(eh + 1) * 512],
                            start=(hp == 0), stop=(hp == 7 and not tail))
                        yield
                y_sb = ysb.tile([128, D], F32, tag="ysb")
                stats = lnp.tile([128, 2, nc.vector.BN_STATS_DIM], F32, tag="st")
                if tail:
                    # residual add on PE (identity matmul); stats from psum
                    for eh in range(2):
                        nc.tensor.matmul(
                            pss[eh], ident[:],
                            x_t[:, eh * 512:(eh + 1) * 512],
                            start=False, stop=True)
                    for eh in range(2):
                        nc.vector.bn_stats(stats[:, eh, :], pss[eh])
                else:
                    for eh in range(2):
                        nc.vector.tensor_add(
                            y_sb[:, eh * 512:(eh + 1) * 512],
                            pss[eh], x_t[:, eh * 512:(eh + 1) * 512])
                    if apply_bias:
                        nc.vector.tensor_add(y_sb[:], y_sb[:], bo_bc[:])
                    nc.vector.bn_stats(stats[:, 0, :], y_sb[:, 0:512])
                    nc.vector.bn_stats(stats[:, 1, :], y_sb[:, 512:1024])
                mv = lnp.tile([128, nc.vector.BN_AGGR_DIM], F32, tag="mv")
                nc.vector.bn_aggr(mv[:], stats[:])
                lnv = lnp.tile([128, 1], F32, tag="lnv")
                rstd = lnp.tile([128, 1], F32, tag="rstd")
                nc.scalar.activation(
                    out=lnv[:], in_=mv[:, 1:2],
                    func=A.Ln, bias=eps_t[:], scale=1.0)
                nc.scalar.activation(
                    out=rstd[:], in_=lnv[:], func=A.Exp, scale=-0.5)
                if ln_on_scalar:
                    nmu = lnp.tile([128, 1], F32, tag="nmu")
                    nc.vector.tensor_scalar(
                        out=nmu[:], in0=mv[:, 0:1], scalar1=rstd[:],
                        scalar2=-1.0, op0=mybir.AluOpType.mult,
                        op1=mybir.AluOpType.mult)
                    for eh in range(2):
                        nc.scalar.activation(
                            out=y_sb[:, eh * 512:(eh + 1) * 512],
                            in_=y_sb[:, eh * 512:(eh + 1) * 512],
                            func=A.Identity, scale=rstd[:], bias=nmu[:])
                        if not apply_gb:
                            nc.sync.dma_start(
                                y_d.ap()[tt * 128:(tt + 1) * 128,
                                         eh * 512:(eh + 1) * 512],
                                y_sb[:, eh * 512:(eh + 1) * 512])
                else:
                    nc.gpsimd.tensor_scalar(
                        out=y_sb[:], in0=y_sb[:],
                        scalar1=mv[:, 0:1], scalar2=rstd[:],
                        op0=mybir.AluOpType.subtract,
                        op1=mybir.AluOpType.mult)
                if apply_gb:
                    nc.gpsimd.tensor_mul(y_sb[:], y_sb[:], g_bc[:])
                    nc.gpsimd.tensor_add(y_sb[:], y_sb[:], b_bc[:])
                if not ln_on_scalar or apply_gb:
                    nc.sync.dma_start(
                        y_d.ap()[tt * 128:(tt + 1) * 128, :], y_sb[:])

            def pull(gen, n):
                if gen is None:
                    return None
                for _ in range(n):
                    try:
                        next(gen)
                    except StopIteration:
                        return None
                return gen

            def drain(gen):
                if gen is not None:
                    for _ in gen:
                        pass

            # ---------------- prelude projections ----------------
            emit_now = ([("k", 0, 0, ih) for ih in range(2)]
                        + [("k", 0, 1, ih) for ih in range(2)]
                        + [("q", 0, 0, ih) for ih in range(2)]
                        + [("v", kt, 0) for kt in range(4)])
            drain(groups_gen(emit_now))

            # deadline-ordered weave (block h0 pulls 10/unit): V-hg0 paced 2
            # groups/unit just ahead of attn@V, K tg2/tg3 slotted to land
            # before their first scores; then quads 1-3, V-hg1 (before block
            # h8), Q-qg1.
            weave_a = ([("v", 4, 0), ("v", 5, 0)]
                       + [("k", 0, 2, ih) for ih in range(2)]
                       + [("v", 6, 0), ("v", 7, 0), ("v", 8, 0), ("v", 9, 0)]
                       + [("k", 0, 3, ih) for ih in range(2)]
                       + [("v", kt, 0) for kt in range(10, NKT)]
                       + [it for quad in (1, 2, 3) for it in
                          [("k", quad, tg, ih)
                           for tg in range(NTG) for ih in range(2)]
                          + [("q", quad, 0, ih) for ih in range(2)]]
                       + [("v", kt, 1) for kt in range(NKT)]
                       + [("q", quad, 1, ih)
                          for quad in range(4) for ih in range(2)])

            # ---------------- attention blocks ----------------
            # Software-pipelined one unit deep on scores AND on attn@V: PE
            # never waits for the current unit's exp, and a third score slot
            # (the two mm banks) rotates in once the weave is drained so the
            # two exp engines overlap fully.
            o_live = {}
            weave_box = [groups_gen(weave_a)]
            cgen_box = [None]
            cqueue = []

            def wpull(n):
                if weave_box[0] is not None:
                    weave_box[0] = pull(weave_box[0], n)
                    if weave_box[0] is None and cqueue:
                        cgen_box[0] = c_block(cqueue.pop(0))
                elif cgen_box[0] is not None:
                    cgen_box[0] = pull(cgen_box[0], n)
                    if cgen_box[0] is None and cqueue:
                        cgen_box[0] = c_block(cqueue.pop(0))

            units = [(qg, h, ktp)
                     for qg in range(2) for h in range(H)
                     for ktp in range(NKT // 2)]

            def dve_unit(u):
                qg, h, ktp = u
                if qg == 0 and 8 <= h < 12:
                    return ktp in (1, 3, 5, 7)
                return ktp in DVE_KTPS

            def alloc_sc(u):
                free_mm = (weave_box[0] is None and cgen_box[0] is None
                           and not cqueue)
                if free_mm and dve_unit(u):
                    ta = ps_mm.tile([128, 512], F32, tag="mm")
                    tb = ps_mm.tile([128, 512], F32, tag="mm")
                    return (ta, tb)
                sc2 = ps_sc.tile([128, 1024], F32, tag="sc")
                return (sc2,)

            def emit_scores(u, sct):
                qg, h, ktp = u
                quad, sl = h // 4, h % 4
                p0 = 32 * sl
                for j in range(2):
                    kt = 2 * ktp + j
                    if len(sct) == 1:
                        out = sct[0][:, j * 512:(j + 1) * 512]
                    else:
                        out = sct[j][:]
                    nc.tensor.matmul(
                        out,
                        kS[p0:p0 + 32, quad, kt // 4, :,
                           (kt % 4) * 128:(kt % 4 + 1) * 128],
                        qS[p0:p0 + 32, quad, qg, :, :],
                        start=True, stop=True, perf_mode=DR,
                        tile_position=(p0, 0))

            def emit_exp(u, sct):
                qg, h, ktp = u
                if dve_unit(u):
                    et = ebp.tile([128, 2, 512], I16, tag="eb")
                    if len(sct) == 1:
                        nc.vector.tensor_scalar(
                            out=et[:].rearrange("p i t -> p (i t)"),
                            in0=sct[0][:], scalar1=BT_A, scalar2=BT_B,
                            op0=mybir.AluOpType.mult,
                            op1=mybir.AluOpType.add)
                    else:
                        for j in range(2):
                            nc.vector.tensor_scalar(
                                out=et[:, j, :],
                                in0=sct[j][:], scalar1=BT_A, scalar2=BT_B,
                                op0=mybir.AluOpType.mult,
                                op1=mybir.AluOpType.add)
                else:
                    et = e2p.tile([128, 2, 512], E4, tag="e2")
                    if len(sct) == 1:
                        nc.scalar.activation(
                            out=et[:].rearrange("p i t -> p (i t)"),
                            in_=sct[0][:], func=A.Exp,
                            scale=SEFF, bias=bias_t[:])
                    else:
                        for j in range(2):
                            nc.scalar.activation(
                                out=et[:, j, :], in_=sct[j][:], func=A.Exp,
                                scale=SEFF, bias=bias_t[:])
                return et

            def emit_attnv(u, et):
                qg, h, ktp = u
                if ktp == 0:
                    o_ps = ps_o.tile([128, 512], F32, tag="o")
                    o_live[h] = o_ps
                else:
                    o_ps = o_live[h]
                first = ktp == 0
                last_ktp = ktp == NKT // 2 - 1
                if dve_unit(u):
                    for j in range(2):
                        for qt in range(4):
                            nc.tensor.matmul(
                                o_ps[:, qt * 65:qt * 65 + 65],
                                et[:, j, qt * 128:(qt + 1) * 128]
                                .bitcast(BF16),
                                va[:, 2 * ktp + j, h, :],
                                start=(first and j == 0 and qt == 0),
                                stop=(last_ktp and j == 1 and qt == 3))
                else:
                    for qt in range(4):
                        nc.tensor.matmul(
                            o_ps[:, qt * 65:qt * 65 + 65],
                            et[:, :, qt * 128:(qt + 1) * 128],
                            va[:, 2 * ktp:2 * ktp + 2, h, :],
                            start=(first and qt == 0),
                            stop=(last_ktp and qt == 3),
                            perf_mode=DR)
                if last_ktp:
                    finish_block(qg, h)

            def finish_block(qg, h):
                if h % 2 == 1:
                    hp = h // 2
                    opE, opO = o_live.pop(h - 1), o_live.pop(h)
                    rec = lnp.tile([128, 8], F32, tag="rec")
                    for hh, op in ((0, opE), (1, opO)):
                        nc.vector.reciprocal(
                            rec[:, hh * 4:(hh + 1) * 4],
                            op[:, 0:260].rearrange(
                                "p (q c) -> p q c", c=65)[:, :, DK])
                    for qt in range(4):
                        onrm = onp.tile([128, 128], BF16, tag="onrm")
                        nc.scalar.activation(
                            out=onrm[:, 0:64],
                            in_=opE[:, qt * 65:qt * 65 + 64],
                            func=A.Copy, scale=rec[:, qt:qt + 1])
                        nc.vector.tensor_scalar(
                            out=onrm[:, 64:128],
                            in0=opO[:, qt * 65:qt * 65 + 64],
                            scalar1=rec[:, 4 + qt:5 + qt], scalar2=None,
                            op0=mybir.AluOpType.mult)
                        nc.sync.dma_start(
                            oT[:, hp, qg * 512 + qt * 128:
                               qg * 512 + (qt + 1) * 128],
                            onrm[:], transpose=True)
                if h == 0 and qg == 1:
                    cqueue.extend([0, 1, 2, 3])
                    if weave_box[0] is None and cgen_box[0] is None:
                        cgen_box[0] = c_block(cqueue.pop(0))

            sc_next = alloc_sc(units[0])
            emit_scores(units[0], sc_next)
            att_pend = None

            for idx, u in enumerate(units):
                sc2 = sc_next
                if idx + 1 < len(units):
                    sc_next = alloc_sc(units[idx + 1])
                    emit_scores(units[idx + 1], sc_next)
                et = emit_exp(u, sc2)
                half_pull = 5 if idx < 8 else 2
                wpull(half_pull)
                if att_pend is not None:
                    emit_attnv(*att_pend)
                att_pend = (u, et)
                wpull(half_pull)
            emit_attnv(*att_pend)

            # tail: remaining c_blocks (qg0 leftovers + all of qg1)
            drain(weave_box[0])
            drain(cgen_box[0])
            for tt in cqueue:
                drain(c_block(tt))
            for a, b in ((4, 5), (6, 7)):
                ga = c_block(a, ln_on_scalar=True)
                gb_ = c_block(b, alt=True, ln_on_scalar=True)
                while ga is not None or gb_ is not None:
                    ga = pull(ga, 4)
                    gb_ = pull(gb_, 4)

    nc.compile()
    return nc


def _prep_shared(w_q, w_k, w_v, w_o):
    """Host-side weight layouts (shared across cores)."""
    import ml_dtypes
    bf16 = ml_dtypes.bfloat16
    e4 = ml_dtypes.float8_e4m3

    def qk_perm(wT):
        # wT: [c=1024, d-cols=1024] scaled. Column order for quad/ih/slot:
        # col(quad, ih, ptil) = head(4*quad + ptil//32), d = ih*32 + ptil%32
        w = wT.reshape(1024, 16, 64)                      # [c, head, d]
        out = np.empty((1024, 4, 2, 128), np.float32)
        for quad in range(4):
            for ih in range(2):
                for sl in range(4):
                    hsel = 4 * quad + sl
                    out[:, quad, ih, sl * 32:(sl + 1) * 32] = \
                        w[:, hsel, ih * 32:(ih + 1) * 32]
        # rows c -> [ch, i, p]: c = ch*256 + i*128 + p
        out = out.reshape(4, 2, 128, 4, 2, 128)           # ch i p quad ih col
        out = out.transpose(2, 3, 0, 4, 1, 5)             # p quad ch ih i col
        # dram layout [128, ch, i, 1024-cols(quad,ih,128)]
        out = out.transpose(0, 2, 4, 1, 3, 5)             # p ch i quad ih col
        return np.ascontiguousarray(
            out.reshape(128, 4, 2, 1024)).astype(e4).reshape(128, -1)

    def v_perm(wT):
        # plain col order; rows c -> [ch, i, p]
        out = wT.reshape(4, 2, 128, 1024).transpose(2, 0, 1, 3)
        return np.ascontiguousarray(
            out.reshape(128, 4, 2, 1024)).astype(e4).reshape(128, -1)

    wqT = np.ascontiguousarray(w_q.T) * WS
    wkT = np.ascontiguousarray(w_k.T) * WS
    wvT = np.ascontiguousarray(w_v.T) * WS
    # wo tile [p, hp, e] = w_o.T[hp*128 + p, e]
    woT = np.ascontiguousarray(
        w_o.T.reshape(8, 128, 1024).transpose(1, 0, 2)).astype(bf16)
    return {
        "wq": qk_perm(wqT), "wk": qk_perm(wkT), "wv": v_perm(wvT),
        "wo": woT.reshape(128, -1),
    }


def kernel(x, w_q, w_k, w_v, w_o, b_o, ln_g, ln_b):
    import ml_dtypes
    bf16 = ml_dtypes.bfloat16
    e4 = ml_dtypes.float8_e4m3

    x = np.asarray(x, dtype=np.float32)
    w_q = np.asarray(w_q, dtype=np.float32)
    w_k = np.asarray(w_k, dtype=np.float32)
    w_v = np.asarray(w_v, dtype=np.float32)
    w_o = np.asarray(w_o, dtype=np.float32)
    b_o = np.asarray(b_o, dtype=np.float32)
    ln_g = np.asarray(ln_g, dtype=np.float32)
    ln_b = np.asarray(ln_b, dtype=np.float32)

    apply_gb = not (np.all(ln_g == 1.0) and np.all(ln_b == 0.0))
    apply_bias = bool(np.any(b_o != 0.0))
    key = (apply_gb, apply_bias)
    if key not in _CACHE:
        _CACHE[key] = build(apply_gb, apply_bias)
    nc = _CACHE[key]

    shared = _prep_shared(w_q, w_k, w_v, w_o)
    gb = np.stack([ln_g, ln_b]).astype(np.float32)
    ident_np = np.eye(128, dtype=np.float32).astype(bf16)
    bo = np.ascontiguousarray(b_o.reshape(1, D))

    in_maps = []
    for c in range(N_CORES):
        b = c // 2
        half = c % 2
        xb = x[b]
        if half == 1:
            xb = np.roll(xb, -TOK, axis=0)
        # xdr[p, ch, i, t] = xb[t, ch*256 + i*128 + p]
        xdr = xb.T.reshape(4, 2, 128, S).transpose(2, 0, 1, 3)
        xdr = np.ascontiguousarray(xdr).astype(e4).reshape(128, -1)
        xmy = np.ascontiguousarray(xb[0:TOK]).astype(bf16)
        in_maps.append({
            "xdr": xdr, "xmy": xmy, "bo": bo, "gb": gb,
            "ident": ident_np, **shared,
        })

    res = bass_utils.run_bass_kernel_spmd(nc, in_maps,
                                          core_ids=list(range(N_CORES)))
    y = np.stack([res.results[c]["y"] for c in range(N_CORES)])
    return y.reshape(B, S, D)


# revision 75
# speedup vs baseline: 1.5217x; 1.0018x over previous
"""Multi-head self-attention + residual + LayerNorm on 8 Trainium2 NeuronCores.

Problem: B=4, S=2048, D=1024, H=16, d_k=64, fp32.

Sharding: token-parallel, zero collectives. Core c owns batch b=c//2 and a
1024-query-token half of it (host rotates tokens so own queries are rows
0..1023; softmax/attn@V are permutation-invariant over keys). Each core
recomputes K/V for its full batch.

v9 (256us vs 386us bf16 baseline): fp8 DoubleRow matmuls + engine-balanced
softmax.
 - All projection/score/attnV operands are fp8e4m3 (weights host-scaled x16,
   descale folded into the exp scale and the 16.0 ones-column). DoubleRow
   contracts 256 deep at 0.5 cycles/row: projections cost 1/4, scores 1/2 of
   bf16. Scores put d_k=64 on [32 partitions x 2]; four heads share the
   partition dim via 32-row slots (explicit tile_position (32s, 0)).
 - exp carries bias -4.5 (cancels in softmax, keeps e inside e4m3 range;
   raw scores reach +-9). The work splits across engines per kt-pair:
   ScalarE activation-Exp -> e4m3 (feeds DoubleRow attn@V), DVE Schraudolph
   bit-trick (one tensor_scalar f32->int16, bitcast bf16, feeds mixed
   bf16xfp8 attn@V). The split ratio adapts per phase to DVE's eviction
   load. Ones column gives denominators in psum col 64 for free.
 - Engine schedule: units (head, qg, kt-pair) run software-pipelined one
   unit deep on scores AND one unit deep on attn@V, so PE never blocks on
   the current exp and the two exp engines overlap. Score psums rotate over
   2x[128,1024] plus, once the projection weave drains, the two [128,512]
   ps_mm banks (split exps) - effectively 3-deep.
 - Projections are 4-matmul half-groups on the 2-buffer ps_mm pool
   (ping-pong: group N+1 computes while N evicts). Evictions balance:
   quad0/V-hg0 prep on ScalarE/DVE, later K/Q on DVE, V-hg1 on ScalarE.
   Each group's eviction is emitted before its last yield so woven
   consumers can never be emitted ahead of the data they read.
 - o_nrm: ScalarE Copy(scale=1/den) for head-even, DVE tensor_scalar for
   head-odd, into one [128,(hh,64)] bf16 tile; SBUF->SBUF DMA XBAR
   transpose writes oT directly (no PE transposes, no separate eviction).
 - o-proj/residual/LN stay bf16/f32: o-proj in fp8 fails the 2e-2 error
   budget. LN: bn_stats/aggr + residual adds on DVE, rstd=exp(-.5 ln(var+eps))
   on ScalarE, affine on GPSIMD (woven) or ScalarE Identity (tail, with
   split half stores). Exp/Ln/Copy/Identity pinned to one activation table.

Measured rel err vs f32 reference: 1.17e-2 (gate 2e-2); error budget is
dominated by e4m3 quantization of Q/K/V/e, validated in sim_numerics.py.
"""

import numpy as np

import concourse.mybir as mybir
import concourse.tile as tile
from concourse import bacc
from concourse import bass_utils

F32 = mybir.dt.float32
BF16 = mybir.dt.bfloat16
E4 = mybir.dt.float8e4
I16 = mybir.dt.int16
DR = mybir.MatmulPerfMode.DoubleRow

B, S, D, H, DK = 4, 2048, 1024, 16, 64
N_CORES = 8
TOK = (B * S) // N_CORES            # 1024 query tokens per core
NKT = S // 128                      # 16 k-tiles per batch
NTG = S // 512                      # 4 token groups per batch
EPS = 1e-5
WS = 16.0                           # host weight upscale before fp8 quant
SEFF = 0.125 / (WS * WS)            # exp scale on raw psum scores
EBIAS = -4.5                        # exp bias (cancels in softmax)
LOG2E = 1.4426950408889634
BT_A = float(SEFF * LOG2E * 128.0)  # bit-trick multiplier
BT_B = float((127.0 - 0.0579) * 128.0 + EBIAS * LOG2E * 128.0)

# kt-pairs whose exp runs on DVE (bit-trick); rest on ScalarE
DVE_KTPS = (2, 4, 6)

_CACHE = {}


def build(apply_gb: bool, apply_bias: bool):
    nc = bacc.Bacc("TRN2", target_bir_lowering=False, debug=False,
                   num_devices=N_CORES)
    # Pin every ScalarE function we use (Exp, Ln, Copy) to the one table that
    # holds them all, so the activation table is loaded exactly once.
    from concourse.hw_specs import get_activation_tables
    A = mybir.ActivationFunctionType
    tabs = get_activation_tables(nc.m.arch)
    for name, s in tabs.items():
        if name != "natural_log_exp_and_others":
            s.discard(A.Exp)
            s.discard(A.Ln)
            s.discard(A.Copy)
            s.discard(A.Identity)

    xdr_d = nc.dram_tensor("xdr", [128, 4 * 2 * S], E4, kind="ExternalInput")
    wq_d = nc.dram_tensor("wq", [128, 4 * 2 * 1024], E4, kind="ExternalInput")
    wk_d = nc.dram_tensor("wk", [128, 4 * 2 * 1024], E4, kind="ExternalInput")
    wv_d = nc.dram_tensor("wv", [128, 4 * 2 * 1024], E4, kind="ExternalInput")
    wo_d = nc.dram_tensor("wo", [128, 8 * 1024], BF16, kind="ExternalInput")
    ident_d = nc.dram_tensor("ident", [128, 128], BF16, kind="ExternalInput")
    xmy_d = nc.dram_tensor("xmy", [TOK, D], BF16, kind="ExternalInput")
    bo_d = nc.dram_tensor("bo", [1, D], F32, kind="ExternalInput")
    gb_d = nc.dram_tensor("gb", [2, D], F32, kind="ExternalInput")
    y_d = nc.dram_tensor("y", [TOK, D], F32, kind="ExternalOutput")

    with tile.TileContext(nc) as tc:
        with (
            tc.tile_pool(name="big", bufs=1) as big,
            tc.tile_pool(name="e2p", bufs=6) as e2p,
            tc.tile_pool(name="ebp", bufs=6) as ebp,
            tc.tile_pool(name="onp", bufs=8) as onp,
            tc.tile_pool(name="xr", bufs=4) as xr,
            tc.tile_pool(name="ysb", bufs=4) as ysb,
            tc.tile_pool(name="ln", bufs=6) as lnp,
            tc.tile_pool(name="small", bufs=1) as small,
            tc.tile_pool(name="ps_sc", bufs=2, space="PSUM") as ps_sc,
            tc.tile_pool(name="ps_o", bufs=2, space="PSUM") as ps_o,
            tc.tile_pool(name="ps_mm", bufs=2, space="PSUM") as ps_mm,
        ):
            xdr = big.tile([128, 4, 2, S], E4, tag="xdr")            # 16K/p
            wq = big.tile([128, 4, 2, 1024], E4, tag="wq")           # 8K/p
            wk = big.tile([128, 4, 2, 1024], E4, tag="wk")
            wv = big.tile([128, 4, 2, 1024], E4, tag="wv")
            wo = big.tile([128, 8, 1024], BF16, tag="wo")            # 16K/p
            # K: [quad, tg, i, t]; Q: [quad, qg, i, t]
            kS = big.tile([128, 4, NTG, 2, 512], E4, tag="kS")       # 16K/p
            qS = big.tile([128, 4, 2, 2, 512], E4, tag="qS")         # 8K/p
            # va: [t-part, kt, head, dk+1]; col dk holds 16.0
            va = big.tile([128, NKT, H, DK + 1], E4, tag="va")       # 16.25K/p
            oT = big.tile([128, 8, TOK], BF16, tag="oT")             # 16K/p

            nc.vector.memset(va[:, :, :, DK:DK + 1], WS)
            bias_t = small.tile([128, 1], F32, tag="bias")
            nc.vector.memset(bias_t[:], EBIAS)
            eps_t = small.tile([128, 1], F32, tag="eps")
            nc.vector.memset(eps_t[:], EPS)

            # ---------------- DMA loads (first-use order) ----------------
            # xdr feeds every projection; quad0 K/Q cols + hg0 V cols next.
            xdr_v = xdr_d.ap().rearrange("p (c i t) -> p c i t", c=4, i=2)
            wkv = wk_d.ap().rearrange("p (c i t) -> p c i t", c=4, i=2)
            wqv = wq_d.ap().rearrange("p (c i t) -> p c i t", c=4, i=2)
            wvv = wv_d.ap().rearrange("p (c i t) -> p c i t", c=4, i=2)
            nc.sync.dma_start(xdr[:, :, :, 0:512], xdr_v[:, :, :, 0:512])
            nc.sync.dma_start(wk[:, :, :, 0:256], wkv[:, :, :, 0:256])
            nc.sync.dma_start(wq[:, :, :, 0:256], wqv[:, :, :, 0:256])
            nc.sync.dma_start(wv[:, :, :, 0:512], wvv[:, :, :, 0:512])
            nc.sync.dma_start(xdr[:, :, :, 512:1024], xdr_v[:, :, :, 512:1024])
            nc.sync.dma_start(xdr[:, :, :, 1024:2048], xdr_v[:, :, :, 1024:2048])
            nc.sync.dma_start(wk[:, :, :, 256:1024], wkv[:, :, :, 256:1024])
            nc.sync.dma_start(wq[:, :, :, 256:1024], wqv[:, :, :, 256:1024])
            nc.sync.dma_start(wv[:, :, :, 512:1024], wvv[:, :, :, 512:1024])
            nc.sync.dma_start(
                wo[:], wo_d.ap().rearrange("p (h t) -> p h t", h=8))
            ident = big.tile([128, 128], BF16, tag="ident")
            nc.sync.dma_start(ident[:], ident_d.ap())
            if apply_bias:
                bo_bc = small.tile([128, D], F32, tag="bobc")
                nc.sync.dma_start(bo_bc[:],
                                  bo_d.ap()[0:1, :].broadcast_to((128, D)))
            if apply_gb:
                g_bc = small.tile([128, D], F32, tag="gbc")
                b_bc = small.tile([128, D], F32, tag="bbc")
                nc.sync.dma_start(g_bc[:],
                                  gb_d.ap()[0:1, :].broadcast_to((128, D)))
                nc.sync.dma_start(b_bc[:],
                                  gb_d.ap()[1:2, :].broadcast_to((128, D)))

            # ---------------- projection group emitters ----------------
            # every group is a 4-matmul chain into one [128,512] psum (one
            # bank); the ps_mm pool's two buffers ping-pong so group N+1's
            # matmuls overlap group N's eviction.
            def k_half(quad, tg, ih):
                ps = ps_mm.tile([128, 512], F32, tag="mm")
                for ch in range(4):
                    nc.tensor.matmul(
                        ps[:],
                        wk[:, ch, :, quad * 256 + ih * 128:
                           quad * 256 + (ih + 1) * 128],
                        xdr[:, ch, :, tg * 512:(tg + 1) * 512],
                        start=(ch == 0), stop=(ch == 3), perf_mode=DR)
                    if ch < 3:
                        yield
                if quad == 0:
                    nc.scalar.activation(
                        out=kS[:, quad, tg, ih, :], in_=ps[:], func=A.Copy)
                else:
                    nc.vector.tensor_copy(kS[:, quad, tg, ih, :], ps[:])
                yield

            def q_half(quad, qg, ih):
                ps = ps_mm.tile([128, 512], F32, tag="mm")
                for ch in range(4):
                    nc.tensor.matmul(
                        ps[:],
                        wq[:, ch, :, quad * 256 + ih * 128:
                           quad * 256 + (ih + 1) * 128],
                        xdr[:, ch, :, qg * 512:(qg + 1) * 512],
                        start=(ch == 0), stop=(ch == 3), perf_mode=DR)
                    if ch < 3:
                        yield
                if quad == 0 and qg == 0:
                    nc.scalar.activation(
                        out=qS[:, quad, qg, ih, :], in_=ps[:], func=A.Copy)
                else:
                    nc.vector.tensor_copy(qS[:, quad, qg, ih, :], ps[:])

            def v_group(kt, hg):
                ps = ps_mm.tile([128, 512], F32, tag="mm")
                for ch in range(4):
                    nc.tensor.matmul(
                        ps[:],
                        xdr[:, ch, :, kt * 128:(kt + 1) * 128],
                        wv[:, ch, :, hg * 512:(hg + 1) * 512],
                        start=(ch == 0), stop=(ch == 3), perf_mode=DR)
                    yield
                if hg == 0:
                    nc.vector.tensor_copy(
                        va[:, kt, 0:8, 0:DK],
                        ps[:].rearrange("p (h c) -> p h c", h=8))
                else:
                    nc.scalar.activation(
                        out=va[:, kt, 8:16, 0:DK],
                        in_=ps[:].rearrange("p (h c) -> p h c", h=8),
                        func=A.Copy)

            def groups_gen(worklist):
                for item in worklist:
                    if item[0] == "k":
                        yield from k_half(item[1], item[2], item[3])
                    elif item[0] == "q":
                        yield from q_half(item[1], item[2], item[3])
                    else:
                        yield from v_group(item[1], item[2])

            # ---------------- c_block: o-proj + residual + LN ----------------
            def c_block(tt, alt=False, ln_on_scalar=False, tail=False):
                x_t = xr.tile([128, D], BF16, tag="xres")
                nc.sync.dma_start(x_t[:], xmy_d.ap()[tt * 128:(tt + 1) * 128, :])
                if alt == 2:
                    ps_e0 = ps_o.tile([128, 512], F32, tag="o")
                    ps_e1 = ps_o.tile([128, 512], F32, tag="o")
                    pss = [ps_e0[:], ps_e1[:]]
                elif alt:
                    big_ps = ps_sc.tile([128, 1024], F32, tag="sc")
                    pss = [big_ps[:, 0:512], big_ps[:, 512:1024]]
                else:
                    ps_e0 = ps_mm.tile([128, 512], F32, tag="mm")
                    ps_e1 = ps_mm.tile([128, 512], F32, tag="mm")
                    pss = [ps_e0[:], ps_e1[:]]
                for eh in range(2):
                    for hp in range(8):
                        nc.tensor.matmul(
                            pss[eh], oT[:, hp, tt * 128:(tt + 1) * 128],
                            wo[:, hp, eh * 512:(eh + 1) * 512],
                            start=(hp == 0), stop=(hp == 7 and not tail))
                        yield
                y_sb = ysb.tile([128, D], F32, tag="ysb")
                stats = lnp.tile([128, 2, nc.vector.BN_STATS_DIM], F32, tag="st")
                if tail:
                    # residual add on PE (identity matmul); stats from psum
                    for eh in range(2):
                        nc.tensor.matmul(
                            pss[eh], ident[:],
                            x_t[:, eh * 512:(eh + 1) * 512],
                            start=False, stop=True)
                    for eh in range(2):
                        nc.vector.bn_stats(stats[:, eh, :], pss[eh])
                else:
                    for eh in range(2):
                        nc.vector.tensor_add(
                            y_sb[:, eh * 512:(eh + 1) * 512],
                            pss[eh], x_t[:, eh * 512:(eh + 1) * 512])
                    if apply_bias:
                        nc.vector.tensor_add(y_sb[:], y_sb[:], bo_bc[:])
                    nc.vector.bn_stats(stats[:, 0, :], y_sb[:, 0:512])
                    nc.vector.bn_stats(stats[:, 1, :], y_sb[:, 512:1024])
                mv = lnp.tile([128, nc.vector.BN_AGGR_DIM], F32, tag="mv")
                nc.vector.bn_aggr(mv[:], stats[:])
                lnv = lnp.tile([128, 1], F32, tag="lnv")
                rstd = lnp.tile([128, 1], F32, tag="rstd")
                nc.scalar.activation(
                    out=lnv[:], in_=mv[:, 1:2],
                    func=A.Ln, bias=eps_t[:], scale=1.0)
                nc.scalar.activation(
                    out=rstd[:], in_=lnv[:], func=A.Exp, scale=-0.5)
                if ln_on_scalar:
                    nmu = lnp.tile([128, 1], F32, tag="nmu")
                    nc.vector.tensor_scalar(
                        out=nmu[:], in0=mv[:, 0:1], scalar1=rstd[:],
                        scalar2=-1.0, op0=mybir.AluOpType.mult,
                        op1=mybir.AluOpType.mult)
                    for eh in range(2):
                        nc.scalar.activation(
                            out=y_sb[:, eh * 512:(eh + 1) * 512],
                            in_=y_sb[:, eh * 512:(eh + 1) * 512],
                            func=A.Identity, scale=rstd[:], bias=nmu[:])
                        if not apply_gb:
                            nc.sync.dma_start(
                                y_d.ap()[tt * 128:(tt + 1) * 128,
                                         eh * 512:(eh + 1) * 512],
                                y_sb[:, eh * 512:(eh + 1) * 512])
                else:
                    nc.gpsimd.tensor_scalar(
                        out=y_sb[:], in0=y_sb[:],
                        scalar1=mv[:, 0:1], scalar2=rstd[:],
                        op0=mybir.AluOpType.subtract,
                        op1=mybir.AluOpType.mult)
                if apply_gb:
                    nc.gpsimd.tensor_mul(y_sb[:], y_sb[:], g_bc[:])
                    nc.gpsimd.tensor_add(y_sb[:], y_sb[:], b_bc[:])
                if not ln_on_scalar or apply_gb:
                    nc.sync.dma_start(
                        y_d.ap()[tt * 128:(tt + 1) * 128, :], y_sb[:])

            def pull(gen, n):
                if gen is None:
                    return None
                for _ in range(n):
                    try:
                        next(gen)
                    except StopIteration:
                        return None
                return gen

            def drain(gen):
                if gen is not None:
                    for _ in gen:
                        pass

            # ---------------- prelude projections ----------------
            emit_now = ([("k", 0, 0, ih) for ih in range(2)]
                        + [("k", 0, 1, ih) for ih in range(2)]
                        + [("q", 0, 0, ih) for ih in range(2)]
                        + [("v", kt, 0) for kt in range(4)])
            drain(groups_gen(emit_now))

            # deadline-ordered weave (block h0 pulls 10/unit): V-hg0 paced 2
            # groups/unit just ahead of attn@V, K tg2/tg3 slotted to land
            # before their first scores; then quads 1-3, V-hg1 (before block
            # h8), Q-qg1.
            weave_a = ([("v", 4, 0), ("v", 5, 0)]
                       + [("k", 0, 2, ih) for ih in range(2)]
                       + [("v", 6, 0), ("v", 7, 0), ("v", 8, 0), ("v", 9, 0)]
                       + [("k", 0, 3, ih) for ih in range(2)]
                       + [("v", kt, 0) for kt in range(10, NKT)]
                       + [it for quad in (1, 2, 3) for it in
                          [("k", quad, tg, ih)
                           for tg in range(NTG) for ih in range(2)]
                          + [("q", quad, 0, ih) for ih in range(2)]]
                       + [("v", kt, 1) for kt in range(NKT)]
                       + [("q", quad, 1, ih)
                          for quad in range(4) for ih in range(2)])

            # ---------------- attention blocks ----------------
            # Software-pipelined one unit deep on scores AND on attn@V: PE
            # never waits for the current unit's exp, and a third score slot
            # (the two mm banks) rotates in once the weave is drained so the
            # two exp engines overlap fully.
            o_live = {}
            weave_box = [groups_gen(weave_a)]
            cgen_box = [None]
            cqueue = []

            def wpull(n):
                if weave_box[0] is not None:
                    weave_box[0] = pull(weave_box[0], n)
                    if weave_box[0] is None and cqueue:
                        cgen_box[0] = c_block(cqueue.pop(0))
                elif cgen_box[0] is not None:
                    cgen_box[0] = pull(cgen_box[0], n)
                    if cgen_box[0] is None and cqueue:
                        cgen_box[0] = c_block(cqueue.pop(0))

            units = [(qg, h, ktp)
                     for qg in range(2) for h in range(H)
                     for ktp in range(NKT // 2)]

            def dve_unit(u):
                qg, h, ktp = u
                if qg == 0 and 8 <= h < 12:
                    return ktp in (1, 3, 5, 7)
                return ktp in DVE_KTPS

            def alloc_sc(u):
                free_mm = (weave_box[0] is None and cgen_box[0] is None
                           and not cqueue)
                if free_mm and dve_unit(u):
                    ta = ps_mm.tile([128, 512], F32, tag="mm")
                    tb = ps_mm.tile([128, 512], F32, tag="mm")
                    return (ta, tb)
                sc2 = ps_sc.tile([128, 1024], F32, tag="sc")
                return (sc2,)

            def emit_scores(u, sct):
                qg, h, ktp = u
                quad, sl = h // 4, h % 4
                p0 = 32 * sl
                for j in range(2):
                    kt = 2 * ktp + j
                    if len(sct) == 1:
                        out = sct[0][:, j * 512:(j + 1) * 512]
                    else:
                        out = sct[j][:]
                    nc.tensor.matmul(
                        out,
                        kS[p0:p0 + 32, quad, kt // 4, :,
                           (kt % 4) * 128:(kt % 4 + 1) * 128],
                        qS[p0:p0 + 32, quad, qg, :, :],
                        start=True, stop=True, perf_mode=DR,
                        tile_position=(p0, 0))

            def emit_exp(u, sct):
                qg, h, ktp = u
                if dve_unit(u):
                    et = ebp.tile([128, 2, 512], I16, tag="eb")
                    if len(sct) == 1:
                        nc.vector.tensor_scalar(
                            out=et[:].rearrange("p i t -> p (i t)"),
                            in0=sct[0][:], scalar1=BT_A, scalar2=BT_B,
                            op0=mybir.AluOpType.mult,
                            op1=mybir.AluOpType.add)
                    else:
                        for j in range(2):
                            nc.vector.tensor_scalar(
                                out=et[:, j, :],
                                in0=sct[j][:], scalar1=BT_A, scalar2=BT_B,
                                op0=mybir.AluOpType.mult,
                                op1=mybir.AluOpType.add)
                else:
                    et = e2p.tile([128, 2, 512], E4, tag="e2")
                    if len(sct) == 1:
                        nc.scalar.activation(
                            out=et[:].rearrange("p i t -> p (i t)"),
                            in_=sct[0][:], func=A.Exp,
                            scale=SEFF, bias=bias_t[:])
                    else:
                        for j in range(2):
                            nc.scalar.activation(
                                out=et[:, j, :], in_=sct[j][:], func=A.Exp,
                                scale=SEFF, bias=bias_t[:])
                return et

            def emit_attnv(u, et):
                qg, h, ktp = u
                if ktp == 0:
                    o_ps = ps_o.tile([128, 512], F32, tag="o")
                    o_live[h] = o_ps
                else:
                    o_ps = o_live[h]
                first = ktp == 0
                last_ktp = ktp == NKT // 2 - 1
                if dve_unit(u):
                    for j in range(2):
                        for qt in range(4):
                            nc.tensor.matmul(
                                o_ps[:, qt * 65:qt * 65 + 65],
                                et[:, j, qt * 128:(qt + 1) * 128]
                                .bitcast(BF16),
                                va[:, 2 * ktp + j, h, :],
                                start=(first and j == 0 and qt == 0),
                                stop=(last_ktp and j == 1 and qt == 3))
                else:
                    for qt in range(4):
                        nc.tensor.matmul(
                            o_ps[:, qt * 65:qt * 65 + 65],
                            et[:, :, qt * 128:(qt + 1) * 128],
                            va[:, 2 * ktp:2 * ktp + 2, h, :],
                            start=(first and qt == 0),
                            stop=(last_ktp and qt == 3),
                            perf_mode=DR)
                if last_ktp:
                    finish_block(qg, h)

            def finish_block(qg, h):
                if h % 2 == 1:
                    hp = h // 2
                    opE, opO = o_live.pop(h - 1), o_live.pop(h)
                    rec = lnp.tile([128, 8], F32, tag="rec")
                    for hh, op in ((0, opE), (1, opO)):
                        nc.vector.reciprocal(
                            rec[:, hh * 4:(hh + 1) * 4],
                            op[:, 0:260].rearrange(
                                "p (q c) -> p q c", c=65)[:, :, DK])
                    for qt in range(4):
                        onrm = onp.tile([128, 128], BF16, tag="onrm")
                        nc.scalar.activation(
                            out=onrm[:, 0:64],
                            in_=opE[:, qt * 65:qt * 65 + 64],
                            func=A.Copy, scale=rec[:, qt:qt + 1])
                        nc.vector.tensor_scalar(
                            out=onrm[:, 64:128],
                            in0=opO[:, qt * 65:qt * 65 + 64],
                            scalar1=rec[:, 4 + qt:5 + qt], scalar2=None,
                            op0=mybir.AluOpType.mult)
                        nc.sync.dma_start(
                            oT[:, hp, qg * 512 + qt * 128:
                               qg * 512 + (qt + 1) * 128],
                            onrm[:], transpose=True)
                if h == 0 and qg == 1:
                    cqueue.extend([0, 1, 2, 3])
                    if weave_box[0] is None and cgen_box[0] is None:
                        cgen_box[0] = c_block(cqueue.pop(0))

            sc_next = alloc_sc(units[0])
            emit_scores(units[0], sc_next)
            att_pend = None

            for idx, u in enumerate(units):
                sc2 = sc_next
                if idx + 1 < len(units):
                    sc_next = alloc_sc(units[idx + 1])
                    emit_scores(units[idx + 1], sc_next)
                et = emit_exp(u, sc2)
                half_pull = 5 if idx < 8 else 2
                wpull(half_pull)
                if att_pend is not None:
                    emit_attnv(*att_pend)
                att_pend = (u, et)
                wpull(half_pull)
            emit_attnv(*att_pend)

            # tail: remaining c_blocks (qg0 leftovers + all of qg1)
            drain(weave_box[0])
            drain(cgen_box[0])
            for tt in cqueue:
                drain(c_block(tt))
            for a, b in ((4, 5), (6, 7)):
                ga = c_block(a, ln_on_scalar=True)
                gb_ = c_block(b, alt=True, ln_on_scalar=True)
                while ga is not None or gb_ is not None:
                    ga = pull(ga, 16)
                    gb_ = pull(gb_, 16)

    nc.compile()
    return nc


def _prep_shared(w_q, w_k, w_v, w_o):
    """Host-side weight layouts (shared across cores)."""
    import ml_dtypes
    bf16 = ml_dtypes.bfloat16
    e4 = ml_dtypes.float8_e4m3

    def qk_perm(wT):
        # wT: [c=1024, d-cols=1024] scaled. Column order for quad/ih/slot:
        # col(quad, ih, ptil) = head(4*quad + ptil//32), d = ih*32 + ptil%32
        w = wT.reshape(1024, 16, 64)                      # [c, head, d]
        out = np.empty((1024, 4, 2, 128), np.float32)
        for quad in range(4):
            for ih in range(2):
                for sl in range(4):
                    hsel = 4 * quad + sl
                    out[:, quad, ih, sl * 32:(sl + 1) * 32] = \
                        w[:, hsel, ih * 32:(ih + 1) * 32]
        # rows c -> [ch, i, p]: c = ch*256 + i*128 + p
        out = out.reshape(4, 2, 128, 4, 2, 128)           # ch i p quad ih col
        out = out.transpose(2, 3, 0, 4, 1, 5)             # p quad ch ih i col
        # dram layout [128, ch, i, 1024-cols(quad,ih,128)]
        out = out.transpose(0, 2, 4, 1, 3, 5)             # p ch i quad ih col
        return np.ascontiguousarray(
            out.reshape(128, 4, 2, 1024)).astype(e4).reshape(128, -1)

    def v_perm(wT):
        # plain col order; rows c -> [ch, i, p]
        out = wT.reshape(4, 2, 128, 1024).transpose(2, 0, 1, 3)
        return np.ascontiguousarray(
            out.reshape(128, 4, 2, 1024)).astype(e4).reshape(128, -1)

    wqT = np.ascontiguousarray(w_q.T) * WS
    wkT = np.ascontiguousarray(w_k.T) * WS
    wvT = np.ascontiguousarray(w_v.T) * WS
    # wo tile [p, hp, e] = w_o.T[hp*128 + p, e]
    woT = np.ascontiguousarray(
        w_o.T.reshape(8, 128, 1024).transpose(1, 0, 2)).astype(bf16)
    return {
        "wq": qk_perm(wqT), "wk": qk_perm(wkT), "wv": v_perm(wvT),
        "wo": woT.reshape(128, -1),
    }


def kernel(x, w_q, w_k, w_v, w_o, b_o, ln_g, ln_b):
    import ml_dtypes
    bf16 = ml_dtypes.bfloat16
    e4 = ml_dtypes.float8_e4m3

    x = np.asarray(x, dtype=np.float32)
    w_q = np.asarray(w_q, dtype=np.float32)
    w_k = np.asarray(w_k, dtype=np.float32)
    w_v = np.asarray(w_v, dtype=np.float32)
    w_o = np.asarray(w_o, dtype=np.float32)
    b_o = np.asarray(b_o, dtype=np.float32)
    ln_g = np.asarray(ln_g, dtype=np.float32)
    ln_b = np.asarray(ln_b, dtype=np.float32)

    apply_gb = not (np.all(ln_g == 1.0) and np.all(ln_b == 0.0))
    apply_bias = bool(np.any(b_o != 0.0))
    key = (apply_gb, apply_bias)
    if key not in _CACHE:
        _CACHE[key] = build(apply_gb, apply_bias)
    nc = _CACHE[key]

    shared = _prep_shared(w_q, w_k, w_v, w_o)
    gb = np.stack([ln_g, ln_b]).astype(np.float32)
    ident_np = np.eye(128, dtype=np.float32).astype(bf16)
    bo = np.ascontiguousarray(b_o.reshape(1, D))

    in_maps = []
    for c in range(N_CORES):
        b = c // 2
        half = c % 2
        xb = x[b]
        if half == 1:
            xb = np.roll(xb, -TOK, axis=0)
        # xdr[p, ch, i, t] = xb[t, ch*256 + i*128 + p]
        xdr = xb.T.reshape(4, 2, 128, S).transpose(2, 0, 1, 3)
        xdr = np.ascontiguousarray(xdr).astype(e4).reshape(128, -1)
        xmy = np.ascontiguousarray(xb[0:TOK]).astype(bf16)
        in_maps.append({
            "xdr": xdr, "xmy": xmy, "bo": bo, "gb": gb,
            "ident": ident_np, **shared,
        })

    res = bass_utils.run_bass_kernel_spmd(nc, in_maps,
                                          core_ids=list(range(N_CORES)))
    y = np.stack([res.results[c]["y"] for c in range(N_CORES)])
    return y.reshape(B, S, D)


# revision 79
# speedup vs baseline: 1.5222x; 1.0003x over previous
"""Multi-head self-attention + residual + LayerNorm on 8 Trainium2 NeuronCores.

Problem: B=4, S=2048, D=1024, H=16, d_k=64, fp32.

Sharding: token-parallel, zero collectives. Core c owns batch b=c//2 and a
1024-query-token half of it (host rotates tokens so own queries are rows
0..1023; softmax/attn@V are permutation-invariant over keys). Each core
recomputes K/V for its full batch.

v9 (256us vs 386us bf16 baseline): fp8 DoubleRow matmuls + engine-balanced
softmax.
 - All projection/score/attnV operands are fp8e4m3 (weights host-scaled x16,
   descale folded into the exp scale and the 16.0 ones-column). DoubleRow
   contracts 256 deep at 0.5 cycles/row: projections cost 1/4, scores 1/2 of
   bf16. Scores put d_k=64 on [32 partitions x 2]; four heads share the
   partition dim via 32-row slots (explicit tile_position (32s, 0)).
 - exp carries bias -4.5 (cancels in softmax, keeps e inside e4m3 range;
   raw scores reach +-9). The work splits across engines per kt-pair:
   ScalarE activation-Exp -> e4m3 (feeds DoubleRow attn@V), DVE Schraudolph
   bit-trick (one tensor_scalar f32->int16, bitcast bf16, feeds mixed
   bf16xfp8 attn@V). The split ratio adapts per phase to DVE's eviction
   load. Ones column gives denominators in psum col 64 for free.
 - Engine schedule: units (head, qg, kt-pair) run software-pipelined one
   unit deep on scores AND one unit deep on attn@V, so PE never blocks on
   the current exp and the two exp engines overlap. Score psums rotate over
   2x[128,1024] plus, once the projection weave drains, the two [128,512]
   ps_mm banks (split exps) - effectively 3-deep.
 - Projections are 4-matmul half-groups on the 2-buffer ps_mm pool
   (ping-pong: group N+1 computes while N evicts). Evictions balance:
   quad0/V-hg0 prep on ScalarE/DVE, later K/Q on DVE, V-hg1 on ScalarE.
   Each group's eviction is emitted before its last yield so woven
   consumers can never be emitted ahead of the data they read.
 - o_nrm: ScalarE Copy(scale=1/den) for head-even, DVE tensor_scalar for
   head-odd, into one [128,(hh,64)] bf16 tile; SBUF->SBUF DMA XBAR
   transpose writes oT directly (no PE transposes, no separate eviction).
 - o-proj/residual/LN stay bf16/f32: o-proj in fp8 fails the 2e-2 error
   budget. LN: bn_stats/aggr + residual adds on DVE, rstd=exp(-.5 ln(var+eps))
   on ScalarE, affine on GPSIMD (woven) or ScalarE Identity (tail, with
   split half stores). Exp/Ln/Copy/Identity pinned to one activation table.

Measured rel err vs f32 reference: 1.17e-2 (gate 2e-2); error budget is
dominated by e4m3 quantization of Q/K/V/e, validated in sim_numerics.py.
"""

import numpy as np

import concourse.mybir as mybir
import concourse.tile as tile
from concourse import bacc
from concourse import bass_utils

F32 = mybir.dt.float32
BF16 = mybir.dt.bfloat16
E4 = mybir.dt.float8e4
I16 = mybir.dt.int16
DR = mybir.MatmulPerfMode.DoubleRow

B, S, D, H, DK = 4, 2048, 1024, 16, 64
N_CORES = 8
TOK = (B * S) // N_CORES            # 1024 query tokens per core
NKT = S // 128                      # 16 k-tiles per batch
NTG = S // 512                      # 4 token groups per batch
EPS = 1e-5
WS = 16.0                           # host weight upscale before fp8 quant
SEFF = 0.125 / (WS * WS)            # exp scale on raw psum scores
EBIAS = -4.5                        # exp bias (cancels in softmax)
LOG2E = 1.4426950408889634
BT_A = float(SEFF * LOG2E * 128.0)  # bit-trick multiplier
BT_B = float((127.0 - 0.0579) * 128.0 + EBIAS * LOG2E * 128.0)

# kt-pairs whose exp runs on DVE (bit-trick); rest on ScalarE
DVE_KTPS = (2, 4, 6)

_CACHE = {}


def build(apply_gb: bool, apply_bias: bool):
    nc = bacc.Bacc("TRN2", target_bir_lowering=False, debug=False,
                   num_devices=N_CORES)
    # Pin every ScalarE function we use (Exp, Ln, Copy) to the one table that
    # holds them all, so the activation table is loaded exactly once.
    from concourse.hw_specs import get_activation_tables
    A = mybir.ActivationFunctionType
    tabs = get_activation_tables(nc.m.arch)
    for name, s in tabs.items():
        if name != "natural_log_exp_and_others":
            s.discard(A.Exp)
            s.discard(A.Ln)
            s.discard(A.Copy)
            s.discard(A.Identity)

    xdr_d = nc.dram_tensor("xdr", [128, 4 * 2 * S], E4, kind="ExternalInput")
    wq_d = nc.dram_tensor("wq", [128, 4 * 2 * 1024], E4, kind="ExternalInput")
    wk_d = nc.dram_tensor("wk", [128, 4 * 2 * 1024], E4, kind="ExternalInput")
    wv_d = nc.dram_tensor("wv", [128, 4 * 2 * 1024], E4, kind="ExternalInput")
    wo_d = nc.dram_tensor("wo", [128, 8 * 1024], BF16, kind="ExternalInput")
    ident_d = nc.dram_tensor("ident", [128, 128], BF16, kind="ExternalInput")
    xmy_d = nc.dram_tensor("xmy", [TOK, D], BF16, kind="ExternalInput")
    bo_d = nc.dram_tensor("bo", [1, D], F32, kind="ExternalInput")
    gb_d = nc.dram_tensor("gb", [2, D], F32, kind="ExternalInput")
    y_d = nc.dram_tensor("y", [TOK, D], F32, kind="ExternalOutput")

    with tile.TileContext(nc) as tc:
        with (
            tc.tile_pool(name="big", bufs=1) as big,
            tc.tile_pool(name="e2p", bufs=6) as e2p,
            tc.tile_pool(name="ebp", bufs=6) as ebp,
            tc.tile_pool(name="onp", bufs=8) as onp,
            tc.tile_pool(name="xr", bufs=4) as xr,
            tc.tile_pool(name="ysb", bufs=4) as ysb,
            tc.tile_pool(name="ln", bufs=6) as lnp,
            tc.tile_pool(name="small", bufs=1) as small,
            tc.tile_pool(name="ps_sc", bufs=2, space="PSUM") as ps_sc,
            tc.tile_pool(name="ps_o", bufs=2, space="PSUM") as ps_o,
            tc.tile_pool(name="ps_mm", bufs=2, space="PSUM") as ps_mm,
        ):
            xdr = big.tile([128, 4, 2, S], E4, tag="xdr")            # 16K/p
            wq = big.tile([128, 4, 2, 1024], E4, tag="wq")           # 8K/p
            wk = big.tile([128, 4, 2, 1024], E4, tag="wk")
            wv = big.tile([128, 4, 2, 1024], E4, tag="wv")
            wo = big.tile([128, 8, 1024], BF16, tag="wo")            # 16K/p
            # K: [quad, tg, i, t]; Q: [quad, qg, i, t]
            kS = big.tile([128, 4, NTG, 2, 512], E4, tag="kS")       # 16K/p
            qS = big.tile([128, 4, 2, 2, 512], E4, tag="qS")         # 8K/p
            # va: [t-part, kt, head, dk+1]; col dk holds 16.0
            va = big.tile([128, NKT, H, DK + 1], E4, tag="va")       # 16.25K/p
            oT = big.tile([128, 8, TOK], BF16, tag="oT")             # 16K/p

            nc.vector.memset(va[:, :, :, DK:DK + 1], WS)
            bias_t = small.tile([128, 1], F32, tag="bias")
            nc.vector.memset(bias_t[:], EBIAS)
            eps_t = small.tile([128, 1], F32, tag="eps")
            nc.vector.memset(eps_t[:], EPS)

            # ---------------- DMA loads (first-use order) ----------------
            # xdr feeds every projection; quad0 K/Q cols + hg0 V cols next.
            xdr_v = xdr_d.ap().rearrange("p (c i t) -> p c i t", c=4, i=2)
            wkv = wk_d.ap().rearrange("p (c i t) -> p c i t", c=4, i=2)
            wqv = wq_d.ap().rearrange("p (c i t) -> p c i t", c=4, i=2)
            wvv = wv_d.ap().rearrange("p (c i t) -> p c i t", c=4, i=2)
            nc.sync.dma_start(xdr[:, :, :, 0:512], xdr_v[:, :, :, 0:512])
            nc.sync.dma_start(wk[:, :, :, 0:256], wkv[:, :, :, 0:256])
            nc.sync.dma_start(wq[:, :, :, 0:256], wqv[:, :, :, 0:256])
            nc.sync.dma_start(wv[:, :, :, 0:512], wvv[:, :, :, 0:512])
            nc.sync.dma_start(xdr[:, :, :, 512:1024], xdr_v[:, :, :, 512:1024])
            nc.sync.dma_start(xdr[:, :, :, 1024:2048], xdr_v[:, :, :, 1024:2048])
            nc.sync.dma_start(wk[:, :, :, 256:1024], wkv[:, :, :, 256:1024])
            nc.sync.dma_start(wq[:, :, :, 256:1024], wqv[:, :, :, 256:1024])
            nc.sync.dma_start(wv[:, :, :, 512:1024], wvv[:, :, :, 512:1024])
            nc.sync.dma_start(
                wo[:], wo_d.ap().rearrange("p (h t) -> p h t", h=8))
            ident = big.tile([128, 128], BF16, tag="ident")
            nc.sync.dma_start(ident[:], ident_d.ap())
            if apply_bias:
                bo_bc = small.tile([128, D], F32, tag="bobc")
                nc.sync.dma_start(bo_bc[:],
                                  bo_d.ap()[0:1, :].broadcast_to((128, D)))
            if apply_gb:
                g_bc = small.tile([128, D], F32, tag="gbc")
                b_bc = small.tile([128, D], F32, tag="bbc")
                nc.sync.dma_start(g_bc[:],
                                  gb_d.ap()[0:1, :].broadcast_to((128, D)))
                nc.sync.dma_start(b_bc[:],
                                  gb_d.ap()[1:2, :].broadcast_to((128, D)))

            # ---------------- projection group emitters ----------------
            # every group is a 4-matmul chain into one [128,512] psum (one
            # bank); the ps_mm pool's two buffers ping-pong so group N+1's
            # matmuls overlap group N's eviction.
            def k_half(quad, tg, ih):
                ps = ps_mm.tile([128, 512], F32, tag="mm")
                for ch in range(4):
                    nc.tensor.matmul(
                        ps[:],
                        wk[:, ch, :, quad * 256 + ih * 128:
                           quad * 256 + (ih + 1) * 128],
                        xdr[:, ch, :, tg * 512:(tg + 1) * 512],
                        start=(ch == 0), stop=(ch == 3), perf_mode=DR)
                    if ch < 3:
                        yield
                if quad == 0:
                    nc.scalar.activation(
                        out=kS[:, quad, tg, ih, :], in_=ps[:], func=A.Copy)
                else:
                    nc.vector.tensor_copy(kS[:, quad, tg, ih, :], ps[:])
                yield

            def q_half(quad, qg, ih):
                ps = ps_mm.tile([128, 512], F32, tag="mm")
                for ch in range(4):
                    nc.tensor.matmul(
                        ps[:],
                        wq[:, ch, :, quad * 256 + ih * 128:
                           quad * 256 + (ih + 1) * 128],
                        xdr[:, ch, :, qg * 512:(qg + 1) * 512],
                        start=(ch == 0), stop=(ch == 3), perf_mode=DR)
                    if ch < 3:
                        yield
                if quad == 0 and qg == 0:
                    nc.scalar.activation(
                        out=qS[:, quad, qg, ih, :], in_=ps[:], func=A.Copy)
                else:
                    nc.vector.tensor_copy(qS[:, quad, qg, ih, :], ps[:])

            def v_group(kt, hg):
                ps = ps_mm.tile([128, 512], F32, tag="mm")
                for ch in range(4):
                    nc.tensor.matmul(
                        ps[:],
                        xdr[:, ch, :, kt * 128:(kt + 1) * 128],
                        wv[:, ch, :, hg * 512:(hg + 1) * 512],
                        start=(ch == 0), stop=(ch == 3), perf_mode=DR)
                    yield
                if hg == 0:
                    nc.vector.tensor_copy(
                        va[:, kt, 0:8, 0:DK],
                        ps[:].rearrange("p (h c) -> p h c", h=8))
                else:
                    nc.scalar.activation(
                        out=va[:, kt, 8:16, 0:DK],
                        in_=ps[:].rearrange("p (h c) -> p h c", h=8),
                        func=A.Copy)

            def groups_gen(worklist):
                for item in worklist:
                    if item[0] == "k":
                        yield from k_half(item[1], item[2], item[3])
                    elif item[0] == "q":
                        yield from q_half(item[1], item[2], item[3])
                    else:
                        yield from v_group(item[1], item[2])

            # ---------------- c_block: o-proj + residual + LN ----------------
            def c_block(tt, alt=False, ln_on_scalar=False, tail=False):
                x_t = xr.tile([128, D], BF16, tag="xres")
                nc.sync.dma_start(x_t[:], xmy_d.ap()[tt * 128:(tt + 1) * 128, :])
                if alt == 2:
                    ps_e0 = ps_o.tile([128, 512], F32, tag="o")
                    ps_e1 = ps_o.tile([128, 512], F32, tag="o")
                    pss = [ps_e0[:], ps_e1[:]]
                elif alt:
                    big_ps = ps_sc.tile([128, 1024], F32, tag="sc")
                    pss = [big_ps[:, 0:512], big_ps[:, 512:1024]]
                else:
                    ps_e0 = ps_mm.tile([128, 512], F32, tag="mm")
                    ps_e1 = ps_mm.tile([128, 512], F32, tag="mm")
                    pss = [ps_e0[:], ps_e1[:]]
                for eh in range(2):
                    for hp in range(8):
                        nc.tensor.matmul(
                            pss[eh], oT[:, hp, tt * 128:(tt + 1) * 128],
                            wo[:, hp, eh * 512:(eh + 1) * 512],
                            start=(hp == 0), stop=(hp == 7 and not tail))
                        yield
                y_sb = ysb.tile([128, D], F32, tag="ysb")
                stats = lnp.tile([128, 2, nc.vector.BN_STATS_DIM], F32, tag="st")
                if tail:
                    # residual add on PE (identity matmul); stats from psum
                    for eh in range(2):
                        nc.tensor.matmul(
                            pss[eh], ident[:],
                            x_t[:, eh * 512:(eh + 1) * 512],
                            start=False, stop=True)
                    for eh in range(2):
                        nc.vector.bn_stats(stats[:, eh, :], pss[eh])
                else:
                    for eh in range(2):
                        nc.vector.tensor_add(
                            y_sb[:, eh * 512:(eh + 1) * 512],
                            pss[eh], x_t[:, eh * 512:(eh + 1) * 512])
                    if apply_bias:
                        nc.vector.tensor_add(y_sb[:], y_sb[:], bo_bc[:])
                    nc.vector.bn_stats(stats[:, 0, :], y_sb[:, 0:512])
                    nc.vector.bn_stats(stats[:, 1, :], y_sb[:, 512:1024])
                mv = lnp.tile([128, nc.vector.BN_AGGR_DIM], F32, tag="mv")
                nc.vector.bn_aggr(mv[:], stats[:])
                lnv = lnp.tile([128, 1], F32, tag="lnv")
                rstd = lnp.tile([128, 1], F32, tag="rstd")
                nc.scalar.activation(
                    out=lnv[:], in_=mv[:, 1:2],
                    func=A.Ln, bias=eps_t[:], scale=1.0)
                nc.scalar.activation(
                    out=rstd[:], in_=lnv[:], func=A.Exp, scale=-0.5)
                if ln_on_scalar:
                    nmu = lnp.tile([128, 1], F32, tag="nmu")
                    nc.vector.tensor_scalar(
                        out=nmu[:], in0=mv[:, 0:1], scalar1=rstd[:],
                        scalar2=-1.0, op0=mybir.AluOpType.mult,
                        op1=mybir.AluOpType.mult)
                    for eh in range(2):
                        nc.scalar.activation(
                            out=y_sb[:, eh * 512:(eh + 1) * 512],
                            in_=y_sb[:, eh * 512:(eh + 1) * 512],
                            func=A.Identity, scale=rstd[:], bias=nmu[:])
                        if not apply_gb:
                            nc.sync.dma_start(
                                y_d.ap()[tt * 128:(tt + 1) * 128,
                                         eh * 512:(eh + 1) * 512],
                                y_sb[:, eh * 512:(eh + 1) * 512])
                else:
                    nc.gpsimd.tensor_scalar(
                        out=y_sb[:], in0=y_sb[:],
                        scalar1=mv[:, 0:1], scalar2=rstd[:],
                        op0=mybir.AluOpType.subtract,
                        op1=mybir.AluOpType.mult)
                if apply_gb:
                    nc.gpsimd.tensor_mul(y_sb[:], y_sb[:], g_bc[:])
                    nc.gpsimd.tensor_add(y_sb[:], y_sb[:], b_bc[:])
                if not ln_on_scalar or apply_gb:
                    nc.sync.dma_start(
                        y_d.ap()[tt * 128:(tt + 1) * 128, :], y_sb[:])

            def pull(gen, n):
                if gen is None:
                    return None
                for _ in range(n):
                    try:
                        next(gen)
                    except StopIteration:
                        return None
                return gen

            def drain(gen):
                if gen is not None:
                    for _ in gen:
                        pass

            # ---------------- prelude projections ----------------
            emit_now = ([("k", 0, 0, ih) for ih in range(2)]
                        + [("k", 0, 1, ih) for ih in range(2)]
                        + [("q", 0, 0, ih) for ih in range(2)]
                        + [("v", kt, 0) for kt in range(4)])
            drain(groups_gen(emit_now))

            # deadline-ordered weave (block h0 pulls 10/unit): V-hg0 paced 2
            # groups/unit just ahead of attn@V, K tg2/tg3 slotted to land
            # before their first scores; then quads 1-3, V-hg1 (before block
            # h8), Q-qg1.
            weave_a = ([("v", 4, 0), ("v", 5, 0)]
                       + [("k", 0, 2, ih) for ih in range(2)]
                       + [("v", 6, 0), ("v", 7, 0), ("v", 8, 0), ("v", 9, 0)]
                       + [("k", 0, 3, ih) for ih in range(2)]
                       + [("v", kt, 0) for kt in range(10, NKT)]
                       + [it for quad in (1, 2, 3) for it in
                          [("k", quad, tg, ih)
                           for tg in range(NTG) for ih in range(2)]
                          + [("q", quad, 0, ih) for ih in range(2)]]
                       + [("v", kt, 1) for kt in range(NKT)]
                       + [("q", quad, 1, ih)
                          for quad in range(4) for ih in range(2)])

            # ---------------- attention blocks ----------------
            # Software-pipelined one unit deep on scores AND on attn@V: PE
            # never waits for the current unit's exp, and a third score slot
            # (the two mm banks) rotates in once the weave is drained so the
            # two exp engines overlap fully.
            o_live = {}
            weave_box = [groups_gen(weave_a)]
            cgen_box = [None]
            cqueue = []

            def wpull(n):
                if weave_box[0] is not None:
                    weave_box[0] = pull(weave_box[0], n)
                    if weave_box[0] is None and cqueue:
                        cgen_box[0] = c_block(cqueue.pop(0))
                elif cgen_box[0] is not None:
                    cgen_box[0] = pull(cgen_box[0], n)
                    if cgen_box[0] is None and cqueue:
                        cgen_box[0] = c_block(cqueue.pop(0))

            units = [(qg, h, ktp)
                     for qg in range(2) for h in range(H)
                     for ktp in range(NKT // 2)]

            def dve_unit(u):
                qg, h, ktp = u
                if qg == 0 and 8 <= h < 12:
                    return ktp in (1, 3, 5, 7)
                return ktp in DVE_KTPS

            def alloc_sc(u):
                free_mm = (weave_box[0] is None and cgen_box[0] is None
                           and not cqueue)
                if free_mm and dve_unit(u):
                    ta = ps_mm.tile([128, 512], F32, tag="mm")
                    tb = ps_mm.tile([128, 512], F32, tag="mm")
                    return (ta, tb)
                sc2 = ps_sc.tile([128, 1024], F32, tag="sc")
                return (sc2,)

            def emit_scores(u, sct):
                qg, h, ktp = u
                quad, sl = h // 4, h % 4
                p0 = 32 * sl
                for j in range(2):
                    kt = 2 * ktp + j
                    if len(sct) == 1:
                        out = sct[0][:, j * 512:(j + 1) * 512]
                    else:
                        out = sct[j][:]
                    nc.tensor.matmul(
                        out,
                        kS[p0:p0 + 32, quad, kt // 4, :,
                           (kt % 4) * 128:(kt % 4 + 1) * 128],
                        qS[p0:p0 + 32, quad, qg, :, :],
                        start=True, stop=True, perf_mode=DR,
                        tile_position=(p0, 0))

            def emit_exp(u, sct):
                qg, h, ktp = u
                if dve_unit(u):
                    et = ebp.tile([128, 2, 512], I16, tag="eb")
                    if len(sct) == 1:
                        nc.vector.tensor_scalar(
                            out=et[:].rearrange("p i t -> p (i t)"),
                            in0=sct[0][:], scalar1=BT_A, scalar2=BT_B,
                            op0=mybir.AluOpType.mult,
                            op1=mybir.AluOpType.add)
                    else:
                        for j in range(2):
                            nc.vector.tensor_scalar(
                                out=et[:, j, :],
                                in0=sct[j][:], scalar1=BT_A, scalar2=BT_B,
                                op0=mybir.AluOpType.mult,
                                op1=mybir.AluOpType.add)
                else:
                    et = e2p.tile([128, 2, 512], E4, tag="e2")
                    if len(sct) == 1:
                        nc.scalar.activation(
                            out=et[:].rearrange("p i t -> p (i t)"),
                            in_=sct[0][:], func=A.Exp,
                            scale=SEFF, bias=bias_t[:])
                    else:
                        for j in range(2):
                            nc.scalar.activation(
                                out=et[:, j, :], in_=sct[j][:], func=A.Exp,
                                scale=SEFF, bias=bias_t[:])
                return et

            def emit_attnv(u, et):
                qg, h, ktp = u
                if ktp == 0:
                    o_ps = ps_o.tile([128, 512], F32, tag="o")
                    o_live[h] = o_ps
                else:
                    o_ps = o_live[h]
                first = ktp == 0
                last_ktp = ktp == NKT // 2 - 1
                if dve_unit(u):
                    for j in range(2):
                        for qt in range(4):
                            nc.tensor.matmul(
                                o_ps[:, qt * 65:qt * 65 + 65],
                                et[:, j, qt * 128:(qt + 1) * 128]
                                .bitcast(BF16),
                                va[:, 2 * ktp + j, h, :],
                                start=(first and j == 0 and qt == 0),
                                stop=(last_ktp and j == 1 and qt == 3))
                else:
                    for qt in range(4):
                        nc.tensor.matmul(
                            o_ps[:, qt * 65:qt * 65 + 65],
                            et[:, :, qt * 128:(qt + 1) * 128],
                            va[:, 2 * ktp:2 * ktp + 2, h, :],
                            start=(first and qt == 0),
                            stop=(last_ktp and qt == 3),
                            perf_mode=DR)
                if last_ktp:
                    finish_block(qg, h)

            def finish_block(qg, h):
                if h % 2 == 1:
                    hp = h // 2
                    opE, opO = o_live.pop(h - 1), o_live.pop(h)
                    rec = lnp.tile([128, 8], F32, tag="rec")
                    for hh, op in ((0, opE), (1, opO)):
                        nc.vector.reciprocal(
                            rec[:, hh * 4:(hh + 1) * 4],
                            op[:, 0:260].rearrange(
                                "p (q c) -> p q c", c=65)[:, :, DK])
                    for qt in range(4):
                        onrm = onp.tile([128, 128], BF16, tag="onrm")
                        nc.scalar.activation(
                            out=onrm[:, 0:64],
                            in_=opE[:, qt * 65:qt * 65 + 64],
                            func=A.Copy, scale=rec[:, qt:qt + 1])
                        nc.vector.tensor_scalar(
                            out=onrm[:, 64:128],
                            in0=opO[:, qt * 65:qt * 65 + 64],
                            scalar1=rec[:, 4 + qt:5 + qt], scalar2=None,
                            op0=mybir.AluOpType.mult)
                        nc.sync.dma_start(
                            oT[:, hp, qg * 512 + qt * 128:
                               qg * 512 + (qt + 1) * 128],
                            onrm[:], transpose=True)
                if h == 0 and qg == 1:
                    cqueue.extend([0, 1, 2, 3])
                    if weave_box[0] is None and cgen_box[0] is None:
                        cgen_box[0] = c_block(cqueue.pop(0))

            sc_next = alloc_sc(units[0])
            emit_scores(units[0], sc_next)
            att_pend = None

            for idx, u in enumerate(units):
                sc2 = sc_next
                if idx + 1 < len(units):
                    sc_next = alloc_sc(units[idx + 1])
                    emit_scores(units[idx + 1], sc_next)
                et = emit_exp(u, sc2)
                half_pull = 5 if idx < 8 else 2
                wpull(half_pull)
                if att_pend is not None:
                    emit_attnv(*att_pend)
                att_pend = (u, et)
                wpull(half_pull)
            emit_attnv(*att_pend)

            # tail: remaining c_blocks (qg0 leftovers + all of qg1)
            drain(weave_box[0])
            drain(cgen_box[0])
            for tt in cqueue:
                drain(c_block(tt))
            for a, b, aalt in ((4, 5, False), (6, 7, 2)):
                ga = c_block(a, alt=aalt, ln_on_scalar=True)
                gb_ = c_block(b, alt=True, ln_on_scalar=True)
                while ga is not None or gb_ is not None:
                    ga = pull(ga, 16)
                    gb_ = pull(gb_, 16)

    nc.compile()
    return nc


def _prep_shared(w_q, w_k, w_v, w_o):
    """Host-side weight layouts (shared across cores)."""
    import ml_dtypes
    bf16 = ml_dtypes.bfloat16
    e4 = ml_dtypes.float8_e4m3

    def qk_perm(wT):
        # wT: [c=1024, d-cols=1024] scaled. Column order for quad/ih/slot:
        # col(quad, ih, ptil) = head(4*quad + ptil//32), d = ih*32 + ptil%32
        w = wT.reshape(1024, 16, 64)                      # [c, head, d]
        out = np.empty((1024, 4, 2, 128), np.float32)
        for quad in range(4):
            for ih in range(2):
                for sl in range(4):
                    hsel = 4 * quad + sl
                    out[:, quad, ih, sl * 32:(sl + 1) * 32] = \
                        w[:, hsel, ih * 32:(ih + 1) * 32]
        # rows c -> [ch, i, p]: c = ch*256 + i*128 + p
        out = out.reshape(4, 2, 128, 4, 2, 128)           # ch i p quad ih col
        out = out.transpose(2, 3, 0, 4, 1, 5)             # p quad ch ih i col
        # dram layout [128, ch, i, 1024-cols(quad,ih,128)]
        out = out.transpose(0, 2, 4, 1, 3, 5)             # p ch i quad ih col
        return np.ascontiguousarray(
            out.reshape(128, 4, 2, 1024)).astype(e4).reshape(128, -1)

    def v_perm(wT):
        # plain col order; rows c -> [ch, i, p]
        out = wT.reshape(4, 2, 128, 1024).transpose(2, 0, 1, 3)
        return np.ascontiguousarray(
            out.reshape(128, 4, 2, 1024)).astype(e4).reshape(128, -1)

    wqT = np.ascontiguousarray(w_q.T) * WS
    wkT = np.ascontiguousarray(w_k.T) * WS
    wvT = np.ascontiguousarray(w_v.T) * WS
    # wo tile [p, hp, e] = w_o.T[hp*128 + p, e]
    woT = np.ascontiguousarray(
        w_o.T.reshape(8, 128, 1024).transpose(1, 0, 2)).astype(bf16)
    return {
        "wq": qk_perm(wqT), "wk": qk_perm(wkT), "wv": v_perm(wvT),
        "wo": woT.reshape(128, -1),
    }


def kernel(x, w_q, w_k, w_v, w_o, b_o, ln_g, ln_b):
    import ml_dtypes
    bf16 = ml_dtypes.bfloat16
    e4 = ml_dtypes.float8_e4m3

    x = np.asarray(x, dtype=np.float32)
    w_q = np.asarray(w_q, dtype=np.float32)
    w_k = np.asarray(w_k, dtype=np.float32)
    w_v = np.asarray(w_v, dtype=np.float32)
    w_o = np.asarray(w_o, dtype=np.float32)
    b_o = np.asarray(b_o, dtype=np.float32)
    ln_g = np.asarray(ln_g, dtype=np.float32)
    ln_b = np.asarray(ln_b, dtype=np.float32)

    apply_gb = not (np.all(ln_g == 1.0) and np.all(ln_b == 0.0))
    apply_bias = bool(np.any(b_o != 0.0))
    key = (apply_gb, apply_bias)
    if key not in _CACHE:
        _CACHE[key] = build(apply_gb, apply_bias)
    nc = _CACHE[key]

    shared = _prep_shared(w_q, w_k, w_v, w_o)
    gb = np.stack([ln_g, ln_b]).astype(np.float32)
    ident_np = np.eye(128, dtype=np.float32).astype(bf16)
    bo = np.ascontiguousarray(b_o.reshape(1, D))

    in_maps = []
    for c in range(N_CORES):
        b = c // 2
        half = c % 2
        xb = x[b]
        if half == 1:
            xb = np.roll(xb, -TOK, axis=0)
        # xdr[p, ch, i, t] = xb[t, ch*256 + i*128 + p]
        xdr = xb.T.reshape(4, 2, 128, S).transpose(2, 0, 1, 3)
        xdr = np.ascontiguousarray(xdr).astype(e4).reshape(128, -1)
        xmy = np.ascontiguousarray(xb[0:TOK]).astype(bf16)
        in_maps.append({
            "xdr": xdr, "xmy": xmy, "bo": bo, "gb": gb,
            "ident": ident_np, **shared,
        })

    res = bass_utils.run_bass_kernel_spmd(nc, in_maps,
                                          core_ids=list(range(N_CORES)))
    y = np.stack([res.results[c]["y"] for c in range(N_CORES)])
    return y.reshape(B, S, D)
